# revision 12
# baseline (speedup 1.0000x reference)
"""Trainium2 Bass kernel for nn_Decoder (DDSP-style decoder) — redesigned.

Data-parallel over batch (32 -> 4 per core x 8 cores). Harmonic synthesis
uses per-frame (bands 0-1) / per-32-sample-subframe quadratic (bands 2-5)
phase bases so every chunk matmul's magnitudes stay small enough for the
fast float32r PE path. Grid coefficients are produced directly in
[frame, (batch,osc)] layout by operand-swapped matmuls and reorganized into
per-chunk lhsT stacks by constant permutation matmuls (no DRAM scratch, no
gather DMAs). The noise branch is real-DFT basis matmuls in float32r on
host-transposed noise.
"""
import numpy as np
import sys

sys.path.insert(0, "/opt/trn_rl_repo")

from concourse import bacc, mybir  # noqa: E402
from concourse.tile import TileContext  # noqa: E402
from concourse.bass_utils import run_bass_kernel_spmd  # noqa: E402

F32 = mybir.dt.float32
F32R = mybir.dt.float32r
BF16 = mybir.dt.bfloat16
ALU = mybir.AluOpType
BAND_SIZES = [512, 1024, 2048, 4096, 8192, 16384]
ADJUST = {512: 0.05, 1024: 0.03, 2048: 0.05, 4096: 0.25, 8192: 1.0, 16384: 20.0}
B, C, N_OSC, NNF = 32, 64, 32, 64
NCORE = 8
BL = B // NCORE
FR = BL * NNF
MAGIC = float(1.5 * 2 ** 23)
TWO_PI = float(2 * np.pi)
TOTAL = 2 * sum(BAND_SIZES)

SPF = [bs // NNF for bs in BAND_SIZES]            # 8..256
NOFF = np.concatenate([[0], np.cumsum(SPF)]).astype(int)   # noise col offsets
NCH = [bs // 512 for bs in BAND_SIZES]            # 1,2,4,8,16,32
CHUNK_BASE = np.concatenate([[0], np.cumsum(NCH)]).astype(int)
HARM_OFF = {}
NZ_OFF = {}
_off = 0
for _k, _bs in enumerate(BAND_SIZES):
    HARM_OFF[_k] = _off
    NZ_OFF[_k] = _off + _bs
    _off += 2 * _bs

_nc_cache = {}

W64_ORDER = ([f'up{i}d{dd}' for i in range(3) for dd in range(3)]
             + [f'find{dd}' for dd in range(3)]
             + [w for k in range(6) for w in
                [f't{k}0', f't{k}1', f't{k}2', f't{k}3', f'bf{k}', f'nup{k}']]
             + ['ident'])
W64_IDX = {n: i for i, n in enumerate(W64_ORDER)}
B64_ORDER = ([f'up{i}' for i in range(3)] + ['fin']
             + [b for k in range(6) for b in
                [f't{k}0', f't{k}1', f't{k}2', f't{k}3', f'bf{k}', f'nup{k}']])
B64_IDX = {n: i for i, n in enumerate(B64_ORDER)}


# ---------------------------------------------------------------- host math
def _band_L2(bs):
    t = np.arange(bs)
    pos = (t + 0.5) * (32.0 / bs) - 0.5
    lo = np.clip(np.floor(pos).astype(int), 0, 31)
    hi = np.clip(lo + 1, 0, 31)
    w = np.clip(pos - lo, 0.0, 1.0)
    L2 = np.zeros((32, bs))
    np.add.at(L2, (lo, t), 1.0 - w)
    np.add.at(L2, (hi, t), w)
    return L2


def _interp_vecs(u):
    r = np.arange(u)
    f = (r + 0.5) / u - 0.5
    gm = np.where(r < u // 2, -f, 0.0)
    g0 = np.where(r < u // 2, 1 + f, 1 - f)
    gp = np.where(r >= u // 2, f, 0.0)
    return gm, g0, gp


def _frame_phase_pi(u):
    Pi = np.zeros((64, 128))
    nq = 512 // u
    nslot = 1 if u == 16 else 2
    for cs in range(nslot):
        for g in range(4):
            for q in range(nq):
                row = cs * 64 + g * nq + q
                f = cs * nq + q
                if g == 0:
                    Pi[max(f - 1, 0), row] = 1.0
                elif g == 1:
                    Pi[f, row] = 1.0
                elif g == 2:
                    Pi[min(f + 1, 31), row] = 1.0
                elif f > 0:
                    Pi[32 + f - 1, row] = 1.0
    return Pi.astype(np.float32)


def _frame_phase_basis(u):
    gm, g0, gp = _interp_vecs(u)
    Gm, G0, Gp = np.cumsum(gm), np.cumsum(g0), np.cumsum(gp)
    nq = 512 // u
    bas = np.zeros((128, 512))
    nrep = 1 if u == 16 else 2
    for rep in range(nrep):
        for q in range(nq):
            cols = slice(q * u, (q + 1) * u)
            bas[rep * 64 + 0 * nq + q, cols] = Gm
            bas[rep * 64 + 1 * nq + q, cols] = G0
            bas[rep * 64 + 2 * nq + q, cols] = Gp
            bas[rep * 64 + 3 * nq + q, cols] = 1.0
    return bas.astype(np.float32)


def _frame_amp_pi(u):
    nq = 512 // u
    if u == 16:
        Pi = np.zeros((32, 96))
        for g in range(3):
            for q in range(nq):
                src = max(q - 1, 0) if g == 0 else (q if g == 1 else min(q + 1, 31))
                Pi[src, g * nq + q] = 1.0
    else:
        Pi = np.zeros((32, 128))
        for cs in range(2):
            for g in range(3):
                for q in range(nq):
                    f = cs * nq + q
                    src = max(f - 1, 0) if g == 0 else (f if g == 1 else min(f + 1, 31))
                    Pi[src, cs * 64 + g * nq + q] = 1.0
    return Pi.astype(np.float32)


def _frame_amp_basis(u, adj):
    gm, g0, gp = _interp_vecs(u)
    inv = 1.0 / adj
    nq = 512 // u
    rows = 96 if u == 16 else 112
    bas = np.zeros((rows, 512))
    nrep = 1 if u == 16 else 2
    for rep in range(nrep):
        for q in range(nq):
            cols = slice(q * u, (q + 1) * u)
            bas[rep * 64 + 0 * nq + q, cols] = gm * inv
            bas[rep * 64 + 1 * nq + q, cols] = g0 * inv
            bas[rep * 64 + 2 * nq + q, cols] = gp * inv
    return bas.astype(np.float32)


def _sub_phase_pis(bs):
    u = bs // 32
    L2 = _band_L2(bs)
    CW = np.zeros_like(L2)
    for F in range(32):
        cols = slice(F * u, (F + 1) * u)
        CW[:, cols] = np.cumsum(L2[:, cols], axis=1)
    pis = []
    for j in range(bs // 1024):
        Pi = np.zeros((64, 128))
        for par in range(2):
            c = 2 * j + par
            for s in range(16):
                t0 = 512 * c + 32 * s
                F = t0 // u
                if t0 % u != 0:
                    Pi[0:32, par * 64 + s] = CW[:, t0 - 1]
                if F >= 1:
                    Pi[32 + F - 1, par * 64 + s] = 1.0
                Pi[0:32, par * 64 + 16 + s] = L2[:, t0]
                Pi[0:32, par * 64 + 32 + s] = L2[:, t0 + 1] - L2[:, t0]
        pis.append(Pi.astype(np.float32))
    return pis


def _sub_phase_basis():
    bas = np.zeros((112, 512))
    i = np.arange(32)
    for rep in range(2):
        for s in range(16):
            cols = slice(32 * s, 32 * (s + 1))
            bas[rep * 64 + s, cols] = 1.0
            bas[rep * 64 + 16 + s, cols] = i + 1
            bas[rep * 64 + 32 + s, cols] = i * (i + 1) / 2.0
    return bas.astype(np.float32)


def _sub_amp_pis(bs, adj):
    L2 = _band_L2(bs)
    inv = 1.0 / adj
    nch = bs // 512
    pis = []
    for j in range((nch + 2) // 3):
        Pi = np.zeros((32, 96))
        for m in range(3):
            c = 3 * j + m
            if c >= nch:
                break
            for s in range(16):
                t0 = 512 * c + 32 * s
                Pi[:, m * 32 + s] = L2[:, t0] * inv
                Pi[:, m * 32 + 16 + s] = (L2[:, t0 + 1] - L2[:, t0]) * inv
        pis.append(Pi.astype(np.float32))
    return pis


def _sub_amp_basis():
    bas = np.zeros((96, 512))
    i = np.arange(32)
    for rep in range(3):
        for s in range(16):
            cols = slice(32 * s, 32 * (s + 1))
            bas[rep * 32 + s, cols] = 1.0
            bas[rep * 32 + 16 + s, cols] = i
    return bas.astype(np.float32)


def _tridiag_M(u):
    M = np.zeros((32, 32))
    for f in range(32):
        M[max(f - 1, 0), f] += u / 8.0
        M[min(f + 1, 31), f] += u / 8.0
        M[f, f] += 3.0 * u / 4.0
    return M.astype(np.float32)


def _band_fir(bs):
    spf = bs // NNF
    nc_ = spf // 2 + 1
    t = np.arange(spf)
    j_re = np.arange(nc_)
    j_im = np.arange(1, nc_ - 1)
    FT = np.concatenate([np.cos(2 * np.pi * np.outer(t, j_re) / spf),
                         -np.sin(2 * np.pi * np.outer(t, j_im) / spf)], axis=1)
    w = np.full(nc_, 2.0)
    w[0] = 1.0
    w[-1] = 1.0
    IR = np.concatenate([
        (w[:, None] * np.cos(2 * np.pi * np.outer(j_re, t) / spf)) / spf,
        (-2.0 * np.sin(2 * np.pi * np.outer(j_im, t) / spf)) / spf,
    ], axis=0) / ADJUST[bs]
    return FT.astype(np.float32), IR.astype(np.float32)


def _build_U(n):
    eye = np.eye(n)
    spec = np.fft.rfft(eye, axis=-1)
    spec = np.pad(spec, ((0, 0), (0, n + 1 - spec.shape[-1])))
    return np.fft.irfft(spec, n=2 * n, axis=-1) * 2


def _mega_entries():
    ents = [('wlin', C + 1, 4 * C), ('ubd4', BL * 4, BL * 8), ('ubd8', BL * 8, BL * 16),
            ('ubd16', BL * 16, BL * 32), ('w64', C, len(W64_ORDER) * C),
            ('bias64', C, len(B64_ORDER)), ('ident128', 128, 128),
            ('selstrip', 128, 256), ('negI', 128, 128), ('ut', 32, 32)]
    for k in range(6):
        ents.append((f'wfrq{k}', C + 1, N_OSC))
        ents.append((f'wamp{k}', C + 1, N_OSC))
        ents.append((f'M{k}', 32, 32))
    ents += [('pi0', 64, 128), ('pia0', 32, 96), ('pi1', 64, 128), ('pia1', 32, 128)]
    for k in (2, 3, 4, 5):
        nch = NCH[k]
        for j in range(nch // 2):
            ents.append((f'pip{k}_{j}', 64, 128))
        for j in range((nch + 2) // 3):
            ents.append((f'piam{k}_{j}', 32, 96))
    ents += [('bas0', 128, 512), ('bas1', 128, 512), ('basS', 112, 512),
             ('basA0', 96, 512), ('basA1', 112, 512), ('basAS', 96, 512)]
    for k in range(6):
        spf = SPF[k]
        nc_ = spf // 2 + 1
        if k < 5:
            ents.append((f'wc{k}', C + 1, spf))
            ents.append((f'ft{k}', spf, spf))
            ents.append((f'ir{k}', spf, spf))
        else:
            ents.append(('wc5a', C + 1, 128))
            ents.append(('wc5b', C + 1, 128))
            ents.append(('ft5_0', 128, 256))
            ents.append(('ft5_1', 128, 256))
            ents.append(('ir5_0', 128, 256))
            ents.append(('ir5_1', 128, 256))
    off = {}
    o = 0
    for name, r, cd in ents:
        off[name] = (r, o, cd)
        o += cd
    return off, o


MEGA_OFF, MEGA_COLS = _mega_entries()


def _build_shared(inp):
    c = {}
    wl = np.zeros((4, C + 1, C), np.float32)
    for t in range(4):
        wl[t, :C] = inp['up_lin_w'][:, t::4]
        wl[t, C] = inp['up_lin_b'][t::4]
    c['wlin'] = wl.transpose(1, 0, 2).reshape(C + 1, 4 * C)
    for n in (4, 8, 16):
        U = _build_U(n)
        ub = np.zeros((BL * n, BL * 2 * n), np.float32)
        for b in range(BL):
            ub[b * n:(b + 1) * n, b * 2 * n:(b + 1) * 2 * n] = U
        c[f'ubd{n}'] = ub

    w64 = np.zeros((C, len(W64_ORDER) * C), np.float32)

    def put64(name, m):
        i = W64_IDX[name]
        w64[:, i * C:(i + 1) * C] = m

    for i in range(3):
        for dd in range(3):
            put64(f'up{i}d{dd}', inp['up_conv_w'][i, :, :, dd].T)
    for dd in range(3):
        put64(f'find{dd}', inp['up_final_w'][:, :, dd].T)
    for k in range(6):
        for j in range(4):
            put64(f't{k}{j}', inp['t_w'][k, j].T + np.eye(C, dtype=np.float32))
        put64(f'bf{k}', inp['band_final_w'][k].T)
        put64(f'nup{k}', inp['noise_up_w'][k].T)
    put64('ident', np.eye(C))
    c['w64'] = w64

    b64 = np.zeros((C, len(B64_ORDER)), np.float32)
    for i in range(3):
        b64[:, B64_IDX[f'up{i}']] = inp['up_conv_b'][i]
    b64[:, B64_IDX['fin']] = inp['up_final_b']
    for k in range(6):
        for j in range(4):
            b64[:, B64_IDX[f't{k}{j}']] = inp['t_b'][k, j]
        b64[:, B64_IDX[f'bf{k}']] = inp['band_final_b'][k]
        b64[:, B64_IDX[f'nup{k}']] = inp['noise_up_b'][k]
    c['bias64'] = b64
    c['ident128'] = np.eye(128, dtype=np.float32)

    sel = np.zeros((128, 256), np.float32)
    for b in range(BL):
        sel[b * N_OSC:(b + 1) * N_OSC, 128 + b] = 1.0
    c['selstrip'] = sel
    c['negI'] = (-np.eye(128)).astype(np.float32)
    c['ut'] = np.triu(np.ones((32, 32))).astype(np.float32)

    for k, bs in enumerate(BAND_SIZES):
        u = bs // 32
        wf = np.zeros((C + 1, N_OSC), np.float32)
        wf[:C] = inp['osc_freq_w'][k].T
        wf[C] = inp['osc_freq_b'][k]
        c[f'wfrq{k}'] = wf
        wa = np.zeros((C + 1, N_OSC), np.float32)
        wa[:C] = inp['osc_amp_w'][k].T
        wa[C] = inp['osc_amp_b'][k]
        c[f'wamp{k}'] = wa
        c[f'M{k}'] = _tridiag_M(u)

    c['pi0'] = _frame_phase_pi(16)
    c['pia0'] = _frame_amp_pi(16)
    c['pi1'] = _frame_phase_pi(32)
    c['pia1'] = _frame_amp_pi(32)
    for k in (2, 3, 4, 5):
        bs = BAND_SIZES[k]
        for j, Pi in enumerate(_sub_phase_pis(bs)):
            c[f'pip{k}_{j}'] = Pi
        for j, Pi in enumerate(_sub_amp_pis(bs, ADJUST[bs])):
            c[f'piam{k}_{j}'] = Pi
    c['bas0'] = _frame_phase_basis(16)
    c['bas1'] = _frame_phase_basis(32)
    c['basS'] = _sub_phase_basis()
    c['basA0'] = _frame_amp_basis(16, ADJUST[512])
    c['basA1'] = _frame_amp_basis(32, ADJUST[1024])
    c['basAS'] = _sub_amp_basis()

    for k, bs in enumerate(BAND_SIZES):
        spf = SPF[k]
        nc_ = spf // 2 + 1
        wcf = np.zeros((C + 1, spf), np.float32)
        wc = np.zeros((C + 1, nc_), np.float32)
        wc[:C] = inp[f'noise_coeff_w_{k}'].T
        wc[C] = inp[f'noise_coeff_b_{k}']
        if k == 0:
            wc[:, 1:] = 0.0
        wcf[:, 0:nc_] = wc
        wcf[:, nc_:spf] = wc[:, 1:nc_ - 1]
        FT, IR = _band_fir(bs)
        if k < 5:
            c[f'wc{k}'] = wcf
            c[f'ft{k}'] = FT
            c[f'ir{k}'] = IR
        else:
            c['wc5a'] = wcf[:, 0:128]
            c['wc5b'] = wcf[:, 128:256]
            c['ft5_0'] = FT[0:128]
            c['ft5_1'] = FT[128:256]
            c['ir5_0'] = IR[0:128]
            c['ir5_1'] = IR[128:256]

    mega = np.zeros((128, MEGA_COLS), np.float32)
    for name, (r, o, cd) in MEGA_OFF.items():
        mega[0:r, o:o + cd] = c[name]
    return {'mega': mega}


# ---------------------------------------------------------------- bass build
def _build_nc():
    nc = bacc.Bacc('TRN2', num_devices=NCORE)
    AF = mybir.ActivationFunctionType

    d = {}
    d['xT'] = nc.dram_tensor("xT", [C + 1, BL], F32, kind="ExternalInput")
    d['mega'] = nc.dram_tensor("mega", [128, MEGA_COLS], F32, kind="ExternalInput")
    for k in range(6):
        d[f'noiseT{k}'] = nc.dram_tensor(f"noiseT{k}", [SPF[k], FR], F32,
                                         kind="ExternalInput")
    harm_d = nc.dram_tensor("harm", [4 * 63, 512], F32, kind="ExternalOutput")
    nz_d = nc.dram_tensor("nz", [FR, int(NOFF[6])], F32, kind="ExternalOutput")


    with TileContext(nc) as tc:
        with tc.tile_pool(name="const", bufs=1) as cp, \
             tc.tile_pool(name="work", bufs=2) as wp, \
             tc.tile_pool(name="hot", bufs=3) as hot, \
             tc.tile_pool(name="phF", bufs=3, space="PSUM") as phF, \
             tc.tile_pool(name="phA", bufs=3, space="PSUM") as phA, \
             tc.tile_pool(name="phH", bufs=2, space="PSUM") as phH:

            def psF(p0, f0):
                t = phF.tile([128, 512], F32, tag="phF", name="psF")
                return t[0:p0, 0:f0]

            def psA(p0, f0):
                t = phA.tile([128, 512], F32, tag="phA", name="psA")
                return t[0:p0, 0:f0]

            def psH(p0, f0):
                t = phH.tile([128, 512], F32, tag="phH", name="psH")
                return t[0:p0, 0:f0]

            mega = cp.tile([128, MEGA_COLS], F32, tag="mega")
            _nsplit = 4
            _cut = [MEGA_COLS * i // _nsplit for i in range(_nsplit + 1)]
            for _i in range(_nsplit):
                nc.gpsimd.dma_start(out=mega[:, _cut[_i]:_cut[_i + 1]],
                                    in_=d['mega'][:, _cut[_i]:_cut[_i + 1]])

            ct = {}
            for name, (r, o, cd) in MEGA_OFF.items():
                ct[name] = mega[0:r, o:o + cd]
            for name in ('selstrip', 'negI', 'basA0', 'basA1', 'basAS'):
                r, o, cd = MEGA_OFF[name]
                t = cp.tile([r, cd], BF16, tag=f"bf_{name}")
                nc.gpsimd.dma_start(out=t, in_=d['mega'][0:r, o:o + cd])
                ct[f'{name}_bf'] = t
            for name in ('bas0', 'bas1', 'basS'):
                r, o, cd = MEGA_OFF[name]
                t = cp.tile([r, cd], F32R, tag=f"r_{name}")
                nc.gpsimd.dma_start(out=t, in_=d['mega'][0:r, o:o + cd])
                ct[f'{name}_r'] = t

            def w64s(name):
                i = W64_IDX[name]
                return ct['w64'][:, i * C:(i + 1) * C]

            def b64s(name):
                return ct['bias64'][:, B64_IDX[name]:B64_IDX[name] + 1]

            ident64 = w64s('ident')

            xT = cp.tile([C + 1, BL], F32, tag="xT")
            nc.sync.dma_start(out=xT, in_=d['xT'][:, :])

            # ---------------- frontend (as baseline)
            h = wp.tile([C, 16], F32, tag="h0")
            for t in range(4):
                pt = psF(C, BL)
                nc.tensor.matmul(out=pt, lhsT=ct['wlin'][:, t * C:(t + 1) * C],
                                 rhs=xT, start=True, stop=True)
                nc.vector.tensor_copy(out=h.rearrange("c (b t) -> c b t", t=4)[:, :, t],
                                      in_=pt)
            for i, n in enumerate((4, 8, 16)):
                pt1 = psF(BL * n, C)
                nc.tensor.transpose(out=pt1, in_=h, identity=ident64)
                t1 = wp.tile([BL * n, C], F32, tag=f"fe_t1_{i}")
                nc.scalar.copy(out=t1, in_=pt1)
                pt2 = psF(BL * 2 * n, C)
                nc.tensor.matmul(out=pt2, lhsT=ct[f'ubd{n}'], rhs=t1, start=True,
                                 stop=True)
                t2 = wp.tile([BL * 2 * n, C], F32, tag=f"fe_t2_{i}")
                nc.scalar.copy(out=t2, in_=pt2)
                pt3 = psF(C, BL * 2 * n)
                nc.tensor.transpose(out=pt3, in_=t2,
                                    identity=ct['ident128'][0:BL * 2 * n, 0:BL * 2 * n])
                hu = wp.tile([C, BL * 2 * n], F32, tag=f"fe_hu_{i}")
                nc.scalar.copy(out=hu, in_=pt3)
                m = 2 * n
                hu3 = hu.rearrange("c (b t) -> c b t", b=BL)
                pc = psF(C, BL * m).rearrange("c (b t) -> c b t", b=BL)
                nc.tensor.matmul(out=pc[:, :, :], lhsT=w64s(f'up{i}d1'), rhs=hu3[:, :, :],
                                 start=True, stop=False)
                nc.tensor.matmul(out=pc[:, :, 1:m], lhsT=w64s(f'up{i}d0'),
                                 rhs=hu3[:, :, 0:m - 1], start=False, stop=False)
                nc.tensor.matmul(out=pc[:, :, 0:m - 1], lhsT=w64s(f'up{i}d2'),
                                 rhs=hu3[:, :, 1:m], start=False, stop=True)
                h = wp.tile([C, BL * m], F32, tag=f"fe_h_{i}")
                nc.scalar.activation(out=h.rearrange("c (b t) -> c b t", b=BL), in_=pc,
                                     func=AF.Prelu, bias=b64s(f'up{i}'), scale=1.0,
                                     alpha=0.2)
            h3 = h.rearrange("c (b t) -> c b t", b=BL)
            pf = psF(C, BL * 32).rearrange("c (b t) -> c b t", b=BL)
            nc.tensor.matmul(out=pf[:, :, :], lhsT=w64s('find1'), rhs=h3[:, :, :],
                             start=True, stop=False)
            nc.tensor.matmul(out=pf[:, :, 1:32], lhsT=w64s('find0'), rhs=h3[:, :, 0:31],
                             start=False, stop=False)
            nc.tensor.matmul(out=pf[:, :, 0:31], lhsT=w64s('find2'), rhs=h3[:, :, 1:32],
                             start=False, stop=True)
            hfin = cp.tile([C, 128], F32, tag="hfin")
            nc.scalar.activation(out=hfin.rearrange("c (b t) -> c b t", b=BL), in_=pf,
                                 func=AF.Identity, bias=b64s('fin'), scale=1.0)

            # ---------------- per-band setup
            nTs_all = {}
            for k in range(6):
                if k < 5:
                    nT = wp.tile([SPF[k], FR], F32, tag=f"nT{k}", name="nT")
                    nc.sync.dma_start(out=nT, in_=d[f'noiseT{k}'][:, :])
                    nTs_all[k] = (nT,)
                else:
                    nT0 = wp.tile([128, FR], F32, tag="nT50")
                    nT1 = wp.tile([128, FR], F32, tag="nT51")
                    nc.sync.dma_start(out=nT0, in_=d['noiseT5'][0:128, :])
                    nc.sync.dma_start(out=nT1, in_=d['noiseT5'][128:256, :])
                    nTs_all[5] = (nT0, nT1)
            stacks = {}
            astacks = {}
            for k, bs in enumerate(BAND_SIZES):
                u = bs // 32
                nch = NCH[k]
                spf = SPF[k]
                nc_ = spf // 2 + 1
                lf = 0.05 if bs == 512 else 0.01

                z = hfin
                for j in range(4):
                    pz = psF(C, 128)
                    nc.tensor.matmul(out=pz, lhsT=w64s(f't{k}{j}'), rhs=z,
                                     start=True, stop=True)
                    z = wp.tile([C, 128], F32, tag=f"z_{j % 2}")
                    nc.scalar.activation(out=z, in_=pz, func=AF.Prelu,
                                         bias=b64s(f't{k}{j}'), scale=1.0, alpha=0.2)
                pz = psF(C, 128)
                nc.tensor.matmul(out=pz, lhsT=w64s(f'bf{k}'), rhs=z, start=True,
                                 stop=True)
                zfa = wp.tile([C + 1, 128], F32, tag="zfa")
                nc.scalar.activation(out=zfa[0:C, :], in_=pz, func=AF.Identity,
                                     bias=b64s(f'bf{k}'), scale=1.0)
                nc.vector.memset(zfa[C:C + 1, :], 1.0)

                # grids: [32 f, 128 (b,o)]
                pgF = psH(N_OSC, 128)
                for b in range(BL):
                    nc.tensor.matmul(out=pgF[:, 32 * b:32 * (b + 1)],
                                     lhsT=zfa[:, 32 * b:32 * (b + 1)],
                                     rhs=ct[f'wfrq{k}'], start=True, stop=True)
                sig = wp.tile([N_OSC, 128], F32, tag="sig")
                nc.scalar.activation(out=sig, in_=pgF, func=AF.Sigmoid, scale=1.0)
                src = wp.tile([64, 128], F32, tag="src")
                nc.vector.tensor_scalar(out=src[0:32, :], in0=sig,
                                        scalar1=float((1.0 - lf) / 2.0),
                                        scalar2=float(lf / 2.0),
                                        op0=ALU.mult, op1=ALU.add)
                pgA = psH(N_OSC, 128)
                for b in range(BL):
                    nc.tensor.matmul(out=pgA[:, 32 * b:32 * (b + 1)],
                                     lhsT=zfa[:, 32 * b:32 * (b + 1)],
                                     rhs=ct[f'wamp{k}'], start=True, stop=True)
                ampg = wp.tile([N_OSC, 128], F32, tag="ampg")
                nc.scalar.activation(out=ampg, in_=pgA, func=AF.Abs, scale=1.0)

                # frame carries
                pS = psH(N_OSC, 128)
                nc.tensor.matmul(out=pS, lhsT=ct[f'M{k}'], rhs=src[0:32, :],
                                 start=True, stop=True)
                rndS = wp.tile([N_OSC, 128], F32, tag="rndS")
                nc.vector.tensor_scalar(out=rndS, in0=pS, scalar1=MAGIC, scalar2=MAGIC,
                                        op0=ALU.add, op1=ALU.subtract)
                Sr = wp.tile([N_OSC, 128], F32, tag="Sr")
                nc.vector.tensor_tensor(out=Sr, in0=pS, in1=rndS, op=ALU.subtract)
                pP = psH(N_OSC, 128)
                nc.tensor.matmul(out=pP, lhsT=ct['ut'], rhs=Sr, start=True, stop=True)
                rndP = wp.tile([N_OSC, 128], F32, tag="rndP")
                nc.vector.tensor_scalar(out=rndP, in0=pP, scalar1=MAGIC, scalar2=MAGIC,
                                        op0=ALU.add, op1=ALU.subtract)
                nc.vector.tensor_tensor(out=src[32:64, :], in0=pP, in1=rndP,
                                        op=ALU.subtract)

                # phase stacks
                if k == 0:
                    stk = cp.tile([128, 128], F32, tag="stk0")
                    pb = psF(128, 128)
                    nc.tensor.matmul(out=pb, lhsT=ct['pi0'], rhs=src, start=True,
                                     stop=True)
                    nc.scalar.copy(out=stk, in_=pb)
                elif k == 1:
                    stk = cp.tile([128, 128], F32, tag="stk1")
                    pb = psF(128, 128)
                    nc.tensor.matmul(out=pb, lhsT=ct['pi1'], rhs=src, start=True,
                                     stop=True)
                    nc.scalar.copy(out=stk, in_=pb)
                else:
                    stk = cp.tile([128, (nch // 2) * 128], F32, tag=f"stk{k}")
                    for j in range(nch // 2):
                        pb = psF(128, 128)
                        nc.tensor.matmul(out=pb, lhsT=ct[f'pip{k}_{j}'], rhs=src,
                                         start=True, stop=True)
                        rnd = wp.tile([128, 128], F32, tag="rndB")
                        nc.vector.tensor_scalar(out=rnd, in0=pb, scalar1=MAGIC,
                                                scalar2=MAGIC, op0=ALU.add,
                                                op1=ALU.subtract)
                        nc.vector.tensor_tensor(out=stk[:, 128 * j:128 * (j + 1)],
                                                in0=pb, in1=rnd, op=ALU.subtract)
                if k == 0:
                    stkr = cp.tile([128, 128], F32R, tag="stkr0", name="stkr")
                elif k == 1:
                    stkr = cp.tile([128, 128], F32R, tag="stkr1", name="stkr")
                else:
                    stkr = cp.tile([128, (nch // 2) * 128], F32R, tag=f"stkr{k}",
                                   name="stkr")
                nc.gpsimd.dma_start(out=stkr, in_=stk)
                stacks[k] = stkr

                # amp stacks (bf16)
                if k == 0:
                    ast = cp.tile([96, 128], BF16, tag="ast0")
                    pb = psF(96, 128)
                    nc.tensor.matmul(out=pb, lhsT=ct['pia0'], rhs=ampg, start=True,
                                     stop=True)
                    nc.scalar.copy(out=ast, in_=pb)
                elif k == 1:
                    ast = cp.tile([128, 128], BF16, tag="ast1")
                    pb = psF(128, 128)
                    nc.tensor.matmul(out=pb, lhsT=ct['pia1'], rhs=ampg, start=True,
                                     stop=True)
                    nc.scalar.copy(out=ast, in_=pb)
                else:
                    nblk = (nch + 2) // 3
                    ast = cp.tile([96, nblk * 128], BF16, tag=f"ast{k}")
                    for j in range(nblk):
                        pb = psF(96, 128)
                        nc.tensor.matmul(out=pb, lhsT=ct[f'piam{k}_{j}'], rhs=ampg,
                                         start=True, stop=True)
                        nc.scalar.copy(out=ast[:, 128 * j:128 * (j + 1)], in_=pb)
                astacks[k] = ast

                # ---------------- noise branch
                zf3 = zfa[0:C, :].rearrange("c (b t) -> c b t", b=BL)
                zrep = zf3.unsqueeze(-1).broadcast_to([C, BL, 32, 2])
                pn = psA(C, FR)
                nc.tensor.matmul(out=pn, lhsT=w64s(f'nup{k}'), rhs=zrep,
                                 start=True, stop=True)
                naug = wp.tile([C + 1, FR], F32, tag="naug")
                nc.scalar.activation(out=naug[0:C, :], in_=pn, func=AF.Prelu,
                                     bias=b64s(f'nup{k}'), scale=1.0, alpha=0.2)
                nc.vector.memset(naug[C:C + 1, :], 1.0)

                if k < 5:
                    nT = nTs_all[k][0]
                    pcA = psH(spf, FR)
                    nc.tensor.matmul(out=pcA, lhsT=ct[f'wc{k}'], rhs=naug,
                                     start=True, stop=True)
                    chat = wp.tile([spf, FR], F32, tag="chat")
                    nc.scalar.copy(out=chat, in_=pcA)
                    psp = psH(spf, FR)
                    nc.tensor.matmul(out=psp, lhsT=ct[f'ft{k}'], rhs=nT,
                                     start=True, stop=True)
                    sA = wp.tile([spf, FR], F32, tag="sA")
                    nc.vector.tensor_tensor(out=sA, in0=chat, in1=psp, op=ALU.mult)
                    sAs = [sA]
                else:
                    nT0, nT1 = nTs_all[5]
                    sAs = []
                    for half, wch in ((0, 'wc5a'), (1, 'wc5b')):
                        pcA = psH(128, FR)
                        nc.tensor.matmul(out=pcA, lhsT=ct[wch], rhs=naug,
                                         start=True, stop=True)
                        chat = wp.tile([128, FR], F32, tag=f"chat5{half}")
                        nc.scalar.copy(out=chat, in_=pcA)
                        psp = psH(128, FR)
                        nc.tensor.matmul(out=psp,
                                         lhsT=ct['ft5_0'][:, 128 * half:128 * (half + 1)],
                                         rhs=nT0, start=True, stop=False)
                        nc.tensor.matmul(out=psp,
                                         lhsT=ct['ft5_1'][:, 128 * half:128 * (half + 1)],
                                         rhs=nT1, start=False, stop=True)
                        sA = wp.tile([128, FR], F32, tag=f"sA5{half}")
                        nc.vector.tensor_tensor(out=sA, in0=chat, in1=psp, op=ALU.mult)
                        sAs.append(sA)

                for fg in range(2):
                    pnz = psA(128, spf)
                    if k < 5:
                        nc.tensor.matmul(out=pnz,
                                         lhsT=sAs[0][:, 128 * fg:128 * (fg + 1)],
                                         rhs=ct[f'ir{k}'], start=True, stop=True)
                    else:
                        nc.tensor.matmul(out=pnz,
                                         lhsT=sAs[0][:, 128 * fg:128 * (fg + 1)],
                                         rhs=ct['ir5_0'], start=True, stop=False)
                        nc.tensor.matmul(out=pnz,
                                         lhsT=sAs[1][:, 128 * fg:128 * (fg + 1)],
                                         rhs=ct['ir5_1'], start=False, stop=True)
                    nzs = wp.tile([128, spf], F32, tag="nzs")
                    nc.scalar.copy(out=nzs, in_=pnz)
                    nc.sync.dma_start(
                        out=nz_d[128 * fg:128 * (fg + 1),
                                 int(NOFF[k]):int(NOFF[k]) + spf],
                        in_=nzs)

            # ---------------- chunk loop (per-chunk, deep software pipeline)
            chunks = []
            for k in range(6):
                nch = NCH[k]
                for gstart in range(0, nch, 8):
                    gs = min(8, nch - gstart)
                    for cc in range(gs):
                        chunks.append((k, gstart + cc, cc, gs))
            Nc = len(chunks)
            state = {}

            def stage_a(i):
                k, c, cc, gs = chunks[i]
                ppt = phF.tile([128, 512], F32, tag="phF", name="ppt")
                if k == 0:
                    nc.tensor.matmul(out=ppt, lhsT=stacks[0], rhs=ct['bas0_r'],
                                     start=True, stop=False)
                elif k == 1:
                    nc.tensor.matmul(out=ppt, lhsT=stacks[1][64 * c:64 * c + 64],
                                     rhs=ct['bas1_r'][64 * c:64 * c + 64],
                                     start=True, stop=False)
                else:
                    j, par = c // 2, c % 2
                    nc.tensor.matmul(
                        out=ppt,
                        lhsT=stacks[k][64 * par:64 * par + 48,
                                       128 * j:128 * (j + 1)],
                        rhs=ct['basS_r'][64 * par:64 * par + 48],
                        start=True, stop=False)
                pat = phA.tile([128, 512], F32, tag="phA", name="pat")
                if k == 0:
                    nc.tensor.matmul(out=pat, lhsT=astacks[0],
                                     rhs=ct['basA0_bf'], start=True, stop=True)
                elif k == 1:
                    nc.tensor.matmul(out=pat, lhsT=astacks[1][64 * c:64 * c + 48],
                                     rhs=ct['basA1_bf'][64 * c:64 * c + 48],
                                     start=True, stop=True)
                else:
                    ja, ma = c // 3, c % 3
                    nc.tensor.matmul(
                        out=pat,
                        lhsT=astacks[k][32 * ma:32 * ma + 32,
                                        128 * ja:128 * (ja + 1)],
                        rhs=ct['basAS_bf'][32 * ma:32 * ma + 32],
                        start=True, stop=True)
                ntile = hot.tile([128, 512], BF16, tag="ntile")
                nc.vector.tensor_scalar(out=ntile, in0=ppt, scalar1=MAGIC,
                                        scalar2=MAGIC, op0=ALU.add,
                                        op1=ALU.subtract)
                if i % 2 == 0:
                    pat_sb = hot.tile([128, 512], BF16, tag="pat_sb")
                    nc.scalar.copy(out=pat_sb, in_=pat)
                else:
                    pat_sb = None
                state[i] = (ppt, pat, ntile, pat_sb)

            def stage_b(i):
                ppt, pat, ntile, pat_sb = state[i]
                nc.tensor.matmul(out=ppt, lhsT=ct['negI_bf'], rhs=ntile,
                                 start=False, stop=True)
                s = hot.tile([128, 512], BF16, tag="sin_t")
                nc.scalar.activation(out=s, in_=ppt, func=AF.Sin, scale=TWO_PI)
                prod = hot.tile([128, 512], BF16, tag="prod_t")
                nc.vector.tensor_tensor(out=prod, in0=s,
                                        in1=pat_sb if pat_sb is not None else pat,
                                        op=ALU.mult)
                state[i] = prod

            hpt_cur = [None]

            def stage_c(i):
                k, c, cc, gs = chunks[i]
                prod = state.pop(i)
                if cc == 0:
                    hpt_cur[0] = phH.tile([128, 512], F32, tag="phH", name="hpt")
                hpt = hpt_cur[0]
                nc.tensor.matmul(out=hpt,
                                 lhsT=ct['selstrip_bf'][:, 128 - 4 * cc:256 - 4 * cc],
                                 rhs=prod, start=(cc == 0), stop=(cc == gs - 1))
                if cc == gs - 1:
                    hsb = wp.tile([32, 512], F32, tag="hsb")
                    nc.scalar.copy(out=hsb[0:4 * gs], in_=hpt[0:4 * gs])
                    g0 = CHUNK_BASE[k] + c - (gs - 1)
                    nc.sync.dma_start(
                        out=harm_d[4 * int(g0):4 * int(g0) + 4 * gs, :],
                        in_=hsb[0:4 * gs])

            for i in range(Nc + 4):
                if i < Nc:
                    stage_a(i)
                if 2 <= i < Nc + 2:
                    stage_b(i - 2)
                if 4 <= i < Nc + 4:
                    stage_c(i - 4)

    nc.finalize()
    return nc


# ---------------------------------------------------------------- host glue
def _prep_inputs(inputs):
    inp = {k: np.asarray(v, np.float32) for k, v in inputs.items()}
    shared = _build_shared(inp)
    in_maps = []
    for core in range(NCORE):
        m = dict(shared)
        sl = slice(core * BL, (core + 1) * BL)
        m['xT'] = np.concatenate([inp['x'][sl].T, np.ones((1, BL), np.float32)],
                                 axis=0)
        for k in range(6):
            nT = inp[f'noise_{k}'][sl].reshape(FR, SPF[k]).T
            m[f'noiseT{k}'] = np.ascontiguousarray(nT, dtype=np.float32)
        in_maps.append(m)
    return in_maps


def kernel(**inputs):
    if 'nc' not in _nc_cache:
        _nc_cache['nc'] = _build_nc()
    nc = _nc_cache['nc']
    in_maps = _prep_inputs(inputs)
    res = run_bass_kernel_spmd(nc, in_maps, list(range(NCORE)))
    out = np.zeros((B, TOTAL), np.float32)
    for core in range(NCORE):
        r = res.results[core]
        harm = np.asarray(r['harm'], np.float32).reshape(63, BL, 512)
        nz = np.asarray(r['nz'], np.float32).reshape(BL, NNF, int(NOFF[6]))
        for k, bs in enumerate(BAND_SIZES):
            nch = NCH[k]
            hb = harm[CHUNK_BASE[k]:CHUNK_BASE[k] + nch]  # [nch, BL, 512]
            hb = hb.transpose(1, 0, 2).reshape(BL, bs)
            out[core * BL:(core + 1) * BL,
                HARM_OFF[k]:HARM_OFF[k] + bs] = hb
            nzb = nz[:, :, int(NOFF[k]):int(NOFF[k]) + SPF[k]].reshape(BL, bs)
            out[core * BL:(core + 1) * BL, NZ_OFF[k]:NZ_OFF[k] + bs] = nzb
    return out.astype(np.float32)


if __name__ == "__main__":
    import reference
    inp = reference.setup_inputs()
    out = kernel(**{k: np.asarray(v) for k, v in inp.items()})
    print("out", out.shape, out.dtype)


# revision 30
# speedup vs baseline: 1.2427x; 1.2427x over previous
"""Trainium2 Bass kernel for nn_Decoder (DDSP-style decoder) — redesigned.

Data-parallel over batch (32 -> 4 per core x 8 cores). Harmonic synthesis
uses per-frame (bands 0-1) / per-32-sample-subframe quadratic (bands 2-5)
phase bases so every chunk matmul's magnitudes stay small enough for the
fast float32r PE path. Grid coefficients are produced directly in
[frame, (batch,osc)] layout by operand-swapped matmuls and reorganized into
per-chunk lhsT stacks by constant permutation matmuls (no DRAM scratch, no
gather DMAs). The noise branch is real-DFT basis matmuls in float32r on
host-transposed noise.
"""
import numpy as np
import sys

sys.path.insert(0, "/opt/trn_rl_repo")

from concourse import bacc, mybir  # noqa: E402
from concourse.tile import TileContext  # noqa: E402
from concourse.bass_utils import run_bass_kernel_spmd  # noqa: E402

F32 = mybir.dt.float32
F32R = mybir.dt.float32r
BF16 = mybir.dt.bfloat16
ALU = mybir.AluOpType
BAND_SIZES = [512, 1024, 2048, 4096, 8192, 16384]
ADJUST = {512: 0.05, 1024: 0.03, 2048: 0.05, 4096: 0.25, 8192: 1.0, 16384: 20.0}
B, C, N_OSC, NNF = 32, 64, 32, 64
NCORE = 8
BL = B // NCORE
FR = BL * NNF
MAGIC = float(1.5 * 2 ** 23)
TWO_PI = float(2 * np.pi)
TOTAL = 2 * sum(BAND_SIZES)

SPF = [bs // NNF for bs in BAND_SIZES]            # 8..256
NOFF = np.concatenate([[0], np.cumsum(SPF)]).astype(int)   # noise col offsets
NCH = [bs // 512 for bs in BAND_SIZES]            # 1,2,4,8,16,32
CHUNK_BASE = np.concatenate([[0], np.cumsum(NCH)]).astype(int)
HARM_OFF = {}
NZ_OFF = {}
_off = 0
for _k, _bs in enumerate(BAND_SIZES):
    HARM_OFF[_k] = _off
    NZ_OFF[_k] = _off + _bs
    _off += 2 * _bs

_nc_cache = {}

W64_ORDER = ([f'up{i}d{dd}' for i in range(3) for dd in range(3)]
             + [f'find{dd}' for dd in range(3)]
             + [w for k in range(6) for w in
                [f't{k}0', f't{k}1', f't{k}2', f't{k}3', f'bf{k}', f'nup{k}']]
             + ['ident'])
W64_IDX = {n: i for i, n in enumerate(W64_ORDER)}
B64_ORDER = ([f'up{i}' for i in range(3)] + ['fin']
             + [b for k in range(6) for b in
                [f't{k}0', f't{k}1', f't{k}2', f't{k}3', f'bf{k}', f'nup{k}']])
B64_IDX = {n: i for i, n in enumerate(B64_ORDER)}


# ---------------------------------------------------------------- host math
def _band_L2(bs):
    t = np.arange(bs)
    pos = (t + 0.5) * (32.0 / bs) - 0.5
    lo = np.clip(np.floor(pos).astype(int), 0, 31)
    hi = np.clip(lo + 1, 0, 31)
    w = np.clip(pos - lo, 0.0, 1.0)
    L2 = np.zeros((32, bs))
    np.add.at(L2, (lo, t), 1.0 - w)
    np.add.at(L2, (hi, t), w)
    return L2


def _interp_vecs(u):
    r = np.arange(u)
    f = (r + 0.5) / u - 0.5
    gm = np.where(r < u // 2, -f, 0.0)
    g0 = np.where(r < u // 2, 1 + f, 1 - f)
    gp = np.where(r >= u // 2, f, 0.0)
    return gm, g0, gp


def _frame_phase_pi(u):
    Pi = np.zeros((64, 128))
    nq = 512 // u
    nslot = 1 if u == 16 else 2
    for cs in range(nslot):
        for g in range(4):
            for q in range(nq):
                row = cs * 64 + g * nq + q
                f = cs * nq + q
                if g == 0:
                    Pi[max(f - 1, 0), row] = 1.0
                elif g == 1:
                    Pi[f, row] = 1.0
                elif g == 2:
                    Pi[min(f + 1, 31), row] = 1.0
                elif f > 0:
                    Pi[32 + f - 1, row] = 1.0
    return Pi.astype(np.float32)


def _frame_phase_basis(u):
    gm, g0, gp = _interp_vecs(u)
    Gm, G0, Gp = np.cumsum(gm), np.cumsum(g0), np.cumsum(gp)
    nq = 512 // u
    bas = np.zeros((128, 512))
    nrep = 1 if u == 16 else 2
    for rep in range(nrep):
        for q in range(nq):
            cols = slice(q * u, (q + 1) * u)
            bas[rep * 64 + 0 * nq + q, cols] = Gm
            bas[rep * 64 + 1 * nq + q, cols] = G0
            bas[rep * 64 + 2 * nq + q, cols] = Gp
            bas[rep * 64 + 3 * nq + q, cols] = 1.0
    return bas.astype(np.float32)


def _frame_amp_pi(u):
    nq = 512 // u
    if u == 16:
        Pi = np.zeros((32, 96))
        for g in range(3):
            for q in range(nq):
                src = max(q - 1, 0) if g == 0 else (q if g == 1 else min(q + 1, 31))
                Pi[src, g * nq + q] = 1.0
    else:
        Pi = np.zeros((32, 128))
        for cs in range(2):
            for g in range(3):
                for q in range(nq):
                    f = cs * nq + q
                    src = max(f - 1, 0) if g == 0 else (f if g == 1 else min(f + 1, 31))
                    Pi[src, cs * 64 + g * nq + q] = 1.0
    return Pi.astype(np.float32)


def _frame_amp_basis(u, adj):
    gm, g0, gp = _interp_vecs(u)
    inv = 1.0 / adj
    nq = 512 // u
    rows = 96 if u == 16 else 112
    bas = np.zeros((rows, 512))
    nrep = 1 if u == 16 else 2
    for rep in range(nrep):
        for q in range(nq):
            cols = slice(q * u, (q + 1) * u)
            bas[rep * 64 + 0 * nq + q, cols] = gm * inv
            bas[rep * 64 + 1 * nq + q, cols] = g0 * inv
            bas[rep * 64 + 2 * nq + q, cols] = gp * inv
    return bas.astype(np.float32)


def _sub_phase_pis(bs):
    u = bs // 32
    L2 = _band_L2(bs)
    CW = np.zeros_like(L2)
    for F in range(32):
        cols = slice(F * u, (F + 1) * u)
        CW[:, cols] = np.cumsum(L2[:, cols], axis=1)
    pis = []
    for j in range(bs // 1024):
        Pi = np.zeros((64, 128))
        for par in range(2):
            c = 2 * j + par
            for s in range(16):
                t0 = 512 * c + 32 * s
                F = t0 // u
                if t0 % u != 0:
                    Pi[0:32, par * 64 + s] = CW[:, t0 - 1]
                if F >= 1:
                    Pi[32 + F - 1, par * 64 + s] = 1.0
                Pi[0:32, par * 64 + 16 + s] = L2[:, t0]
                Pi[0:32, par * 64 + 32 + s] = L2[:, t0 + 1] - L2[:, t0]
        pis.append(Pi.astype(np.float32))
    return pis


def _sub_phase_basis():
    bas = np.zeros((112, 512))
    i = np.arange(32)
    for rep in range(2):
        for s in range(16):
            cols = slice(32 * s, 32 * (s + 1))
            bas[rep * 64 + s, cols] = 1.0
            bas[rep * 64 + 16 + s, cols] = i + 1
            bas[rep * 64 + 32 + s, cols] = i * (i + 1) / 2.0
    return bas.astype(np.float32)


def _sub_amp_pis(bs, adj):
    L2 = _band_L2(bs)
    inv = 1.0 / adj
    nch = bs // 512
    pis = []
    for j in range((nch + 2) // 3):
        Pi = np.zeros((32, 96))
        for m in range(3):
            c = 3 * j + m
            if c >= nch:
                break
            for s in range(16):
                t0 = 512 * c + 32 * s
                Pi[:, m * 32 + s] = L2[:, t0] * inv
                Pi[:, m * 32 + 16 + s] = (L2[:, t0 + 1] - L2[:, t0]) * inv
        pis.append(Pi.astype(np.float32))
    return pis


def _sub_amp_basis():
    bas = np.zeros((96, 512))
    i = np.arange(32)
    for rep in range(3):
        for s in range(16):
            cols = slice(32 * s, 32 * (s + 1))
            bas[rep * 32 + s, cols] = 1.0
            bas[rep * 32 + 16 + s, cols] = i
    return bas.astype(np.float32)


def _tridiag_M(u):
    M = np.zeros((32, 32))
    for f in range(32):
        M[max(f - 1, 0), f] += u / 8.0
        M[min(f + 1, 31), f] += u / 8.0
        M[f, f] += 3.0 * u / 4.0
    return M.astype(np.float32)


def _band_fir(bs):
    spf = bs // NNF
    nc_ = spf // 2 + 1
    t = np.arange(spf)
    j_re = np.arange(nc_)
    j_im = np.arange(1, nc_ - 1)
    FT = np.concatenate([np.cos(2 * np.pi * np.outer(t, j_re) / spf),
                         -np.sin(2 * np.pi * np.outer(t, j_im) / spf)], axis=1)
    w = np.full(nc_, 2.0)
    w[0] = 1.0
    w[-1] = 1.0
    IR = np.concatenate([
        (w[:, None] * np.cos(2 * np.pi * np.outer(j_re, t) / spf)) / spf,
        (-2.0 * np.sin(2 * np.pi * np.outer(j_im, t) / spf)) / spf,
    ], axis=0) / ADJUST[bs]
    return FT.astype(np.float32), IR.astype(np.float32)


def _build_U(n):
    eye = np.eye(n)
    spec = np.fft.rfft(eye, axis=-1)
    spec = np.pad(spec, ((0, 0), (0, n + 1 - spec.shape[-1])))
    return np.fft.irfft(spec, n=2 * n, axis=-1) * 2


def _mega_entries():
    ents = [('wlin', C + 1, 4 * C), ('ubd4', BL * 4, BL * 8), ('ubd8', BL * 8, BL * 16),
            ('ubd16', BL * 16, BL * 32), ('w64', C, len(W64_ORDER) * C),
            ('bias64', C, len(B64_ORDER)), ('ident128', 128, 128), ('ut', 32, 32)]
    for k in range(6):
        ents.append((f'wfrq{k}', C + 1, N_OSC))
        ents.append((f'wamp{k}', C + 1, N_OSC))
        ents.append((f'M{k}', 32, 32))
    ents += [('pi0', 64, 128), ('pia0', 32, 96), ('pi1', 64, 128), ('pia1', 32, 128)]
    for k in (2, 3, 4, 5):
        nch = NCH[k]
        for j in range(nch // 2):
            ents.append((f'pip{k}_{j}', 64, 128))
        for j in range((nch + 2) // 3):
            ents.append((f'piam{k}_{j}', 32, 96))
    ents += [('bas0', 128, 512), ('bas1', 128, 512), ('basS', 112, 512),
             ('basA0', 96, 512), ('basA1', 112, 512), ('basAS', 96, 512),
             ('selstrip', 128, 256), ('negI', 128, 128)]
    for k in range(6):
        spf = SPF[k]
        nc_ = spf // 2 + 1
        if k < 5:
            ents.append((f'wc{k}', C + 1, spf))
            ents.append((f'ft{k}', spf, spf))
            ents.append((f'ir{k}', spf, spf))
        else:
            ents.append(('wc5a', C + 1, 128))
            ents.append(('wc5b', C + 1, 128))
            ents.append(('ft5_0', 128, 256))
            ents.append(('ft5_1', 128, 256))
            ents.append(('ir5_0', 128, 256))
            ents.append(('ir5_1', 128, 256))
    off = {}
    o = 0
    for name, r, cd in ents:
        off[name] = (r, o, cd)
        o += cd
    return off, o


MEGA_OFF, MEGA_COLS = _mega_entries()


def _build_shared(inp):
    c = {}
    wl = np.zeros((4, C + 1, C), np.float32)
    for t in range(4):
        wl[t, :C] = inp['up_lin_w'][:, t::4]
        wl[t, C] = inp['up_lin_b'][t::4]
    c['wlin'] = wl.transpose(1, 0, 2).reshape(C + 1, 4 * C)
    for n in (4, 8, 16):
        U = _build_U(n)
        ub = np.zeros((BL * n, BL * 2 * n), np.float32)
        for b in range(BL):
            ub[b * n:(b + 1) * n, b * 2 * n:(b + 1) * 2 * n] = U
        c[f'ubd{n}'] = ub

    w64 = np.zeros((C, len(W64_ORDER) * C), np.float32)

    def put64(name, m):
        i = W64_IDX[name]
        w64[:, i * C:(i + 1) * C] = m

    for i in range(3):
        for dd in range(3):
            put64(f'up{i}d{dd}', inp['up_conv_w'][i, :, :, dd].T)
    for dd in range(3):
        put64(f'find{dd}', inp['up_final_w'][:, :, dd].T)
    for k in range(6):
        for j in range(4):
            put64(f't{k}{j}', inp['t_w'][k, j].T + np.eye(C, dtype=np.float32))
        put64(f'bf{k}', inp['band_final_w'][k].T)
        put64(f'nup{k}', inp['noise_up_w'][k].T)
    put64('ident', np.eye(C))
    c['w64'] = w64

    b64 = np.zeros((C, len(B64_ORDER)), np.float32)
    for i in range(3):
        b64[:, B64_IDX[f'up{i}']] = inp['up_conv_b'][i]
    b64[:, B64_IDX['fin']] = inp['up_final_b']
    for k in range(6):
        for j in range(4):
            b64[:, B64_IDX[f't{k}{j}']] = inp['t_b'][k, j]
        b64[:, B64_IDX[f'bf{k}']] = inp['band_final_b'][k]
        b64[:, B64_IDX[f'nup{k}']] = inp['noise_up_b'][k]
    c['bias64'] = b64
    c['ident128'] = np.eye(128, dtype=np.float32)

    sel = np.zeros((128, 256), np.float32)
    for b in range(BL):
        sel[b * N_OSC:(b + 1) * N_OSC, 128 + b] = 1.0
    c['selstrip'] = sel
    c['negI'] = (-np.eye(128)).astype(np.float32)
    c['ut'] = np.triu(np.ones((32, 32))).astype(np.float32)

    for k, bs in enumerate(BAND_SIZES):
        u = bs // 32
        wf = np.zeros((C + 1, N_OSC), np.float32)
        wf[:C] = inp['osc_freq_w'][k].T
        wf[C] = inp['osc_freq_b'][k]
        c[f'wfrq{k}'] = wf
        wa = np.zeros((C + 1, N_OSC), np.float32)
        wa[:C] = inp['osc_amp_w'][k].T
        wa[C] = inp['osc_amp_b'][k]
        c[f'wamp{k}'] = wa
        c[f'M{k}'] = _tridiag_M(u)

    c['pi0'] = _frame_phase_pi(16)
    c['pia0'] = _frame_amp_pi(16)
    c['pi1'] = _frame_phase_pi(32)
    c['pia1'] = _frame_amp_pi(32)
    for k in (2, 3, 4, 5):
        bs = BAND_SIZES[k]
        for j, Pi in enumerate(_sub_phase_pis(bs)):
            c[f'pip{k}_{j}'] = Pi
        for j, Pi in enumerate(_sub_amp_pis(bs, ADJUST[bs])):
            c[f'piam{k}_{j}'] = Pi
    c['bas0'] = _frame_phase_basis(16)
    c['bas1'] = _frame_phase_basis(32)
    c['basS'] = _sub_phase_basis()
    c['basA0'] = _frame_amp_basis(16, ADJUST[512])
    c['basA1'] = _frame_amp_basis(32, ADJUST[1024])
    c['basAS'] = _sub_amp_basis()

    for k, bs in enumerate(BAND_SIZES):
        spf = SPF[k]
        nc_ = spf // 2 + 1
        wcf = np.zeros((C + 1, spf), np.float32)
        wc = np.zeros((C + 1, nc_), np.float32)
        wc[:C] = inp[f'noise_coeff_w_{k}'].T
        wc[C] = inp[f'noise_coeff_b_{k}']
        if k == 0:
            wc[:, 1:] = 0.0
        wcf[:, 0:nc_] = wc
        wcf[:, nc_:spf] = wc[:, 1:nc_ - 1]
        FT, IR = _band_fir(bs)
        if k < 5:
            c[f'wc{k}'] = wcf
            c[f'ft{k}'] = FT
            c[f'ir{k}'] = IR
        else:
            c['wc5a'] = wcf[:, 0:128]
            c['wc5b'] = wcf[:, 128:256]
            c['ft5_0'] = FT[0:128]
            c['ft5_1'] = FT[128:256]
            c['ir5_0'] = IR[0:128]
            c['ir5_1'] = IR[128:256]

    mega = np.zeros((128, MEGA_COLS), np.float32)
    for name, (r, o, cd) in MEGA_OFF.items():
        mega[0:r, o:o + cd] = c[name]
    return {'mega': mega}


# ---------------------------------------------------------------- bass build
def _build_nc():
    nc = bacc.Bacc('TRN2', num_devices=NCORE)
    AF = mybir.ActivationFunctionType

    d = {}
    d['xT'] = nc.dram_tensor("xT", [C + 1, BL], F32, kind="ExternalInput")
    d['mega'] = nc.dram_tensor("mega", [128, MEGA_COLS], F32, kind="ExternalInput")
    for k in range(6):
        d[f'noiseT{k}'] = nc.dram_tensor(f"noiseT{k}", [SPF[k], FR], F32,
                                         kind="ExternalInput")
    harm_d = nc.dram_tensor("harm", [4 * 63, 512], F32, kind="ExternalOutput")
    nz_d = nc.dram_tensor("nz", [FR, int(NOFF[6])], F32, kind="ExternalOutput")


    with TileContext(nc) as tc:
        with tc.tile_pool(name="const", bufs=1) as cp, \
             tc.tile_pool(name="work", bufs=2) as wp, \
             tc.tile_pool(name="hot", bufs=3) as hot, \
             tc.tile_pool(name="phF", bufs=3, space="PSUM") as phF, \
             tc.tile_pool(name="phA", bufs=3, space="PSUM") as phA, \
             tc.tile_pool(name="phH", bufs=2, space="PSUM") as phH:

            def psF(p0, f0):
                t = phF.tile([128, 512], F32, tag="phF", name="psF")
                return t[0:p0, 0:f0]

            def psA(p0, f0):
                t = phA.tile([128, 512], F32, tag="phA", name="psA")
                return t[0:p0, 0:f0]

            def psH(p0, f0):
                t = phH.tile([128, 512], F32, tag="phH", name="psH")
                return t[0:p0, 0:f0]

            mega = cp.tile([128, MEGA_COLS], F32, tag="mega")
            _nsplit = 6
            _cut = [MEGA_COLS * i // _nsplit for i in range(_nsplit + 1)]
            for _i in range(_nsplit):
                nc.gpsimd.dma_start(out=mega[:, _cut[_i]:_cut[_i + 1]],
                                    in_=d['mega'][:, _cut[_i]:_cut[_i + 1]])

            ct = {}
            for name, (r, o, cd) in MEGA_OFF.items():
                ct[name] = mega[0:r, o:o + cd]
            for name in ('selstrip', 'negI', 'basA0', 'basA1', 'basAS'):
                r, o, cd = MEGA_OFF[name]
                t = cp.tile([r, cd], BF16, tag=f"bf_{name}")
                nc.gpsimd.dma_start(out=t, in_=d['mega'][0:r, o:o + cd])
                ct[f'{name}_bf'] = t
            for name in ('bas0', 'bas1', 'basS'):
                r, o, cd = MEGA_OFF[name]
                t = cp.tile([r, cd], F32R, tag=f"r_{name}")
                nc.gpsimd.dma_start(out=t, in_=d['mega'][0:r, o:o + cd])
                ct[f'{name}_r'] = t

            def w64s(name):
                i = W64_IDX[name]
                return ct['w64'][:, i * C:(i + 1) * C]

            def b64s(name):
                return ct['bias64'][:, B64_IDX[name]:B64_IDX[name] + 1]

            ident64 = w64s('ident')

            xT = cp.tile([C + 1, BL], F32, tag="xT")
            nc.sync.dma_start(out=xT, in_=d['xT'][:, :])

            # ---------------- frontend (as baseline)
            h = wp.tile([C, 16], F32, tag="h0")
            for t in range(4):
                pt = psF(C, BL)
                nc.tensor.matmul(out=pt, lhsT=ct['wlin'][:, t * C:(t + 1) * C],
                                 rhs=xT, start=True, stop=True)
                nc.vector.tensor_copy(out=h.rearrange("c (b t) -> c b t", t=4)[:, :, t],
                                      in_=pt)
            for i, n in enumerate((4, 8, 16)):
                pt1 = psF(BL * n, C)
                nc.tensor.transpose(out=pt1, in_=h, identity=ident64)
                t1 = wp.tile([BL * n, C], F32, tag=f"fe_t1_{i}")
                nc.scalar.copy(out=t1, in_=pt1)
                pt2 = psF(BL * 2 * n, C)
                nc.tensor.matmul(out=pt2, lhsT=ct[f'ubd{n}'], rhs=t1, start=True,
                                 stop=True)
                t2 = wp.tile([BL * 2 * n, C], F32, tag=f"fe_t2_{i}")
                nc.scalar.copy(out=t2, in_=pt2)
                pt3 = psF(C, BL * 2 * n)
                nc.tensor.transpose(out=pt3, in_=t2,
                                    identity=ct['ident128'][0:BL * 2 * n, 0:BL * 2 * n])
                hu = wp.tile([C, BL * 2 * n], F32, tag=f"fe_hu_{i}")
                nc.scalar.copy(out=hu, in_=pt3)
                m = 2 * n
                hu3 = hu.rearrange("c (b t) -> c b t", b=BL)
                pc = psF(C, BL * m).rearrange("c (b t) -> c b t", b=BL)
                nc.tensor.matmul(out=pc[:, :, :], lhsT=w64s(f'up{i}d1'), rhs=hu3[:, :, :],
                                 start=True, stop=False)
                nc.tensor.matmul(out=pc[:, :, 1:m], lhsT=w64s(f'up{i}d0'),
                                 rhs=hu3[:, :, 0:m - 1], start=False, stop=False)
                nc.tensor.matmul(out=pc[:, :, 0:m - 1], lhsT=w64s(f'up{i}d2'),
                                 rhs=hu3[:, :, 1:m], start=False, stop=True)
                h = wp.tile([C, BL * m], F32, tag=f"fe_h_{i}")
                nc.scalar.activation(out=h.rearrange("c (b t) -> c b t", b=BL), in_=pc,
                                     func=AF.Prelu, bias=b64s(f'up{i}'), scale=1.0,
                                     alpha=0.2)
            h3 = h.rearrange("c (b t) -> c b t", b=BL)
            pf = psF(C, BL * 32).rearrange("c (b t) -> c b t", b=BL)
            nc.tensor.matmul(out=pf[:, :, :], lhsT=w64s('find1'), rhs=h3[:, :, :],
                             start=True, stop=False)
            nc.tensor.matmul(out=pf[:, :, 1:32], lhsT=w64s('find0'), rhs=h3[:, :, 0:31],
                             start=False, stop=False)
            nc.tensor.matmul(out=pf[:, :, 0:31], lhsT=w64s('find2'), rhs=h3[:, :, 1:32],
                             start=False, stop=True)
            hfin = cp.tile([C, 128], F32, tag="hfin")
            nc.scalar.activation(out=hfin.rearrange("c (b t) -> c b t", b=BL), in_=pf,
                                 func=AF.Identity, bias=b64s('fin'), scale=1.0)

            # ---------------- per-band setup
            nTs_all = {}
            for k in range(6):
                if k < 5:
                    nT = wp.tile([SPF[k], FR], F32, tag=f"nT{k}", name="nT", bufs=1)
                    nc.sync.dma_start(out=nT, in_=d[f'noiseT{k}'][:, :])
                    nTs_all[k] = (nT,)
                else:
                    nT0 = wp.tile([128, FR], F32, tag="nT50", bufs=1)
                    nT1 = wp.tile([128, FR], F32, tag="nT51", bufs=1)
                    nc.sync.dma_start(out=nT0, in_=d['noiseT5'][0:128, :])
                    nc.sync.dma_start(out=nT1, in_=d['noiseT5'][128:256, :])
                    nTs_all[5] = (nT0, nT1)
            def noise_branch(k):
                bs = BAND_SIZES[k]
                spf = SPF[k]
                    zf3 = zfas[k][0:C, :].rearrange("c (b t) -> c b t", b=BL)
                    zrep = zf3.unsqueeze(-1).broadcast_to([C, BL, 32, 2])
                    pn = psA(C, FR)
                    nc.tensor.matmul(out=pn, lhsT=w64s(f'nup{k}'), rhs=zrep,
                                         start=True, stop=True)
                    naug = wp.tile([C + 1, FR], F32, tag="naug")
                    nc.scalar.activation(out=naug[0:C, :], in_=pn, func=AF.Prelu,
                                             bias=b64s(f'nup{k}'), scale=1.0, alpha=0.2)
                    nc.vector.memset(naug[C:C + 1, :], 1.0)

                    if k < 5:
                        nT = nTs_all[k][0]
                        pcA = psH(spf, FR)
                        nc.tensor.matmul(out=pcA, lhsT=ct[f'wc{k}'], rhs=naug,
                                             start=True, stop=True)
                        chat = wp.tile([spf, FR], F32, tag="chat")
                        nc.scalar.copy(out=chat, in_=pcA)
                        psp = psH(spf, FR)
                        nc.tensor.matmul(out=psp, lhsT=ct[f'ft{k}'], rhs=nT,
                                             start=True, stop=True)
                        sA = wp.tile([spf, FR], F32, tag="sA")
                        nc.vector.tensor_tensor(out=sA, in0=chat, in1=psp, op=ALU.mult)
                        sAs = [sA]
                    else:
                        nT0, nT1 = nTs_all[5]
                        sAs = []
                        for half, wch in ((0, 'wc5a'), (1, 'wc5b')):
                            pcA = psH(128, FR)
                            nc.tensor.matmul(out=pcA, lhsT=ct[wch], rhs=naug,
                                                 start=True, stop=True)
                            chat = wp.tile([128, FR], F32, tag=f"chat5{half}")
                            nc.scalar.copy(out=chat, in_=pcA)
                            psp = psH(128, FR)
                            nc.tensor.matmul(out=psp,
                                                 lhsT=ct['ft5_0'][:, 128 * half:128 * (half + 1)],
                                                 rhs=nT0, start=True, stop=False)
                            nc.tensor.matmul(out=psp,
                                                 lhsT=ct['ft5_1'][:, 128 * half:128 * (half + 1)],
                                                 rhs=nT1, start=False, stop=True)
                            sA = wp.tile([128, FR], F32, tag=f"sA5{half}")
                            nc.vector.tensor_tensor(out=sA, in0=chat, in1=psp, op=ALU.mult)
                            sAs.append(sA)

                    for fg in range(2):
                        pnz = psA(128, spf)
                        if k < 5:
                            nc.tensor.matmul(out=pnz,
                                                 lhsT=sAs[0][:, 128 * fg:128 * (fg + 1)],
                                                 rhs=ct[f'ir{k}'], start=True, stop=True)
                        else:
                            nc.tensor.matmul(out=pnz,
                                                 lhsT=sAs[0][:, 128 * fg:128 * (fg + 1)],
                                                 rhs=ct['ir5_0'], start=True, stop=False)
                            nc.tensor.matmul(out=pnz,
                                                 lhsT=sAs[1][:, 128 * fg:128 * (fg + 1)],
                                                 rhs=ct['ir5_1'], start=False, stop=True)
                        nzs = wp.tile([128, spf], F32, tag="nzs")
                        nc.scalar.copy(out=nzs, in_=pnz)
                        nc.sync.dma_start(
                            out=nz_d[128 * fg:128 * (fg + 1),
                                         int(NOFF[k]):int(NOFF[k]) + spf],
                            in_=nzs)


            stacks = {}
            astacks = {}
            zfas = {}
            srcs = {}
            ampgs = {}
            LFS = [0.05 if bs == 512 else 0.01 for bs in BAND_SIZES]

            # pass 1: residual stacks, layer-interleaved across bands
            zs = {k: hfin for k in range(6)}
            for j in range(4):
                for k in range(6):
                    pz = psF(C, 128)
                    nc.tensor.matmul(out=pz, lhsT=w64s(f't{k}{j}'), rhs=zs[k],
                                     start=True, stop=True)
                    z = wp.tile([C, 128], F32, tag=f"z{k}_{j % 2}", name="z", bufs=1)
                    nc.scalar.activation(out=z, in_=pz, func=AF.Prelu,
                                         bias=b64s(f't{k}{j}'), scale=1.0, alpha=0.2)
                    zs[k] = z
            for k in range(6):
                pz = psF(C, 128)
                nc.tensor.matmul(out=pz, lhsT=w64s(f'bf{k}'), rhs=zs[k],
                                 start=True, stop=True)
                zfa = wp.tile([C + 1, 128], F32, tag=f"zfa{k}", name="zfa", bufs=1)
                nc.scalar.activation(out=zfa[0:C, :], in_=pz, func=AF.Identity,
                                     bias=b64s(f'bf{k}'), scale=1.0)
                nc.vector.memset(zfa[C:C + 1, :], 1.0)
                zfas[k] = zfa

            # pass 2: freq/amp grids
            for k in range(6):
                zfa = zfas[k]
                pgF = psH(N_OSC, 128)
                for b in range(BL):
                    nc.tensor.matmul(out=pgF[:, 32 * b:32 * (b + 1)],
                                     lhsT=zfa[:, 32 * b:32 * (b + 1)],
                                     rhs=ct[f'wfrq{k}'], start=True, stop=True)
                sig = wp.tile([N_OSC, 128], F32, tag=f"sig{k}", name="sig", bufs=1)
                nc.scalar.activation(out=sig, in_=pgF, func=AF.Sigmoid, scale=1.0)
                src = wp.tile([64, 128], F32, tag=f"src{k}", name="src", bufs=1)
                nc.vector.tensor_scalar(out=src[0:32, :], in0=sig,
                                        scalar1=float((1.0 - LFS[k]) / 2.0),
                                        scalar2=float(LFS[k] / 2.0),
                                        op0=ALU.mult, op1=ALU.add)
                srcs[k] = src
                pgA = psH(N_OSC, 128)
                for b in range(BL):
                    nc.tensor.matmul(out=pgA[:, 32 * b:32 * (b + 1)],
                                     lhsT=zfa[:, 32 * b:32 * (b + 1)],
                                     rhs=ct[f'wamp{k}'], start=True, stop=True)
                ampg = wp.tile([N_OSC, 128], F32, tag=f"ampg{k}", name="ampg", bufs=1)
                nc.scalar.activation(out=ampg, in_=pgA, func=AF.Abs, scale=1.0)
                ampgs[k] = ampg

            # pass 3: frame carries
            Srs = {}
            for k in range(6):
                pS = psH(N_OSC, 128)
                nc.tensor.matmul(out=pS, lhsT=ct[f'M{k}'], rhs=srcs[k][0:32, :],
                                 start=True, stop=True)
                rndS = wp.tile([N_OSC, 128], F32, tag=f"rndS{k}", name="rndS", bufs=1)
                nc.vector.tensor_scalar(out=rndS, in0=pS, scalar1=MAGIC,
                                        scalar2=MAGIC, op0=ALU.add,
                                        op1=ALU.subtract)
                Sr = wp.tile([N_OSC, 128], F32, tag=f"Sr{k}", name="Sr", bufs=1)
                nc.vector.tensor_tensor(out=Sr, in0=pS, in1=rndS, op=ALU.subtract)
                Srs[k] = Sr
            for k in range(6):
                pP = psH(N_OSC, 128)
                nc.tensor.matmul(out=pP, lhsT=ct['ut'], rhs=Srs[k], start=True,
                                 stop=True)
                rndP = wp.tile([N_OSC, 128], F32, tag=f"rndP{k}", name="rndP", bufs=1)
                nc.vector.tensor_scalar(out=rndP, in0=pP, scalar1=MAGIC,
                                        scalar2=MAGIC, op0=ALU.add,
                                        op1=ALU.subtract)
                nc.vector.tensor_tensor(out=srcs[k][32:64, :], in0=pP, in1=rndP,
                                        op=ALU.subtract)

            # pass 4: per-band stacks + noise
            for k, bs in enumerate(BAND_SIZES):
                nch = NCH[k]
                src = srcs[k]
                ampg = ampgs[k]
                if k == 0:
                    stk = cp.tile([128, 128], F32, tag="stk0")
                    pb = psF(128, 128)
                    nc.tensor.matmul(out=pb, lhsT=ct['pi0'], rhs=src, start=True,
                                     stop=True)
                    nc.scalar.copy(out=stk, in_=pb)
                elif k == 1:
                    stk = cp.tile([128, 128], F32, tag="stk1")
                    pb = psF(128, 128)
                    nc.tensor.matmul(out=pb, lhsT=ct['pi1'], rhs=src, start=True,
                                     stop=True)
                    nc.scalar.copy(out=stk, in_=pb)
                else:
                    stk = cp.tile([128, (nch // 2) * 128], F32, tag=f"stk{k}")
                    nblkp = nch // 2
                    for j0 in range(0, nblkp, 4):
                        jn = min(4, nblkp - j0)
                        pb = psF(128, 128 * jn)
                        for jj in range(jn):
                            nc.tensor.matmul(out=pb[:, 128 * jj:128 * (jj + 1)],
                                             lhsT=ct[f'pip{k}_{j0 + jj}'], rhs=src,
                                             start=True, stop=True)
                        rnd = wp.tile([128, 512], F32, tag="rndB")
                        nc.vector.tensor_scalar(out=rnd[:, 0:128 * jn], in0=pb,
                                                scalar1=MAGIC, scalar2=MAGIC,
                                                op0=ALU.add, op1=ALU.subtract)
                        nc.vector.tensor_tensor(
                            out=stk[:, 128 * j0:128 * (j0 + jn)],
                            in0=pb, in1=rnd[:, 0:128 * jn], op=ALU.subtract)
                if k == 0:
                    stkr = cp.tile([128, 128], F32R, tag="stkr0", name="stkr")
                elif k == 1:
                    stkr = cp.tile([128, 128], F32R, tag="stkr1", name="stkr")
                else:
                    stkr = cp.tile([128, (nch // 2) * 128], F32R, tag=f"stkr{k}",
                                   name="stkr")
                nc.gpsimd.dma_start(out=stkr, in_=stk)
                stacks[k] = stkr

                if k == 0:
                    ast = cp.tile([96, 128], BF16, tag="ast0")
                    pb = psF(96, 128)
                    nc.tensor.matmul(out=pb, lhsT=ct['pia0'], rhs=ampg, start=True,
                                     stop=True)
                    nc.scalar.copy(out=ast, in_=pb)
                elif k == 1:
                    ast = cp.tile([128, 128], BF16, tag="ast1")
                    pb = psF(128, 128)
                    nc.tensor.matmul(out=pb, lhsT=ct['pia1'], rhs=ampg, start=True,
                                     stop=True)
                    nc.scalar.copy(out=ast, in_=pb)
                else:
                    nblk = (nch + 2) // 3
                    ast = cp.tile([96, nblk * 128], BF16, tag=f"ast{k}")
                    for j0 in range(0, nblk, 4):
                        jn = min(4, nblk - j0)
                        pb = psF(96, 128 * jn)
                        for jj in range(jn):
                            nc.tensor.matmul(out=pb[:, 128 * jj:128 * (jj + 1)],
                                             lhsT=ct[f'piam{k}_{j0 + jj}'], rhs=ampg,
                                             start=True, stop=True)
                        nc.scalar.copy(out=ast[:, 128 * j0:128 * (j0 + jn)], in_=pb)
                astacks[k] = ast
                noise_branch(k)

            # ---------------- chunk loop (per-chunk, deep software pipeline)
            chunks = []
            for k in (1, 2, 3, 4, 5, 0):
                nch = NCH[k]
                for gstart in range(0, nch, 8):
                    gs = min(8, nch - gstart)
                    for cc in range(gs):
                        chunks.append((k, gstart + cc, cc, gs))
            Nc = len(chunks)
            state = {}

            def stage_a(i):
                k, c, cc, gs = chunks[i]
                ppt = phF.tile([128, 512], F32, tag="phF", name="ppt")
                if k == 0:
                    nc.tensor.matmul(out=ppt, lhsT=stacks[0], rhs=ct['bas0_r'],
                                     start=True, stop=False)
                elif k == 1:
                    nc.tensor.matmul(out=ppt, lhsT=stacks[1][64 * c:64 * c + 64],
                                     rhs=ct['bas1_r'][64 * c:64 * c + 64],
                                     start=True, stop=False)
                else:
                    j, par = c // 2, c % 2
                    nc.tensor.matmul(
                        out=ppt,
                        lhsT=stacks[k][64 * par:64 * par + 48,
                                       128 * j:128 * (j + 1)],
                        rhs=ct['basS_r'][64 * par:64 * par + 48],
                        start=True, stop=False)
                pat = phA.tile([128, 512], F32, tag="phA", name="pat")
                if k == 0:
                    nc.tensor.matmul(out=pat, lhsT=astacks[0],
                                     rhs=ct['basA0_bf'], start=True, stop=True)
                elif k == 1:
                    nc.tensor.matmul(out=pat, lhsT=astacks[1][64 * c:64 * c + 48],
                                     rhs=ct['basA1_bf'][64 * c:64 * c + 48],
                                     start=True, stop=True)
                else:
                    ja, ma = c // 3, c % 3
                    nc.tensor.matmul(
                        out=pat,
                        lhsT=astacks[k][32 * ma:32 * ma + 32,
                                        128 * ja:128 * (ja + 1)],
                        rhs=ct['basAS_bf'][32 * ma:32 * ma + 32],
                        start=True, stop=True)
                ntile = hot.tile([128, 512], BF16, tag="ntile")
                nc.vector.tensor_scalar(out=ntile, in0=ppt, scalar1=MAGIC,
                                        scalar2=MAGIC, op0=ALU.add,
                                        op1=ALU.subtract)
                if i % 2 == 0:
                    pat_sb = hot.tile([128, 512], BF16, tag="pat_sb")
                    nc.scalar.copy(out=pat_sb, in_=pat)
                else:
                    pat_sb = None
                state[i] = (ppt, pat, ntile, pat_sb)

            def stage_b(i):
                ppt, pat, ntile, pat_sb = state[i]
                nc.tensor.matmul(out=ppt, lhsT=ct['negI_bf'], rhs=ntile,
                                 start=False, stop=True)
                s = hot.tile([128, 512], BF16, tag="sin_t")
                nc.scalar.activation(out=s, in_=ppt, func=AF.Sin, scale=TWO_PI)
                prod = hot.tile([128, 512], BF16, tag="prod_t")
                nc.vector.tensor_tensor(out=prod, in0=s,
                                        in1=pat_sb if pat_sb is not None else pat,
                                        op=ALU.mult)
                state[i] = prod

            hpt_cur = [None]

            def stage_c(i):
                k, c, cc, gs = chunks[i]
                prod = state.pop(i)
                if cc == 0:
                    hpt_cur[0] = phH.tile([128, 512], F32, tag="phH", name="hpt")
                hpt = hpt_cur[0]
                nc.tensor.matmul(out=hpt,
                                 lhsT=ct['selstrip_bf'][:, 128 - 4 * cc:256 - 4 * cc],
                                 rhs=prod, start=(cc == 0), stop=(cc == gs - 1))
                if cc == gs - 1:
                    hsb = wp.tile([32, 512], F32, tag="hsb")
                    nc.scalar.copy(out=hsb[0:4 * gs], in_=hpt[0:4 * gs])
                    g0 = CHUNK_BASE[k] + c - (gs - 1)
                    nc.sync.dma_start(
                        out=harm_d[4 * int(g0):4 * int(g0) + 4 * gs, :],
                        in_=hsb[0:4 * gs])

            for i in range(Nc + 4):
                if i < Nc:
                    stage_a(i)
                if 2 <= i < Nc + 2:
                    stage_b(i - 2)
                if 4 <= i < Nc + 4:
                    stage_c(i - 4)

    nc.finalize()
    return nc


# ---------------------------------------------------------------- host glue
def _prep_inputs(inputs):
    inp = {k: np.asarray(v, np.float32) for k, v in inputs.items()}
    shared = _build_shared(inp)
    in_maps = []
    for core in range(NCORE):
        m = dict(shared)
        sl = slice(core * BL, (core + 1) * BL)
        m['xT'] = np.concatenate([inp['x'][sl].T, np.ones((1, BL), np.float32)],
                                 axis=0)
        for k in range(6):
            nT = inp[f'noise_{k}'][sl].reshape(FR, SPF[k]).T
            m[f'noiseT{k}'] = np.ascontiguousarray(nT, dtype=np.float32)
        in_maps.append(m)
    return in_maps


def kernel(**inputs):
    if 'nc' not in _nc_cache:
        _nc_cache['nc'] = _build_nc()
    nc = _nc_cache['nc']
    in_maps = _prep_inputs(inputs)
    res = run_bass_kernel_spmd(nc, in_maps, list(range(NCORE)))
    out = np.zeros((B, TOTAL), np.float32)
    for core in range(NCORE):
        r = res.results[core]
        harm = np.asarray(r['harm'], np.float32).reshape(63, BL, 512)
        nz = np.asarray(r['nz'], np.float32).reshape(BL, NNF, int(NOFF[6]))
        for k, bs in enumerate(BAND_SIZES):
            nch = NCH[k]
            hb = harm[CHUNK_BASE[k]:CHUNK_BASE[k] + nch]  # [nch, BL, 512]
            hb = hb.transpose(1, 0, 2).reshape(BL, bs)
            out[core * BL:(core + 1) * BL,
                HARM_OFF[k]:HARM_OFF[k] + bs] = hb
            nzb = nz[:, :, int(NOFF[k]):int(NOFF[k]) + SPF[k]].reshape(BL, bs)
            out[core * BL:(core + 1) * BL, NZ_OFF[k]:NZ_OFF[k] + bs] = nzb
    return out.astype(np.float32)


if __name__ == "__main__":
    import reference
    inp = reference.setup_inputs()
    out = kernel(**{k: np.asarray(v) for k, v in inp.items()})
    print("out", out.shape, out.dtype)


# revision 34
# speedup vs baseline: 1.2524x; 1.0078x over previous
"""Trainium2 Bass kernel for nn_Decoder (DDSP-style decoder) — redesigned.

Data-parallel over batch (32 -> 4 per core x 8 cores). Harmonic synthesis
uses per-frame (bands 0-1) / per-32-sample-subframe quadratic (bands 2-5)
phase bases so every chunk matmul's magnitudes stay small enough for the
fast float32r PE path. Grid coefficients are produced directly in
[frame, (batch,osc)] layout by operand-swapped matmuls and reorganized into
per-chunk lhsT stacks by constant permutation matmuls (no DRAM scratch, no
gather DMAs). The noise branch is real-DFT basis matmuls in float32r on
host-transposed noise.
"""
import numpy as np
import sys

sys.path.insert(0, "/opt/trn_rl_repo")

from concourse import bacc, mybir  # noqa: E402
from concourse.tile import TileContext  # noqa: E402
from concourse.bass_utils import run_bass_kernel_spmd  # noqa: E402

F32 = mybir.dt.float32
F32R = mybir.dt.float32r
BF16 = mybir.dt.bfloat16
ALU = mybir.AluOpType
BAND_SIZES = [512, 1024, 2048, 4096, 8192, 16384]
ADJUST = {512: 0.05, 1024: 0.03, 2048: 0.05, 4096: 0.25, 8192: 1.0, 16384: 20.0}
B, C, N_OSC, NNF = 32, 64, 32, 64
NCORE = 8
BL = B // NCORE
FR = BL * NNF
MAGIC = float(1.5 * 2 ** 23)
TWO_PI = float(2 * np.pi)
TOTAL = 2 * sum(BAND_SIZES)

SPF = [bs // NNF for bs in BAND_SIZES]            # 8..256
NOFF = np.concatenate([[0], np.cumsum(SPF)]).astype(int)   # noise col offsets
NCH = [bs // 512 for bs in BAND_SIZES]            # 1,2,4,8,16,32
CHUNK_BASE = np.concatenate([[0], np.cumsum(NCH)]).astype(int)
HARM_OFF = {}
NZ_OFF = {}
_off = 0
for _k, _bs in enumerate(BAND_SIZES):
    HARM_OFF[_k] = _off
    NZ_OFF[_k] = _off + _bs
    _off += 2 * _bs

_nc_cache = {}

W64_ORDER = ([f'up{i}d{dd}' for i in range(3) for dd in range(3)]
             + [f'find{dd}' for dd in range(3)]
             + [w for k in range(6) for w in
                [f't{k}0', f't{k}1', f't{k}2', f't{k}3', f'bf{k}', f'nup{k}']]
             + ['ident'])
W64_IDX = {n: i for i, n in enumerate(W64_ORDER)}
B64_ORDER = ([f'up{i}' for i in range(3)] + ['fin']
             + [b for k in range(6) for b in
                [f't{k}0', f't{k}1', f't{k}2', f't{k}3', f'bf{k}', f'nup{k}']])
B64_IDX = {n: i for i, n in enumerate(B64_ORDER)}


# ---------------------------------------------------------------- host math
def _band_L2(bs):
    t = np.arange(bs)
    pos = (t + 0.5) * (32.0 / bs) - 0.5
    lo = np.clip(np.floor(pos).astype(int), 0, 31)
    hi = np.clip(lo + 1, 0, 31)
    w = np.clip(pos - lo, 0.0, 1.0)
    L2 = np.zeros((32, bs))
    np.add.at(L2, (lo, t), 1.0 - w)
    np.add.at(L2, (hi, t), w)
    return L2


def _interp_vecs(u):
    r = np.arange(u)
    f = (r + 0.5) / u - 0.5
    gm = np.where(r < u // 2, -f, 0.0)
    g0 = np.where(r < u // 2, 1 + f, 1 - f)
    gp = np.where(r >= u // 2, f, 0.0)
    return gm, g0, gp


def _frame_phase_pi(u):
    Pi = np.zeros((64, 128))
    nq = 512 // u
    nslot = 1 if u == 16 else 2
    for cs in range(nslot):
        for g in range(4):
            for q in range(nq):
                row = cs * 64 + g * nq + q
                f = cs * nq + q
                if g == 0:
                    Pi[max(f - 1, 0), row] = 1.0
                elif g == 1:
                    Pi[f, row] = 1.0
                elif g == 2:
                    Pi[min(f + 1, 31), row] = 1.0
                elif f > 0:
                    Pi[32 + f - 1, row] = 1.0
    return Pi.astype(np.float32)


def _frame_phase_basis(u):
    gm, g0, gp = _interp_vecs(u)
    Gm, G0, Gp = np.cumsum(gm), np.cumsum(g0), np.cumsum(gp)
    nq = 512 // u
    bas = np.zeros((128, 512))
    nrep = 1 if u == 16 else 2
    for rep in range(nrep):
        for q in range(nq):
            cols = slice(q * u, (q + 1) * u)
            bas[rep * 64 + 0 * nq + q, cols] = Gm
            bas[rep * 64 + 1 * nq + q, cols] = G0
            bas[rep * 64 + 2 * nq + q, cols] = Gp
            bas[rep * 64 + 3 * nq + q, cols] = 1.0
    return bas.astype(np.float32)


def _frame_amp_pi(u):
    nq = 512 // u
    if u == 16:
        Pi = np.zeros((32, 96))
        for g in range(3):
            for q in range(nq):
                src = max(q - 1, 0) if g == 0 else (q if g == 1 else min(q + 1, 31))
                Pi[src, g * nq + q] = 1.0
    else:
        Pi = np.zeros((32, 128))
        for cs in range(2):
            for g in range(3):
                for q in range(nq):
                    f = cs * nq + q
                    src = max(f - 1, 0) if g == 0 else (f if g == 1 else min(f + 1, 31))
                    Pi[src, cs * 64 + g * nq + q] = 1.0
    return Pi.astype(np.float32)


def _frame_amp_basis(u, adj):
    gm, g0, gp = _interp_vecs(u)
    inv = 1.0 / adj
    nq = 512 // u
    rows = 96 if u == 16 else 112
    bas = np.zeros((rows, 512))
    nrep = 1 if u == 16 else 2
    for rep in range(nrep):
        for q in range(nq):
            cols = slice(q * u, (q + 1) * u)
            bas[rep * 64 + 0 * nq + q, cols] = gm * inv
            bas[rep * 64 + 1 * nq + q, cols] = g0 * inv
            bas[rep * 64 + 2 * nq + q, cols] = gp * inv
    return bas.astype(np.float32)


def _sub_phase_pis(bs):
    u = bs // 32
    L2 = _band_L2(bs)
    CW = np.zeros_like(L2)
    for F in range(32):
        cols = slice(F * u, (F + 1) * u)
        CW[:, cols] = np.cumsum(L2[:, cols], axis=1)
    pis = []
    for j in range(bs // 1024):
        Pi = np.zeros((64, 128))
        for par in range(2):
            c = 2 * j + par
            for s in range(16):
                t0 = 512 * c + 32 * s
                F = t0 // u
                if t0 % u != 0:
                    Pi[0:32, par * 64 + s] = CW[:, t0 - 1]
                if F >= 1:
                    Pi[32 + F - 1, par * 64 + s] = 1.0
                Pi[0:32, par * 64 + 16 + s] = L2[:, t0]
                Pi[0:32, par * 64 + 32 + s] = L2[:, t0 + 1] - L2[:, t0]
        pis.append(Pi.astype(np.float32))
    return pis


def _sub_phase_basis():
    bas = np.zeros((112, 512))
    i = np.arange(32)
    for rep in range(2):
        for s in range(16):
            cols = slice(32 * s, 32 * (s + 1))
            bas[rep * 64 + s, cols] = 1.0
            bas[rep * 64 + 16 + s, cols] = i + 1
            bas[rep * 64 + 32 + s, cols] = i * (i + 1) / 2.0
    return bas.astype(np.float32)


def _sub_amp_pis(bs, adj):
    L2 = _band_L2(bs)
    inv = 1.0 / adj
    nch = bs // 512
    pis = []
    for j in range((nch + 2) // 3):
        Pi = np.zeros((32, 96))
        for m in range(3):
            c = 3 * j + m
            if c >= nch:
                break
            for s in range(16):
                t0 = 512 * c + 32 * s
                Pi[:, m * 32 + s] = L2[:, t0] * inv
                Pi[:, m * 32 + 16 + s] = (L2[:, t0 + 1] - L2[:, t0]) * inv
        pis.append(Pi.astype(np.float32))
    return pis


def _sub_amp_basis():
    bas = np.zeros((96, 512))
    i = np.arange(32)
    for rep in range(3):
        for s in range(16):
            cols = slice(32 * s, 32 * (s + 1))
            bas[rep * 32 + s, cols] = 1.0
            bas[rep * 32 + 16 + s, cols] = i
    return bas.astype(np.float32)


def _tridiag_M(u):
    M = np.zeros((32, 32))
    for f in range(32):
        M[max(f - 1, 0), f] += u / 8.0
        M[min(f + 1, 31), f] += u / 8.0
        M[f, f] += 3.0 * u / 4.0
    return M.astype(np.float32)


def _band_fir(bs):
    spf = bs // NNF
    nc_ = spf // 2 + 1
    t = np.arange(spf)
    j_re = np.arange(nc_)
    j_im = np.arange(1, nc_ - 1)
    FT = np.concatenate([np.cos(2 * np.pi * np.outer(t, j_re) / spf),
                         -np.sin(2 * np.pi * np.outer(t, j_im) / spf)], axis=1)
    w = np.full(nc_, 2.0)
    w[0] = 1.0
    w[-1] = 1.0
    IR = np.concatenate([
        (w[:, None] * np.cos(2 * np.pi * np.outer(j_re, t) / spf)) / spf,
        (-2.0 * np.sin(2 * np.pi * np.outer(j_im, t) / spf)) / spf,
    ], axis=0) / ADJUST[bs]
    return FT.astype(np.float32), IR.astype(np.float32)


def _build_U(n):
    eye = np.eye(n)
    spec = np.fft.rfft(eye, axis=-1)
    spec = np.pad(spec, ((0, 0), (0, n + 1 - spec.shape[-1])))
    return np.fft.irfft(spec, n=2 * n, axis=-1) * 2


def _mega_entries():
    ents = [('wlin', C + 1, 4 * C), ('ubd4', BL * 4, BL * 8), ('ubd8', BL * 8, BL * 16),
            ('ubd16', BL * 16, BL * 32), ('w64', C, len(W64_ORDER) * C),
            ('bias64', C, len(B64_ORDER)), ('ident128', 128, 128), ('ut', 32, 32)]
    for k in range(6):
        ents.append((f'wfrq{k}', C + 1, N_OSC))
        ents.append((f'wamp{k}', C + 1, N_OSC))
        ents.append((f'M{k}', 32, 32))
    ents += [('pi0', 64, 128), ('pia0', 32, 96), ('pi1', 64, 128), ('pia1', 32, 128)]
    for k in (2, 3, 4, 5):
        nch = NCH[k]
        for j in range(nch // 2):
            ents.append((f'pip{k}_{j}', 64, 128))
        for j in range((nch + 2) // 3):
            ents.append((f'piam{k}_{j}', 32, 96))
    ents += [('bas0', 128, 512), ('bas1', 128, 512), ('basS', 112, 512),
             ('basA0', 96, 512), ('basA1', 112, 512), ('basAS', 96, 512),
             ('selstrip', 128, 256), ('negI', 128, 128)]
    for k in range(6):
        spf = SPF[k]
        nc_ = spf // 2 + 1
        if k < 5:
            ents.append((f'wc{k}', C + 1, spf))
            ents.append((f'ft{k}', spf, spf))
            ents.append((f'ir{k}', spf, spf))
        else:
            ents.append(('wc5a', C + 1, 128))
            ents.append(('wc5b', C + 1, 128))
            ents.append(('ft5_0', 128, 256))
            ents.append(('ft5_1', 128, 256))
            ents.append(('ir5_0', 128, 256))
            ents.append(('ir5_1', 128, 256))
    off = {}
    o = 0
    for name, r, cd in ents:
        off[name] = (r, o, cd)
        o += cd
    return off, o


MEGA_OFF, MEGA_COLS = _mega_entries()


def _build_shared(inp):
    c = {}
    wl = np.zeros((4, C + 1, C), np.float32)
    for t in range(4):
        wl[t, :C] = inp['up_lin_w'][:, t::4]
        wl[t, C] = inp['up_lin_b'][t::4]
    c['wlin'] = wl.transpose(1, 0, 2).reshape(C + 1, 4 * C)
    for n in (4, 8, 16):
        U = _build_U(n)
        ub = np.zeros((BL * n, BL * 2 * n), np.float32)
        for b in range(BL):
            ub[b * n:(b + 1) * n, b * 2 * n:(b + 1) * 2 * n] = U
        c[f'ubd{n}'] = ub

    w64 = np.zeros((C, len(W64_ORDER) * C), np.float32)

    def put64(name, m):
        i = W64_IDX[name]
        w64[:, i * C:(i + 1) * C] = m

    for i in range(3):
        for dd in range(3):
            put64(f'up{i}d{dd}', inp['up_conv_w'][i, :, :, dd].T)
    for dd in range(3):
        put64(f'find{dd}', inp['up_final_w'][:, :, dd].T)
    for k in range(6):
        for j in range(4):
            put64(f't{k}{j}', inp['t_w'][k, j].T + np.eye(C, dtype=np.float32))
        put64(f'bf{k}', inp['band_final_w'][k].T)
        put64(f'nup{k}', inp['noise_up_w'][k].T)
    put64('ident', np.eye(C))
    c['w64'] = w64

    b64 = np.zeros((C, len(B64_ORDER)), np.float32)
    for i in range(3):
        b64[:, B64_IDX[f'up{i}']] = inp['up_conv_b'][i]
    b64[:, B64_IDX['fin']] = inp['up_final_b']
    for k in range(6):
        for j in range(4):
            b64[:, B64_IDX[f't{k}{j}']] = inp['t_b'][k, j]
        b64[:, B64_IDX[f'bf{k}']] = inp['band_final_b'][k]
        b64[:, B64_IDX[f'nup{k}']] = inp['noise_up_b'][k]
    c['bias64'] = b64
    c['ident128'] = np.eye(128, dtype=np.float32)

    sel = np.zeros((128, 256), np.float32)
    for b in range(BL):
        sel[b * N_OSC:(b + 1) * N_OSC, 128 + b] = 1.0
    c['selstrip'] = sel
    c['negI'] = (-np.eye(128)).astype(np.float32)
    c['ut'] = np.triu(np.ones((32, 32))).astype(np.float32)

    for k, bs in enumerate(BAND_SIZES):
        u = bs // 32
        wf = np.zeros((C + 1, N_OSC), np.float32)
        wf[:C] = inp['osc_freq_w'][k].T
        wf[C] = inp['osc_freq_b'][k]
        c[f'wfrq{k}'] = wf
        wa = np.zeros((C + 1, N_OSC), np.float32)
        wa[:C] = inp['osc_amp_w'][k].T
        wa[C] = inp['osc_amp_b'][k]
        c[f'wamp{k}'] = wa
        c[f'M{k}'] = _tridiag_M(u)

    c['pi0'] = _frame_phase_pi(16)
    c['pia0'] = _frame_amp_pi(16)
    c['pi1'] = _frame_phase_pi(32)
    c['pia1'] = _frame_amp_pi(32)
    for k in (2, 3, 4, 5):
        bs = BAND_SIZES[k]
        for j, Pi in enumerate(_sub_phase_pis(bs)):
            c[f'pip{k}_{j}'] = Pi
        for j, Pi in enumerate(_sub_amp_pis(bs, ADJUST[bs])):
            c[f'piam{k}_{j}'] = Pi
    c['bas0'] = _frame_phase_basis(16)
    c['bas1'] = _frame_phase_basis(32)
    c['basS'] = _sub_phase_basis()
    c['basA0'] = _frame_amp_basis(16, ADJUST[512])
    c['basA1'] = _frame_amp_basis(32, ADJUST[1024])
    c['basAS'] = _sub_amp_basis()

    for k, bs in enumerate(BAND_SIZES):
        spf = SPF[k]
        nc_ = spf // 2 + 1
        wcf = np.zeros((C + 1, spf), np.float32)
        wc = np.zeros((C + 1, nc_), np.float32)
        wc[:C] = inp[f'noise_coeff_w_{k}'].T
        wc[C] = inp[f'noise_coeff_b_{k}']
        if k == 0:
            wc[:, 1:] = 0.0
        wcf[:, 0:nc_] = wc
        wcf[:, nc_:spf] = wc[:, 1:nc_ - 1]
        FT, IR = _band_fir(bs)
        if k < 5:
            c[f'wc{k}'] = wcf
            c[f'ft{k}'] = FT
            c[f'ir{k}'] = IR
        else:
            c['wc5a'] = wcf[:, 0:128]
            c['wc5b'] = wcf[:, 128:256]
            c['ft5_0'] = FT[0:128]
            c['ft5_1'] = FT[128:256]
            c['ir5_0'] = IR[0:128]
            c['ir5_1'] = IR[128:256]

    mega = np.zeros((128, MEGA_COLS), np.float32)
    for name, (r, o, cd) in MEGA_OFF.items():
        mega[0:r, o:o + cd] = c[name]
    return {'mega': mega}


# ---------------------------------------------------------------- bass build
def _build_nc():
    nc = bacc.Bacc('TRN2', num_devices=NCORE)
    AF = mybir.ActivationFunctionType

    d = {}
    d['xT'] = nc.dram_tensor("xT", [C + 1, BL], F32, kind="ExternalInput")
    d['mega'] = nc.dram_tensor("mega", [128, MEGA_COLS], F32, kind="ExternalInput")
    for k in range(6):
        d[f'noiseT{k}'] = nc.dram_tensor(f"noiseT{k}", [SPF[k], FR], F32,
                                         kind="ExternalInput")
    harm_d = nc.dram_tensor("harm", [4 * 63, 512], F32, kind="ExternalOutput")
    nz_d = nc.dram_tensor("nz", [FR, int(NOFF[6])], F32, kind="ExternalOutput")


    with TileContext(nc) as tc:
        with tc.tile_pool(name="const", bufs=1) as cp, \
             tc.tile_pool(name="work", bufs=2) as wp, \
             tc.tile_pool(name="hot", bufs=3) as hot, \
             tc.tile_pool(name="phF", bufs=3, space="PSUM") as phF, \
             tc.tile_pool(name="phA", bufs=3, space="PSUM") as phA, \
             tc.tile_pool(name="phH", bufs=2, space="PSUM") as phH:

            def psF(p0, f0):
                t = phF.tile([128, 512], F32, tag="phF", name="psF")
                return t[0:p0, 0:f0]

            def psA(p0, f0):
                t = phA.tile([128, 512], F32, tag="phA", name="psA")
                return t[0:p0, 0:f0]

            def psH(p0, f0):
                t = phH.tile([128, 512], F32, tag="phH", name="psH")
                return t[0:p0, 0:f0]

            mega = cp.tile([128, MEGA_COLS], F32, tag="mega")
            _nsplit = 6
            _cut = [MEGA_COLS * i // _nsplit for i in range(_nsplit + 1)]
            for _i in range(_nsplit):
                nc.gpsimd.dma_start(out=mega[:, _cut[_i]:_cut[_i + 1]],
                                    in_=d['mega'][:, _cut[_i]:_cut[_i + 1]])

            ct = {}
            for name, (r, o, cd) in MEGA_OFF.items():
                ct[name] = mega[0:r, o:o + cd]
            for name in ('selstrip', 'negI', 'basA0', 'basA1', 'basAS'):
                r, o, cd = MEGA_OFF[name]
                t = cp.tile([r, cd], BF16, tag=f"bf_{name}")
                nc.gpsimd.dma_start(out=t, in_=d['mega'][0:r, o:o + cd])
                ct[f'{name}_bf'] = t
            for name in ('bas0', 'bas1', 'basS'):
                r, o, cd = MEGA_OFF[name]
                t = cp.tile([r, cd], F32R, tag=f"r_{name}")
                nc.gpsimd.dma_start(out=t, in_=d['mega'][0:r, o:o + cd])
                ct[f'{name}_r'] = t

            def w64s(name):
                i = W64_IDX[name]
                return ct['w64'][:, i * C:(i + 1) * C]

            def b64s(name):
                return ct['bias64'][:, B64_IDX[name]:B64_IDX[name] + 1]

            ident64 = w64s('ident')

            xT = cp.tile([C + 1, BL], F32, tag="xT")
            nc.sync.dma_start(out=xT, in_=d['xT'][:, :])

            # ---------------- frontend (as baseline)
            h = wp.tile([C, 16], F32, tag="h0")
            for t in range(4):
                pt = psF(C, BL)
                nc.tensor.matmul(out=pt, lhsT=ct['wlin'][:, t * C:(t + 1) * C],
                                 rhs=xT, start=True, stop=True)
                nc.vector.tensor_copy(out=h.rearrange("c (b t) -> c b t", t=4)[:, :, t],
                                      in_=pt)
            for i, n in enumerate((4, 8, 16)):
                pt1 = psF(BL * n, C)
                nc.tensor.transpose(out=pt1, in_=h, identity=ident64)
                t1 = wp.tile([BL * n, C], F32, tag=f"fe_t1_{i}")
                nc.vector.tensor_copy(out=t1, in_=pt1)
                pt2 = psF(BL * 2 * n, C)
                nc.tensor.matmul(out=pt2, lhsT=ct[f'ubd{n}'], rhs=t1, start=True,
                                 stop=True)
                t2 = wp.tile([BL * 2 * n, C], F32, tag=f"fe_t2_{i}")
                nc.vector.tensor_copy(out=t2, in_=pt2)
                pt3 = psF(C, BL * 2 * n)
                nc.tensor.transpose(out=pt3, in_=t2,
                                    identity=ct['ident128'][0:BL * 2 * n, 0:BL * 2 * n])
                hu = wp.tile([C, BL * 2 * n], F32, tag=f"fe_hu_{i}")
                nc.vector.tensor_copy(out=hu, in_=pt3)
                m = 2 * n
                hu3 = hu.rearrange("c (b t) -> c b t", b=BL)
                pc = psF(C, BL * m).rearrange("c (b t) -> c b t", b=BL)
                nc.tensor.matmul(out=pc[:, :, :], lhsT=w64s(f'up{i}d1'), rhs=hu3[:, :, :],
                                 start=True, stop=False)
                nc.tensor.matmul(out=pc[:, :, 1:m], lhsT=w64s(f'up{i}d0'),
                                 rhs=hu3[:, :, 0:m - 1], start=False, stop=False)
                nc.tensor.matmul(out=pc[:, :, 0:m - 1], lhsT=w64s(f'up{i}d2'),
                                 rhs=hu3[:, :, 1:m], start=False, stop=True)
                h = wp.tile([C, BL * m], F32, tag=f"fe_h_{i}")
                nc.scalar.activation(out=h.rearrange("c (b t) -> c b t", b=BL), in_=pc,
                                     func=AF.Prelu, bias=b64s(f'up{i}'), scale=1.0,
                                     alpha=0.2)
            h3 = h.rearrange("c (b t) -> c b t", b=BL)
            pf = psF(C, BL * 32).rearrange("c (b t) -> c b t", b=BL)
            nc.tensor.matmul(out=pf[:, :, :], lhsT=w64s('find1'), rhs=h3[:, :, :],
                             start=True, stop=False)
            nc.tensor.matmul(out=pf[:, :, 1:32], lhsT=w64s('find0'), rhs=h3[:, :, 0:31],
                             start=False, stop=False)
            nc.tensor.matmul(out=pf[:, :, 0:31], lhsT=w64s('find2'), rhs=h3[:, :, 1:32],
                             start=False, stop=True)
            hfin = cp.tile([C, 128], F32, tag="hfin")
            nc.scalar.activation(out=hfin.rearrange("c (b t) -> c b t", b=BL), in_=pf,
                                 func=AF.Identity, bias=b64s('fin'), scale=1.0)

            # ---------------- per-band setup
            nTs_all = {}
            for k in range(6):
                if k < 5:
                    nT = wp.tile([SPF[k], FR], F32, tag=f"nT{k}", name="nT", bufs=1)
                    nc.sync.dma_start(out=nT, in_=d[f'noiseT{k}'][:, :])
                    nTs_all[k] = (nT,)
                else:
                    nT0 = wp.tile([128, FR], F32, tag="nT50", bufs=1)
                    nT1 = wp.tile([128, FR], F32, tag="nT51", bufs=1)
                    nc.sync.dma_start(out=nT0, in_=d['noiseT5'][0:128, :])
                    nc.sync.dma_start(out=nT1, in_=d['noiseT5'][128:256, :])
                    nTs_all[5] = (nT0, nT1)
            def noise_branch(k):
                bs = BAND_SIZES[k]
                spf = SPF[k]
                    zf3 = zfas[k][0:C, :].rearrange("c (b t) -> c b t", b=BL)
                    zrep = zf3.unsqueeze(-1).broadcast_to([C, BL, 32, 2])
                    pn = psA(C, FR)
                    nc.tensor.matmul(out=pn, lhsT=w64s(f'nup{k}'), rhs=zrep,
                                         start=True, stop=True)
                    naug = wp.tile([C + 1, FR], F32, tag="naug")
                    nc.scalar.activation(out=naug[0:C, :], in_=pn, func=AF.Prelu,
                                             bias=b64s(f'nup{k}'), scale=1.0, alpha=0.2)
                    nc.vector.memset(naug[C:C + 1, :], 1.0)

                    if k < 5:
                        nT = nTs_all[k][0]
                        pcA = psH(spf, FR)
                        nc.tensor.matmul(out=pcA, lhsT=ct[f'wc{k}'], rhs=naug,
                                             start=True, stop=True)
                        chat = wp.tile([spf, FR], F32, tag="chat")
                        nc.scalar.copy(out=chat, in_=pcA)
                        psp = psH(spf, FR)
                        nc.tensor.matmul(out=psp, lhsT=ct[f'ft{k}'], rhs=nT,
                                             start=True, stop=True)
                        sA = wp.tile([spf, FR], F32, tag="sA")
                        nc.vector.tensor_tensor(out=sA, in0=chat, in1=psp, op=ALU.mult)
                        sAs = [sA]
                    else:
                        nT0, nT1 = nTs_all[5]
                        sAs = []
                        for half, wch in ((0, 'wc5a'), (1, 'wc5b')):
                            pcA = psH(128, FR)
                            nc.tensor.matmul(out=pcA, lhsT=ct[wch], rhs=naug,
                                                 start=True, stop=True)
                            chat = wp.tile([128, FR], F32, tag=f"chat5{half}")
                            nc.scalar.copy(out=chat, in_=pcA)
                            psp = psH(128, FR)
                            nc.tensor.matmul(out=psp,
                                                 lhsT=ct['ft5_0'][:, 128 * half:128 * (half + 1)],
                                                 rhs=nT0, start=True, stop=False)
                            nc.tensor.matmul(out=psp,
                                                 lhsT=ct['ft5_1'][:, 128 * half:128 * (half + 1)],
                                                 rhs=nT1, start=False, stop=True)
                            sA = wp.tile([128, FR], F32, tag=f"sA5{half}")
                            nc.vector.tensor_tensor(out=sA, in0=chat, in1=psp, op=ALU.mult)
                            sAs.append(sA)

                    for fg in range(2):
                        pnz = psA(128, spf)
                        if k < 5:
                            nc.tensor.matmul(out=pnz,
                                                 lhsT=sAs[0][:, 128 * fg:128 * (fg + 1)],
                                                 rhs=ct[f'ir{k}'], start=True, stop=True)
                        else:
                            nc.tensor.matmul(out=pnz,
                                                 lhsT=sAs[0][:, 128 * fg:128 * (fg + 1)],
                                                 rhs=ct['ir5_0'], start=True, stop=False)
                            nc.tensor.matmul(out=pnz,
                                                 lhsT=sAs[1][:, 128 * fg:128 * (fg + 1)],
                                                 rhs=ct['ir5_1'], start=False, stop=True)
                        nzs = wp.tile([128, spf], F32, tag="nzs")
                        nc.scalar.copy(out=nzs, in_=pnz)
                        nc.sync.dma_start(
                            out=nz_d[128 * fg:128 * (fg + 1),
                                         int(NOFF[k]):int(NOFF[k]) + spf],
                            in_=nzs)


            stacks = {}
            astacks = {}
            zfas = {}
            srcs = {}
            ampgs = {}
            LFS = [0.05 if bs == 512 else 0.01 for bs in BAND_SIZES]

            # pass 1: residual stacks, layer-interleaved across bands
            zs = {k: hfin for k in range(6)}
            for j in range(4):
                for k in range(6):
                    pz = psF(C, 128)
                    nc.tensor.matmul(out=pz, lhsT=w64s(f't{k}{j}'), rhs=zs[k],
                                     start=True, stop=True)
                    z = wp.tile([C, 128], F32, tag=f"z{k}_{j % 2}", name="z", bufs=1)
                    nc.scalar.activation(out=z, in_=pz, func=AF.Prelu,
                                         bias=b64s(f't{k}{j}'), scale=1.0, alpha=0.2)
                    zs[k] = z
            for k in range(6):
                pz = psF(C, 128)
                nc.tensor.matmul(out=pz, lhsT=w64s(f'bf{k}'), rhs=zs[k],
                                 start=True, stop=True)
                zfa = wp.tile([C + 1, 128], F32, tag=f"zfa{k}", name="zfa", bufs=1)
                nc.scalar.activation(out=zfa[0:C, :], in_=pz, func=AF.Identity,
                                     bias=b64s(f'bf{k}'), scale=1.0)
                nc.vector.memset(zfa[C:C + 1, :], 1.0)
                zfas[k] = zfa

            # pass 2: freq/amp grids
            for k in range(6):
                zfa = zfas[k]
                pgF = psH(N_OSC, 128)
                for b in range(BL):
                    nc.tensor.matmul(out=pgF[:, 32 * b:32 * (b + 1)],
                                     lhsT=zfa[:, 32 * b:32 * (b + 1)],
                                     rhs=ct[f'wfrq{k}'], start=True, stop=True)
                sig = wp.tile([N_OSC, 128], F32, tag=f"sig{k}", name="sig", bufs=1)
                nc.scalar.activation(out=sig, in_=pgF, func=AF.Sigmoid, scale=1.0)
                src = wp.tile([64, 128], F32, tag=f"src{k}", name="src", bufs=1)
                nc.vector.tensor_scalar(out=src[0:32, :], in0=sig,
                                        scalar1=float((1.0 - LFS[k]) / 2.0),
                                        scalar2=float(LFS[k] / 2.0),
                                        op0=ALU.mult, op1=ALU.add)
                srcs[k] = src
                pgA = psH(N_OSC, 128)
                for b in range(BL):
                    nc.tensor.matmul(out=pgA[:, 32 * b:32 * (b + 1)],
                                     lhsT=zfa[:, 32 * b:32 * (b + 1)],
                                     rhs=ct[f'wamp{k}'], start=True, stop=True)
                ampg = wp.tile([N_OSC, 128], F32, tag=f"ampg{k}", name="ampg", bufs=1)
                nc.scalar.activation(out=ampg, in_=pgA, func=AF.Abs, scale=1.0)
                ampgs[k] = ampg

            # pass 3: frame carries
            Srs = {}
            for k in range(6):
                pS = psH(N_OSC, 128)
                nc.tensor.matmul(out=pS, lhsT=ct[f'M{k}'], rhs=srcs[k][0:32, :],
                                 start=True, stop=True)
                rndS = wp.tile([N_OSC, 128], F32, tag=f"rndS{k}", name="rndS", bufs=1)
                nc.vector.tensor_scalar(out=rndS, in0=pS, scalar1=MAGIC,
                                        scalar2=MAGIC, op0=ALU.add,
                                        op1=ALU.subtract)
                Sr = wp.tile([N_OSC, 128], F32, tag=f"Sr{k}", name="Sr", bufs=1)
                nc.vector.tensor_tensor(out=Sr, in0=pS, in1=rndS, op=ALU.subtract)
                Srs[k] = Sr
            for k in range(6):
                pP = psH(N_OSC, 128)
                nc.tensor.matmul(out=pP, lhsT=ct['ut'], rhs=Srs[k], start=True,
                                 stop=True)
                rndP = wp.tile([N_OSC, 128], F32, tag=f"rndP{k}", name="rndP", bufs=1)
                nc.vector.tensor_scalar(out=rndP, in0=pP, scalar1=MAGIC,
                                        scalar2=MAGIC, op0=ALU.add,
                                        op1=ALU.subtract)
                nc.vector.tensor_tensor(out=srcs[k][32:64, :], in0=pP, in1=rndP,
                                        op=ALU.subtract)

            # pass 4: per-band stacks + noise
            for k, bs in enumerate(BAND_SIZES):
                nch = NCH[k]
                src = srcs[k]
                ampg = ampgs[k]
                if k == 0:
                    stk = cp.tile([128, 128], F32, tag="stk0")
                    pb = psF(128, 128)
                    nc.tensor.matmul(out=pb, lhsT=ct['pi0'], rhs=src, start=True,
                                     stop=True)
                    nc.scalar.copy(out=stk, in_=pb)
                elif k == 1:
                    stk = cp.tile([128, 128], F32, tag="stk1")
                    pb = psF(128, 128)
                    nc.tensor.matmul(out=pb, lhsT=ct['pi1'], rhs=src, start=True,
                                     stop=True)
                    nc.scalar.copy(out=stk, in_=pb)
                else:
                    stk = cp.tile([128, (nch // 2) * 128], F32, tag=f"stk{k}")
                    nblkp = nch // 2
                    for j0 in range(0, nblkp, 4):
                        jn = min(4, nblkp - j0)
                        pb = psF(128, 128 * jn)
                        for jj in range(jn):
                            nc.tensor.matmul(out=pb[:, 128 * jj:128 * (jj + 1)],
                                             lhsT=ct[f'pip{k}_{j0 + jj}'], rhs=src,
                                             start=True, stop=True)
                        rnd = wp.tile([128, 512], F32, tag="rndB")
                        nc.vector.tensor_scalar(out=rnd[:, 0:128 * jn], in0=pb,
                                                scalar1=MAGIC, scalar2=MAGIC,
                                                op0=ALU.add, op1=ALU.subtract)
                        nc.vector.tensor_tensor(
                            out=stk[:, 128 * j0:128 * (j0 + jn)],
                            in0=pb, in1=rnd[:, 0:128 * jn], op=ALU.subtract)
                if k == 0:
                    stkr = cp.tile([128, 128], F32R, tag="stkr0", name="stkr")
                elif k == 1:
                    stkr = cp.tile([128, 128], F32R, tag="stkr1", name="stkr")
                else:
                    stkr = cp.tile([128, (nch // 2) * 128], F32R, tag=f"stkr{k}",
                                   name="stkr")
                nc.gpsimd.dma_start(out=stkr, in_=stk)
                stacks[k] = stkr

                if k == 0:
                    ast = cp.tile([96, 128], BF16, tag="ast0")
                    pb = psF(96, 128)
                    nc.tensor.matmul(out=pb, lhsT=ct['pia0'], rhs=ampg, start=True,
                                     stop=True)
                    nc.scalar.copy(out=ast, in_=pb)
                elif k == 1:
                    ast = cp.tile([128, 128], BF16, tag="ast1")
                    pb = psF(128, 128)
                    nc.tensor.matmul(out=pb, lhsT=ct['pia1'], rhs=ampg, start=True,
                                     stop=True)
                    nc.scalar.copy(out=ast, in_=pb)
                else:
                    nblk = (nch + 2) // 3
                    ast = cp.tile([96, nblk * 128], BF16, tag=f"ast{k}")
                    for j0 in range(0, nblk, 4):
                        jn = min(4, nblk - j0)
                        pb = psF(96, 128 * jn)
                        for jj in range(jn):
                            nc.tensor.matmul(out=pb[:, 128 * jj:128 * (jj + 1)],
                                             lhsT=ct[f'piam{k}_{j0 + jj}'], rhs=ampg,
                                             start=True, stop=True)
                        nc.scalar.copy(out=ast[:, 128 * j0:128 * (j0 + jn)], in_=pb)
                astacks[k] = ast
                noise_branch(k)

            # ---------------- chunk loop (per-chunk, deep software pipeline)
            chunks = []
            for k in (1, 2, 3, 4, 5, 0):
                nch = NCH[k]
                for gstart in range(0, nch, 8):
                    gs = min(8, nch - gstart)
                    for cc in range(gs):
                        chunks.append((k, gstart + cc, cc, gs))
            Nc = len(chunks)
            state = {}

            def stage_a(i):
                k, c, cc, gs = chunks[i]
                ppt = phF.tile([128, 512], F32, tag="phF", name="ppt")
                if k == 0:
                    nc.tensor.matmul(out=ppt, lhsT=stacks[0], rhs=ct['bas0_r'],
                                     start=True, stop=False)
                elif k == 1:
                    nc.tensor.matmul(out=ppt, lhsT=stacks[1][64 * c:64 * c + 64],
                                     rhs=ct['bas1_r'][64 * c:64 * c + 64],
                                     start=True, stop=False)
                else:
                    j, par = c // 2, c % 2
                    nc.tensor.matmul(
                        out=ppt,
                        lhsT=stacks[k][64 * par:64 * par + 48,
                                       128 * j:128 * (j + 1)],
                        rhs=ct['basS_r'][64 * par:64 * par + 48],
                        start=True, stop=False)
                pat = phA.tile([128, 512], F32, tag="phA", name="pat")
                if k == 0:
                    nc.tensor.matmul(out=pat, lhsT=astacks[0],
                                     rhs=ct['basA0_bf'], start=True, stop=True)
                elif k == 1:
                    nc.tensor.matmul(out=pat, lhsT=astacks[1][64 * c:64 * c + 48],
                                     rhs=ct['basA1_bf'][64 * c:64 * c + 48],
                                     start=True, stop=True)
                else:
                    ja, ma = c // 3, c % 3
                    nc.tensor.matmul(
                        out=pat,
                        lhsT=astacks[k][32 * ma:32 * ma + 32,
                                        128 * ja:128 * (ja + 1)],
                        rhs=ct['basAS_bf'][32 * ma:32 * ma + 32],
                        start=True, stop=True)
                ntile = hot.tile([128, 512], BF16, tag="ntile")
                nc.vector.tensor_scalar(out=ntile, in0=ppt, scalar1=MAGIC,
                                        scalar2=MAGIC, op0=ALU.add,
                                        op1=ALU.subtract)
                if i % 2 == 0:
                    pat_sb = hot.tile([128, 512], BF16, tag="pat_sb")
                    nc.scalar.copy(out=pat_sb, in_=pat)
                else:
                    pat_sb = None
                state[i] = (ppt, pat, ntile, pat_sb)

            def stage_b(i):
                ppt, pat, ntile, pat_sb = state[i]
                nc.tensor.matmul(out=ppt, lhsT=ct['negI_bf'], rhs=ntile,
                                 start=False, stop=True)
                s = hot.tile([128, 512], BF16, tag="sin_t")
                nc.scalar.activation(out=s, in_=ppt, func=AF.Sin, scale=TWO_PI)
                prod = hot.tile([128, 512], BF16, tag="prod_t")
                nc.vector.tensor_tensor(out=prod, in0=s,
                                        in1=pat_sb if pat_sb is not None else pat,
                                        op=ALU.mult)
                state[i] = prod

            hpt_cur = [None]

            def stage_c(i):
                k, c, cc, gs = chunks[i]
                prod = state.pop(i)
                if cc == 0:
                    hpt_cur[0] = phH.tile([128, 512], F32, tag="phH", name="hpt")
                hpt = hpt_cur[0]
                nc.tensor.matmul(out=hpt,
                                 lhsT=ct['selstrip_bf'][:, 128 - 4 * cc:256 - 4 * cc],
                                 rhs=prod, start=(cc == 0), stop=(cc == gs - 1))
                if cc == gs - 1:
                    hsb = wp.tile([32, 512], F32, tag="hsb")
                    nc.scalar.copy(out=hsb[0:4 * gs], in_=hpt[0:4 * gs])
                    g0 = CHUNK_BASE[k] + c - (gs - 1)
                    nc.sync.dma_start(
                        out=harm_d[4 * int(g0):4 * int(g0) + 4 * gs, :],
                        in_=hsb[0:4 * gs])

            for i in range(Nc + 4):
                if i < Nc:
                    stage_a(i)
                if 2 <= i < Nc + 2:
                    stage_b(i - 2)
                if 4 <= i < Nc + 4:
                    stage_c(i - 4)

    nc.finalize()
    return nc


# ---------------------------------------------------------------- host glue
def _prep_inputs(inputs):
    inp = {k: np.asarray(v, np.float32) for k, v in inputs.items()}
    shared = _build_shared(inp)
    in_maps = []
    for core in range(NCORE):
        m = dict(shared)
        sl = slice(core * BL, (core + 1) * BL)
        m['xT'] = np.concatenate([inp['x'][sl].T, np.ones((1, BL), np.float32)],
                                 axis=0)
        for k in range(6):
            nT = inp[f'noise_{k}'][sl].reshape(FR, SPF[k]).T
            m[f'noiseT{k}'] = np.ascontiguousarray(nT, dtype=np.float32)
        in_maps.append(m)
    return in_maps


def kernel(**inputs):
    if 'nc' not in _nc_cache:
        _nc_cache['nc'] = _build_nc()
    nc = _nc_cache['nc']
    in_maps = _prep_inputs(inputs)
    res = run_bass_kernel_spmd(nc, in_maps, list(range(NCORE)))
    out = np.zeros((B, TOTAL), np.float32)
    for core in range(NCORE):
        r = res.results[core]
        harm = np.asarray(r['harm'], np.float32).reshape(63, BL, 512)
        nz = np.asarray(r['nz'], np.float32).reshape(BL, NNF, int(NOFF[6]))
        for k, bs in enumerate(BAND_SIZES):
            nch = NCH[k]
            hb = harm[CHUNK_BASE[k]:CHUNK_BASE[k] + nch]  # [nch, BL, 512]
            hb = hb.transpose(1, 0, 2).reshape(BL, bs)
            out[core * BL:(core + 1) * BL,
                HARM_OFF[k]:HARM_OFF[k] + bs] = hb
            nzb = nz[:, :, int(NOFF[k]):int(NOFF[k]) + SPF[k]].reshape(BL, bs)
            out[core * BL:(core + 1) * BL, NZ_OFF[k]:NZ_OFF[k] + bs] = nzb
    return out.astype(np.float32)


if __name__ == "__main__":
    import reference
    inp = reference.setup_inputs()
    out = kernel(**{k: np.asarray(v) for k, v in inp.items()})
    print("out", out.shape, out.dtype)


# revision 36
# speedup vs baseline: 1.2662x; 1.0110x over previous
"""Trainium2 Bass kernel for nn_Decoder (DDSP-style decoder) — redesigned.

Data-parallel over batch (32 -> 4 per core x 8 cores). Harmonic synthesis
uses per-frame (bands 0-1) / per-32-sample-subframe quadratic (bands 2-5)
phase bases so every chunk matmul's magnitudes stay small enough for the
fast float32r PE path. Grid coefficients are produced directly in
[frame, (batch,osc)] layout by operand-swapped matmuls and reorganized into
per-chunk lhsT stacks by constant permutation matmuls (no DRAM scratch, no
gather DMAs). The noise branch is real-DFT basis matmuls in float32r on
host-transposed noise.
"""
import numpy as np
import sys

sys.path.insert(0, "/opt/trn_rl_repo")

from concourse import bacc, mybir  # noqa: E402
from concourse.tile import TileContext  # noqa: E402
from concourse.bass_utils import run_bass_kernel_spmd  # noqa: E402

F32 = mybir.dt.float32
F32R = mybir.dt.float32r
BF16 = mybir.dt.bfloat16
ALU = mybir.AluOpType
BAND_SIZES = [512, 1024, 2048, 4096, 8192, 16384]
ADJUST = {512: 0.05, 1024: 0.03, 2048: 0.05, 4096: 0.25, 8192: 1.0, 16384: 20.0}
B, C, N_OSC, NNF = 32, 64, 32, 64
NCORE = 8
BL = B // NCORE
FR = BL * NNF
MAGIC = float(1.5 * 2 ** 23)
TWO_PI = float(2 * np.pi)
TOTAL = 2 * sum(BAND_SIZES)

SPF = [bs // NNF for bs in BAND_SIZES]            # 8..256
NOFF = np.concatenate([[0], np.cumsum(SPF)]).astype(int)   # noise col offsets
NCH = [bs // 512 for bs in BAND_SIZES]            # 1,2,4,8,16,32
CHUNK_BASE = np.concatenate([[0], np.cumsum(NCH)]).astype(int)
HARM_OFF = {}
NZ_OFF = {}
_off = 0
for _k, _bs in enumerate(BAND_SIZES):
    HARM_OFF[_k] = _off
    NZ_OFF[_k] = _off + _bs
    _off += 2 * _bs

_nc_cache = {}

W64_ORDER = ([f'up{i}d{dd}' for i in range(3) for dd in range(3)]
             + [f'find{dd}' for dd in range(3)]
             + [w for k in range(6) for w in
                [f't{k}0', f't{k}1', f't{k}2', f't{k}3', f'bf{k}', f'nup{k}']]
             + ['ident'])
W64_IDX = {n: i for i, n in enumerate(W64_ORDER)}
B64_ORDER = ([f'up{i}' for i in range(3)] + ['fin']
             + [b for k in range(6) for b in
                [f't{k}0', f't{k}1', f't{k}2', f't{k}3', f'bf{k}', f'nup{k}']])
B64_IDX = {n: i for i, n in enumerate(B64_ORDER)}


# ---------------------------------------------------------------- host math
def _band_L2(bs):
    t = np.arange(bs)
    pos = (t + 0.5) * (32.0 / bs) - 0.5
    lo = np.clip(np.floor(pos).astype(int), 0, 31)
    hi = np.clip(lo + 1, 0, 31)
    w = np.clip(pos - lo, 0.0, 1.0)
    L2 = np.zeros((32, bs))
    np.add.at(L2, (lo, t), 1.0 - w)
    np.add.at(L2, (hi, t), w)
    return L2


def _interp_vecs(u):
    r = np.arange(u)
    f = (r + 0.5) / u - 0.5
    gm = np.where(r < u // 2, -f, 0.0)
    g0 = np.where(r < u // 2, 1 + f, 1 - f)
    gp = np.where(r >= u // 2, f, 0.0)
    return gm, g0, gp


def _frame_phase_pi(u):
    Pi = np.zeros((64, 128))
    nq = 512 // u
    nslot = 1 if u == 16 else 2
    for cs in range(nslot):
        for g in range(4):
            for q in range(nq):
                row = cs * 64 + g * nq + q
                f = cs * nq + q
                if g == 0:
                    Pi[max(f - 1, 0), row] = 1.0
                elif g == 1:
                    Pi[f, row] = 1.0
                elif g == 2:
                    Pi[min(f + 1, 31), row] = 1.0
                elif f > 0:
                    Pi[32 + f - 1, row] = 1.0
    return Pi.astype(np.float32)


def _frame_phase_basis(u):
    gm, g0, gp = _interp_vecs(u)
    Gm, G0, Gp = np.cumsum(gm), np.cumsum(g0), np.cumsum(gp)
    nq = 512 // u
    bas = np.zeros((128, 512))
    nrep = 1 if u == 16 else 2
    for rep in range(nrep):
        for q in range(nq):
            cols = slice(q * u, (q + 1) * u)
            bas[rep * 64 + 0 * nq + q, cols] = Gm
            bas[rep * 64 + 1 * nq + q, cols] = G0
            bas[rep * 64 + 2 * nq + q, cols] = Gp
            bas[rep * 64 + 3 * nq + q, cols] = 1.0
    return bas.astype(np.float32)


def _frame_amp_pi(u):
    nq = 512 // u
    if u == 16:
        Pi = np.zeros((32, 96))
        for g in range(3):
            for q in range(nq):
                src = max(q - 1, 0) if g == 0 else (q if g == 1 else min(q + 1, 31))
                Pi[src, g * nq + q] = 1.0
    else:
        Pi = np.zeros((32, 128))
        for cs in range(2):
            for g in range(3):
                for q in range(nq):
                    f = cs * nq + q
                    src = max(f - 1, 0) if g == 0 else (f if g == 1 else min(f + 1, 31))
                    Pi[src, cs * 64 + g * nq + q] = 1.0
    return Pi.astype(np.float32)


def _frame_amp_basis(u, adj):
    gm, g0, gp = _interp_vecs(u)
    inv = 1.0 / adj
    nq = 512 // u
    rows = 96 if u == 16 else 112
    bas = np.zeros((rows, 512))
    nrep = 1 if u == 16 else 2
    for rep in range(nrep):
        for q in range(nq):
            cols = slice(q * u, (q + 1) * u)
            bas[rep * 64 + 0 * nq + q, cols] = gm * inv
            bas[rep * 64 + 1 * nq + q, cols] = g0 * inv
            bas[rep * 64 + 2 * nq + q, cols] = gp * inv
    return bas.astype(np.float32)


def _sub_phase_pis(bs):
    u = bs // 32
    L2 = _band_L2(bs)
    CW = np.zeros_like(L2)
    for F in range(32):
        cols = slice(F * u, (F + 1) * u)
        CW[:, cols] = np.cumsum(L2[:, cols], axis=1)
    pis = []
    for j in range(bs // 1024):
        Pi = np.zeros((64, 128))
        for par in range(2):
            c = 2 * j + par
            for s in range(16):
                t0 = 512 * c + 32 * s
                F = t0 // u
                if t0 % u != 0:
                    Pi[0:32, par * 64 + s] = CW[:, t0 - 1]
                if F >= 1:
                    Pi[32 + F - 1, par * 64 + s] = 1.0
                Pi[0:32, par * 64 + 16 + s] = L2[:, t0]
                Pi[0:32, par * 64 + 32 + s] = L2[:, t0 + 1] - L2[:, t0]
        pis.append(Pi.astype(np.float32))
    return pis


def _sub_phase_basis():
    bas = np.zeros((112, 512))
    i = np.arange(32)
    for rep in range(2):
        for s in range(16):
            cols = slice(32 * s, 32 * (s + 1))
            bas[rep * 64 + s, cols] = 1.0
            bas[rep * 64 + 16 + s, cols] = i + 1
            bas[rep * 64 + 32 + s, cols] = i * (i + 1) / 2.0
    return bas.astype(np.float32)


def _sub_amp_pis(bs, adj):
    L2 = _band_L2(bs)
    inv = 1.0 / adj
    nch = bs // 512
    pis = []
    for j in range((nch + 2) // 3):
        Pi = np.zeros((32, 96))
        for m in range(3):
            c = 3 * j + m
            if c >= nch:
                break
            for s in range(16):
                t0 = 512 * c + 32 * s
                Pi[:, m * 32 + s] = L2[:, t0] * inv
                Pi[:, m * 32 + 16 + s] = (L2[:, t0 + 1] - L2[:, t0]) * inv
        pis.append(Pi.astype(np.float32))
    return pis


def _sub_amp_basis():
    bas = np.zeros((96, 512))
    i = np.arange(32)
    for rep in range(3):
        for s in range(16):
            cols = slice(32 * s, 32 * (s + 1))
            bas[rep * 32 + s, cols] = 1.0
            bas[rep * 32 + 16 + s, cols] = i
    return bas.astype(np.float32)


def _tridiag_M(u):
    M = np.zeros((32, 32))
    for f in range(32):
        M[max(f - 1, 0), f] += u / 8.0
        M[min(f + 1, 31), f] += u / 8.0
        M[f, f] += 3.0 * u / 4.0
    return M.astype(np.float32)


def _band_fir(bs):
    spf = bs // NNF
    nc_ = spf // 2 + 1
    t = np.arange(spf)
    j_re = np.arange(nc_)
    j_im = np.arange(1, nc_ - 1)
    FT = np.concatenate([np.cos(2 * np.pi * np.outer(t, j_re) / spf),
                         -np.sin(2 * np.pi * np.outer(t, j_im) / spf)], axis=1)
    w = np.full(nc_, 2.0)
    w[0] = 1.0
    w[-1] = 1.0
    IR = np.concatenate([
        (w[:, None] * np.cos(2 * np.pi * np.outer(j_re, t) / spf)) / spf,
        (-2.0 * np.sin(2 * np.pi * np.outer(j_im, t) / spf)) / spf,
    ], axis=0) / ADJUST[bs]
    return FT.astype(np.float32), IR.astype(np.float32)


def _build_U(n):
    eye = np.eye(n)
    spec = np.fft.rfft(eye, axis=-1)
    spec = np.pad(spec, ((0, 0), (0, n + 1 - spec.shape[-1])))
    return np.fft.irfft(spec, n=2 * n, axis=-1) * 2


def _mega_entries():
    ents = [('wlin', C + 1, 4 * C), ('ubd4', BL * 4, BL * 8), ('ubd8', BL * 8, BL * 16),
            ('ubd16', BL * 16, BL * 32), ('w64', C, len(W64_ORDER) * C),
            ('bias64', C, len(B64_ORDER)), ('ident128', 128, 128), ('ut', 32, 32)]
    for k in range(6):
        ents.append((f'wfrq{k}', C + 1, N_OSC))
        ents.append((f'wamp{k}', C + 1, N_OSC))
        ents.append((f'M{k}', 32, 32))
    ents += [('pi0', 64, 128), ('pia0', 32, 96), ('pi1', 64, 128), ('pia1', 32, 128)]
    for k in (2, 3, 4, 5):
        nch = NCH[k]
        for j in range(nch // 2):
            ents.append((f'pip{k}_{j}', 64, 128))
        for j in range((nch + 2) // 3):
            ents.append((f'piam{k}_{j}', 32, 96))
    ents += [('bas0', 128, 512), ('bas1', 128, 512), ('basS', 112, 512),
             ('basA0', 96, 512), ('basA1', 112, 512), ('basAS', 96, 512),
             ('selstrip', 128, 256), ('negI', 128, 128)]
    for k in range(6):
        spf = SPF[k]
        nc_ = spf // 2 + 1
        if k < 5:
            ents.append((f'wc{k}', C + 1, spf))
            ents.append((f'ft{k}', spf, spf))
            ents.append((f'ir{k}', spf, spf))
        else:
            ents.append(('wc5a', C + 1, 128))
            ents.append(('wc5b', C + 1, 128))
            ents.append(('ft5_0', 128, 256))
            ents.append(('ft5_1', 128, 256))
            ents.append(('ir5_0', 128, 256))
            ents.append(('ir5_1', 128, 256))
    off = {}
    o = 0
    for name, r, cd in ents:
        off[name] = (r, o, cd)
        o += cd
    return off, o


MEGA_OFF, MEGA_COLS = _mega_entries()


def _build_shared(inp):
    c = {}
    wl = np.zeros((4, C + 1, C), np.float32)
    for t in range(4):
        wl[t, :C] = inp['up_lin_w'][:, t::4]
        wl[t, C] = inp['up_lin_b'][t::4]
    c['wlin'] = wl.transpose(1, 0, 2).reshape(C + 1, 4 * C)
    for n in (4, 8, 16):
        U = _build_U(n)
        ub = np.zeros((BL * n, BL * 2 * n), np.float32)
        for b in range(BL):
            ub[b * n:(b + 1) * n, b * 2 * n:(b + 1) * 2 * n] = U
        c[f'ubd{n}'] = ub

    w64 = np.zeros((C, len(W64_ORDER) * C), np.float32)

    def put64(name, m):
        i = W64_IDX[name]
        w64[:, i * C:(i + 1) * C] = m

    for i in range(3):
        for dd in range(3):
            put64(f'up{i}d{dd}', inp['up_conv_w'][i, :, :, dd].T)
    for dd in range(3):
        put64(f'find{dd}', inp['up_final_w'][:, :, dd].T)
    for k in range(6):
        for j in range(4):
            put64(f't{k}{j}', inp['t_w'][k, j].T + np.eye(C, dtype=np.float32))
        put64(f'bf{k}', inp['band_final_w'][k].T)
        put64(f'nup{k}', inp['noise_up_w'][k].T)
    put64('ident', np.eye(C))
    c['w64'] = w64

    b64 = np.zeros((C, len(B64_ORDER)), np.float32)
    for i in range(3):
        b64[:, B64_IDX[f'up{i}']] = inp['up_conv_b'][i]
    b64[:, B64_IDX['fin']] = inp['up_final_b']
    for k in range(6):
        for j in range(4):
            b64[:, B64_IDX[f't{k}{j}']] = inp['t_b'][k, j]
        b64[:, B64_IDX[f'bf{k}']] = inp['band_final_b'][k]
        b64[:, B64_IDX[f'nup{k}']] = inp['noise_up_b'][k]
    c['bias64'] = b64
    c['ident128'] = np.eye(128, dtype=np.float32)

    sel = np.zeros((128, 256), np.float32)
    for b in range(BL):
        sel[b * N_OSC:(b + 1) * N_OSC, 128 + b] = 1.0
    c['selstrip'] = sel
    c['negI'] = (-np.eye(128)).astype(np.float32)
    c['ut'] = np.triu(np.ones((32, 32))).astype(np.float32)

    for k, bs in enumerate(BAND_SIZES):
        u = bs // 32
        wf = np.zeros((C + 1, N_OSC), np.float32)
        wf[:C] = inp['osc_freq_w'][k].T
        wf[C] = inp['osc_freq_b'][k]
        c[f'wfrq{k}'] = wf
        wa = np.zeros((C + 1, N_OSC), np.float32)
        wa[:C] = inp['osc_amp_w'][k].T
        wa[C] = inp['osc_amp_b'][k]
        c[f'wamp{k}'] = wa
        c[f'M{k}'] = _tridiag_M(u)

    c['pi0'] = _frame_phase_pi(16)
    c['pia0'] = _frame_amp_pi(16)
    c['pi1'] = _frame_phase_pi(32)
    c['pia1'] = _frame_amp_pi(32)
    for k in (2, 3, 4, 5):
        bs = BAND_SIZES[k]
        for j, Pi in enumerate(_sub_phase_pis(bs)):
            c[f'pip{k}_{j}'] = Pi
        for j, Pi in enumerate(_sub_amp_pis(bs, ADJUST[bs])):
            c[f'piam{k}_{j}'] = Pi
    c['bas0'] = _frame_phase_basis(16)
    c['bas1'] = _frame_phase_basis(32)
    c['basS'] = _sub_phase_basis()
    c['basA0'] = _frame_amp_basis(16, ADJUST[512])
    c['basA1'] = _frame_amp_basis(32, ADJUST[1024])
    c['basAS'] = _sub_amp_basis()

    for k, bs in enumerate(BAND_SIZES):
        spf = SPF[k]
        nc_ = spf // 2 + 1
        wcf = np.zeros((C + 1, spf), np.float32)
        wc = np.zeros((C + 1, nc_), np.float32)
        wc[:C] = inp[f'noise_coeff_w_{k}'].T
        wc[C] = inp[f'noise_coeff_b_{k}']
        if k == 0:
            wc[:, 1:] = 0.0
        wcf[:, 0:nc_] = wc
        wcf[:, nc_:spf] = wc[:, 1:nc_ - 1]
        FT, IR = _band_fir(bs)
        if k < 5:
            c[f'wc{k}'] = wcf
            c[f'ft{k}'] = FT
            c[f'ir{k}'] = IR
        else:
            c['wc5a'] = wcf[:, 0:128]
            c['wc5b'] = wcf[:, 128:256]
            c['ft5_0'] = FT[0:128]
            c['ft5_1'] = FT[128:256]
            c['ir5_0'] = IR[0:128]
            c['ir5_1'] = IR[128:256]

    mega = np.zeros((128, MEGA_COLS), np.float32)
    for name, (r, o, cd) in MEGA_OFF.items():
        mega[0:r, o:o + cd] = c[name]
    return {'mega': mega}


# ---------------------------------------------------------------- bass build
def _build_nc():
    nc = bacc.Bacc('TRN2', num_devices=NCORE)
    AF = mybir.ActivationFunctionType

    d = {}
    d['xT'] = nc.dram_tensor("xT", [C + 1, BL], F32, kind="ExternalInput")
    d['mega'] = nc.dram_tensor("mega", [128, MEGA_COLS], F32, kind="ExternalInput")
    for k in range(6):
        d[f'noiseT{k}'] = nc.dram_tensor(f"noiseT{k}", [SPF[k], FR], F32,
                                         kind="ExternalInput")
    harm_d = nc.dram_tensor("harm", [4 * 63, 512], F32, kind="ExternalOutput")
    nz_d = nc.dram_tensor("nz", [FR, int(NOFF[6])], F32, kind="ExternalOutput")


    with TileContext(nc) as tc:
        with tc.tile_pool(name="const", bufs=1) as cp, \
             tc.tile_pool(name="work", bufs=2) as wp, \
             tc.tile_pool(name="hot", bufs=3) as hot, \
             tc.tile_pool(name="phF", bufs=3, space="PSUM") as phF, \
             tc.tile_pool(name="phA", bufs=3, space="PSUM") as phA, \
             tc.tile_pool(name="phH", bufs=2, space="PSUM") as phH:

            def psF(p0, f0):
                t = phF.tile([128, 512], F32, tag="phF", name="psF")
                return t[0:p0, 0:f0]

            def psA(p0, f0):
                t = phA.tile([128, 512], F32, tag="phA", name="psA")
                return t[0:p0, 0:f0]

            def psH(p0, f0):
                t = phH.tile([128, 512], F32, tag="phH", name="psH")
                return t[0:p0, 0:f0]

            mega = cp.tile([128, MEGA_COLS], F32, tag="mega")
            _nsplit = 6
            _cut = [MEGA_COLS * i // _nsplit for i in range(_nsplit + 1)]
            for _i in range(_nsplit):
                nc.gpsimd.dma_start(out=mega[:, _cut[_i]:_cut[_i + 1]],
                                    in_=d['mega'][:, _cut[_i]:_cut[_i + 1]])

            ct = {}
            for name, (r, o, cd) in MEGA_OFF.items():
                ct[name] = mega[0:r, o:o + cd]
            for name in ('selstrip', 'negI', 'basA0', 'basA1', 'basAS'):
                r, o, cd = MEGA_OFF[name]
                t = cp.tile([r, cd], BF16, tag=f"bf_{name}")
                nc.gpsimd.dma_start(out=t, in_=d['mega'][0:r, o:o + cd])
                ct[f'{name}_bf'] = t
            for name in ('bas0', 'bas1', 'basS'):
                r, o, cd = MEGA_OFF[name]
                t = cp.tile([r, cd], F32R, tag=f"r_{name}")
                nc.gpsimd.dma_start(out=t, in_=d['mega'][0:r, o:o + cd])
                ct[f'{name}_r'] = t

            def w64s(name):
                i = W64_IDX[name]
                return ct['w64'][:, i * C:(i + 1) * C]

            def b64s(name):
                return ct['bias64'][:, B64_IDX[name]:B64_IDX[name] + 1]

            ident64 = w64s('ident')

            xT = cp.tile([C + 1, BL], F32, tag="xT")
            nc.sync.dma_start(out=xT, in_=d['xT'][:, :])

            # ---------------- frontend (as baseline)
            h = wp.tile([C, 16], F32, tag="h0")
            for t in range(4):
                pt = psF(C, BL)
                nc.tensor.matmul(out=pt, lhsT=ct['wlin'][:, t * C:(t + 1) * C],
                                 rhs=xT, start=True, stop=True)
                nc.vector.tensor_copy(out=h.rearrange("c (b t) -> c b t", t=4)[:, :, t],
                                      in_=pt)
            for i, n in enumerate((4, 8, 16)):
                pt1 = psF(BL * n, C)
                nc.tensor.transpose(out=pt1, in_=h, identity=ident64)
                t1 = wp.tile([BL * n, C], F32, tag=f"fe_t1_{i}")
                nc.vector.tensor_copy(out=t1, in_=pt1)
                pt2 = psF(BL * 2 * n, C)
                nc.tensor.matmul(out=pt2, lhsT=ct[f'ubd{n}'], rhs=t1, start=True,
                                 stop=True)
                t2 = wp.tile([BL * 2 * n, C], F32, tag=f"fe_t2_{i}")
                nc.vector.tensor_copy(out=t2, in_=pt2)
                pt3 = psF(C, BL * 2 * n)
                nc.tensor.transpose(out=pt3, in_=t2,
                                    identity=ct['ident128'][0:BL * 2 * n, 0:BL * 2 * n])
                hu = wp.tile([C, BL * 2 * n], F32, tag=f"fe_hu_{i}")
                nc.vector.tensor_copy(out=hu, in_=pt3)
                m = 2 * n
                hu3 = hu.rearrange("c (b t) -> c b t", b=BL)
                pc = psF(C, BL * m).rearrange("c (b t) -> c b t", b=BL)
                nc.tensor.matmul(out=pc[:, :, :], lhsT=w64s(f'up{i}d1'), rhs=hu3[:, :, :],
                                 start=True, stop=False)
                nc.tensor.matmul(out=pc[:, :, 1:m], lhsT=w64s(f'up{i}d0'),
                                 rhs=hu3[:, :, 0:m - 1], start=False, stop=False)
                nc.tensor.matmul(out=pc[:, :, 0:m - 1], lhsT=w64s(f'up{i}d2'),
                                 rhs=hu3[:, :, 1:m], start=False, stop=True)
                h = wp.tile([C, BL * m], F32, tag=f"fe_h_{i}")
                nc.scalar.activation(out=h.rearrange("c (b t) -> c b t", b=BL), in_=pc,
                                     func=AF.Prelu, bias=b64s(f'up{i}'), scale=1.0,
                                     alpha=0.2)
            h3 = h.rearrange("c (b t) -> c b t", b=BL)
            pf = psF(C, BL * 32).rearrange("c (b t) -> c b t", b=BL)
            nc.tensor.matmul(out=pf[:, :, :], lhsT=w64s('find1'), rhs=h3[:, :, :],
                             start=True, stop=False)
            nc.tensor.matmul(out=pf[:, :, 1:32], lhsT=w64s('find0'), rhs=h3[:, :, 0:31],
                             start=False, stop=False)
            nc.tensor.matmul(out=pf[:, :, 0:31], lhsT=w64s('find2'), rhs=h3[:, :, 1:32],
                             start=False, stop=True)
            hfin = cp.tile([C, 128], F32, tag="hfin")
            nc.scalar.activation(out=hfin.rearrange("c (b t) -> c b t", b=BL), in_=pf,
                                 func=AF.Identity, bias=b64s('fin'), scale=1.0)

            # ---------------- per-band setup
            nTs_all = {}
            for k in range(6):
                if k < 5:
                    nT = wp.tile([SPF[k], FR], F32, tag=f"nT{k}", name="nT", bufs=1)
                    nc.sync.dma_start(out=nT, in_=d[f'noiseT{k}'][:, :])
                    nTs_all[k] = (nT,)
                else:
                    nT0 = wp.tile([128, FR], F32, tag="nT50", bufs=1)
                    nT1 = wp.tile([128, FR], F32, tag="nT51", bufs=1)
                    nc.sync.dma_start(out=nT0, in_=d['noiseT5'][0:128, :])
                    nc.sync.dma_start(out=nT1, in_=d['noiseT5'][128:256, :])
                    nTs_all[5] = (nT0, nT1)
            def noise_branch(k):
                bs = BAND_SIZES[k]
                spf = SPF[k]
                    zf3 = zfas[k][0:C, :].rearrange("c (b t) -> c b t", b=BL)
                    zrep = zf3.unsqueeze(-1).broadcast_to([C, BL, 32, 2])
                    pn = psA(C, FR)
                    nc.tensor.matmul(out=pn, lhsT=w64s(f'nup{k}'), rhs=zrep,
                                         start=True, stop=True)
                    naug = wp.tile([C + 1, FR], F32, tag="naug")
                    nc.scalar.activation(out=naug[0:C, :], in_=pn, func=AF.Prelu,
                                             bias=b64s(f'nup{k}'), scale=1.0, alpha=0.2)
                    nc.vector.memset(naug[C:C + 1, :], 1.0)

                    if k < 5:
                        nT = nTs_all[k][0]
                        pcA = psH(spf, FR)
                        nc.tensor.matmul(out=pcA, lhsT=ct[f'wc{k}'], rhs=naug,
                                             start=True, stop=True)
                        chat = wp.tile([spf, FR], F32, tag="chat")
                        nc.scalar.copy(out=chat, in_=pcA)
                        psp = psH(spf, FR)
                        nc.tensor.matmul(out=psp, lhsT=ct[f'ft{k}'], rhs=nT,
                                             start=True, stop=True)
                        sA = wp.tile([spf, FR], F32, tag="sA")
                        nc.vector.tensor_tensor(out=sA, in0=chat, in1=psp, op=ALU.mult)
                        sAs = [sA]
                    else:
                        nT0, nT1 = nTs_all[5]
                        sAs = []
                        for half, wch in ((0, 'wc5a'), (1, 'wc5b')):
                            pcA = psH(128, FR)
                            nc.tensor.matmul(out=pcA, lhsT=ct[wch], rhs=naug,
                                                 start=True, stop=True)
                            chat = wp.tile([128, FR], F32, tag=f"chat5{half}")
                            nc.scalar.copy(out=chat, in_=pcA)
                            psp = psH(128, FR)
                            nc.tensor.matmul(out=psp,
                                                 lhsT=ct['ft5_0'][:, 128 * half:128 * (half + 1)],
                                                 rhs=nT0, start=True, stop=False)
                            nc.tensor.matmul(out=psp,
                                                 lhsT=ct['ft5_1'][:, 128 * half:128 * (half + 1)],
                                                 rhs=nT1, start=False, stop=True)
                            sA = wp.tile([128, FR], F32, tag=f"sA5{half}")
                            nc.vector.tensor_tensor(out=sA, in0=chat, in1=psp, op=ALU.mult)
                            sAs.append(sA)

                    for fg in range(2):
                        pnz = psA(128, spf)
                        if k < 5:
                            nc.tensor.matmul(out=pnz,
                                                 lhsT=sAs[0][:, 128 * fg:128 * (fg + 1)],
                                                 rhs=ct[f'ir{k}'], start=True, stop=True)
                        else:
                            nc.tensor.matmul(out=pnz,
                                                 lhsT=sAs[0][:, 128 * fg:128 * (fg + 1)],
                                                 rhs=ct['ir5_0'], start=True, stop=False)
                            nc.tensor.matmul(out=pnz,
                                                 lhsT=sAs[1][:, 128 * fg:128 * (fg + 1)],
                                                 rhs=ct['ir5_1'], start=False, stop=True)
                        nzs = wp.tile([128, spf], F32, tag="nzs")
                        nc.scalar.copy(out=nzs, in_=pnz)
                        nc.sync.dma_start(
                            out=nz_d[128 * fg:128 * (fg + 1),
                                         int(NOFF[k]):int(NOFF[k]) + spf],
                            in_=nzs)


            stacks = {}
            astacks = {}
            zfas = {}
            srcs = {}
            ampgs = {}
            LFS = [0.05 if bs == 512 else 0.01 for bs in BAND_SIZES]

            # pass 1: residual stacks, layer-interleaved across bands
            zs = {k: hfin for k in range(6)}
            for j in range(4):
                for k in range(6):
                    pz = psF(C, 128)
                    nc.tensor.matmul(out=pz, lhsT=w64s(f't{k}{j}'), rhs=zs[k],
                                     start=True, stop=True)
                    z = wp.tile([C, 128], F32, tag=f"z{k}_{j % 2}", name="z", bufs=1)
                    nc.scalar.activation(out=z, in_=pz, func=AF.Prelu,
                                         bias=b64s(f't{k}{j}'), scale=1.0, alpha=0.2)
                    zs[k] = z
            for k in range(6):
                pz = psF(C, 128)
                nc.tensor.matmul(out=pz, lhsT=w64s(f'bf{k}'), rhs=zs[k],
                                 start=True, stop=True)
                zfa = wp.tile([C + 1, 128], F32, tag=f"zfa{k}", name="zfa", bufs=1)
                nc.vector.tensor_scalar(out=zfa[0:C, :], in0=pz,
                                        scalar1=b64s(f'bf{k}'), scalar2=None,
                                        op0=ALU.add)
                nc.vector.memset(zfa[C:C + 1, :], 1.0)
                zfas[k] = zfa

            # pass 2: freq/amp grids
            for k in range(6):
                zfa = zfas[k]
                pgF = psH(N_OSC, 128)
                for b in range(BL):
                    nc.tensor.matmul(out=pgF[:, 32 * b:32 * (b + 1)],
                                     lhsT=zfa[:, 32 * b:32 * (b + 1)],
                                     rhs=ct[f'wfrq{k}'], start=True, stop=True)
                sig = wp.tile([N_OSC, 128], F32, tag=f"sig{k}", name="sig", bufs=1)
                nc.scalar.activation(out=sig, in_=pgF, func=AF.Sigmoid, scale=1.0)
                src = wp.tile([64, 128], F32, tag=f"src{k}", name="src", bufs=1)
                nc.vector.tensor_scalar(out=src[0:32, :], in0=sig,
                                        scalar1=float((1.0 - LFS[k]) / 2.0),
                                        scalar2=float(LFS[k] / 2.0),
                                        op0=ALU.mult, op1=ALU.add)
                srcs[k] = src
                pgA = psH(N_OSC, 128)
                for b in range(BL):
                    nc.tensor.matmul(out=pgA[:, 32 * b:32 * (b + 1)],
                                     lhsT=zfa[:, 32 * b:32 * (b + 1)],
                                     rhs=ct[f'wamp{k}'], start=True, stop=True)
                ampg = wp.tile([N_OSC, 128], F32, tag=f"ampg{k}", name="ampg", bufs=1)
                nc.scalar.activation(out=ampg, in_=pgA, func=AF.Abs, scale=1.0)
                ampgs[k] = ampg

            # pass 3: frame carries
            Srs = {}
            for k in range(6):
                pS = psH(N_OSC, 128)
                nc.tensor.matmul(out=pS, lhsT=ct[f'M{k}'], rhs=srcs[k][0:32, :],
                                 start=True, stop=True)
                rndS = wp.tile([N_OSC, 128], F32, tag=f"rndS{k}", name="rndS", bufs=1)
                nc.vector.tensor_scalar(out=rndS, in0=pS, scalar1=MAGIC,
                                        scalar2=MAGIC, op0=ALU.add,
                                        op1=ALU.subtract)
                Sr = wp.tile([N_OSC, 128], F32, tag=f"Sr{k}", name="Sr", bufs=1)
                nc.vector.tensor_tensor(out=Sr, in0=pS, in1=rndS, op=ALU.subtract)
                Srs[k] = Sr
            for k in range(6):
                pP = psH(N_OSC, 128)
                nc.tensor.matmul(out=pP, lhsT=ct['ut'], rhs=Srs[k], start=True,
                                 stop=True)
                rndP = wp.tile([N_OSC, 128], F32, tag=f"rndP{k}", name="rndP", bufs=1)
                nc.vector.tensor_scalar(out=rndP, in0=pP, scalar1=MAGIC,
                                        scalar2=MAGIC, op0=ALU.add,
                                        op1=ALU.subtract)
                nc.vector.tensor_tensor(out=srcs[k][32:64, :], in0=pP, in1=rndP,
                                        op=ALU.subtract)

            # pass 4: per-band stacks + noise
            for k, bs in enumerate(BAND_SIZES):
                nch = NCH[k]
                src = srcs[k]
                ampg = ampgs[k]
                if k == 0:
                    stk = cp.tile([128, 128], F32, tag="stk0")
                    pb = psF(128, 128)
                    nc.tensor.matmul(out=pb, lhsT=ct['pi0'], rhs=src, start=True,
                                     stop=True)
                    nc.scalar.copy(out=stk, in_=pb)
                elif k == 1:
                    stk = cp.tile([128, 128], F32, tag="stk1")
                    pb = psF(128, 128)
                    nc.tensor.matmul(out=pb, lhsT=ct['pi1'], rhs=src, start=True,
                                     stop=True)
                    nc.scalar.copy(out=stk, in_=pb)
                else:
                    stk = cp.tile([128, (nch // 2) * 128], F32, tag=f"stk{k}")
                    nblkp = nch // 2
                    for j0 in range(0, nblkp, 4):
                        jn = min(4, nblkp - j0)
                        pb = psF(128, 128 * jn)
                        for jj in range(jn):
                            nc.tensor.matmul(out=pb[:, 128 * jj:128 * (jj + 1)],
                                             lhsT=ct[f'pip{k}_{j0 + jj}'], rhs=src,
                                             start=True, stop=True)
                        rnd = wp.tile([128, 512], F32, tag="rndB")
                        nc.vector.tensor_scalar(out=rnd[:, 0:128 * jn], in0=pb,
                                                scalar1=MAGIC, scalar2=MAGIC,
                                                op0=ALU.add, op1=ALU.subtract)
                        nc.vector.tensor_tensor(
                            out=stk[:, 128 * j0:128 * (j0 + jn)],
                            in0=pb, in1=rnd[:, 0:128 * jn], op=ALU.subtract)
                if k == 0:
                    stkr = cp.tile([128, 128], F32R, tag="stkr0", name="stkr")
                elif k == 1:
                    stkr = cp.tile([128, 128], F32R, tag="stkr1", name="stkr")
                else:
                    stkr = cp.tile([128, (nch // 2) * 128], F32R, tag=f"stkr{k}",
                                   name="stkr")
                nc.gpsimd.dma_start(out=stkr, in_=stk)
                stacks[k] = stkr

                if k == 0:
                    ast = cp.tile([96, 128], BF16, tag="ast0")
                    pb = psF(96, 128)
                    nc.tensor.matmul(out=pb, lhsT=ct['pia0'], rhs=ampg, start=True,
                                     stop=True)
                    nc.scalar.copy(out=ast, in_=pb)
                elif k == 1:
                    ast = cp.tile([128, 128], BF16, tag="ast1")
                    pb = psF(128, 128)
                    nc.tensor.matmul(out=pb, lhsT=ct['pia1'], rhs=ampg, start=True,
                                     stop=True)
                    nc.scalar.copy(out=ast, in_=pb)
                else:
                    nblk = (nch + 2) // 3
                    ast = cp.tile([96, nblk * 128], BF16, tag=f"ast{k}")
                    for j0 in range(0, nblk, 4):
                        jn = min(4, nblk - j0)
                        pb = psF(96, 128 * jn)
                        for jj in range(jn):
                            nc.tensor.matmul(out=pb[:, 128 * jj:128 * (jj + 1)],
                                             lhsT=ct[f'piam{k}_{j0 + jj}'], rhs=ampg,
                                             start=True, stop=True)
                        nc.scalar.copy(out=ast[:, 128 * j0:128 * (j0 + jn)], in_=pb)
                astacks[k] = ast
                noise_branch(k)

            # ---------------- chunk loop (per-chunk, deep software pipeline)
            chunks = []
            for k in (1, 2, 3, 4, 5, 0):
                nch = NCH[k]
                for gstart in range(0, nch, 8):
                    gs = min(8, nch - gstart)
                    for cc in range(gs):
                        chunks.append((k, gstart + cc, cc, gs))
            Nc = len(chunks)
            state = {}

            def stage_a(i):
                k, c, cc, gs = chunks[i]
                ppt = phF.tile([128, 512], F32, tag="phF", name="ppt")
                if k == 0:
                    nc.tensor.matmul(out=ppt, lhsT=stacks[0], rhs=ct['bas0_r'],
                                     start=True, stop=False)
                elif k == 1:
                    nc.tensor.matmul(out=ppt, lhsT=stacks[1][64 * c:64 * c + 64],
                                     rhs=ct['bas1_r'][64 * c:64 * c + 64],
                                     start=True, stop=False)
                else:
                    j, par = c // 2, c % 2
                    nc.tensor.matmul(
                        out=ppt,
                        lhsT=stacks[k][64 * par:64 * par + 48,
                                       128 * j:128 * (j + 1)],
                        rhs=ct['basS_r'][64 * par:64 * par + 48],
                        start=True, stop=False)
                pat = phA.tile([128, 512], F32, tag="phA", name="pat")
                if k == 0:
                    nc.tensor.matmul(out=pat, lhsT=astacks[0],
                                     rhs=ct['basA0_bf'], start=True, stop=True)
                elif k == 1:
                    nc.tensor.matmul(out=pat, lhsT=astacks[1][64 * c:64 * c + 48],
                                     rhs=ct['basA1_bf'][64 * c:64 * c + 48],
                                     start=True, stop=True)
                else:
                    ja, ma = c // 3, c % 3
                    nc.tensor.matmul(
                        out=pat,
                        lhsT=astacks[k][32 * ma:32 * ma + 32,
                                        128 * ja:128 * (ja + 1)],
                        rhs=ct['basAS_bf'][32 * ma:32 * ma + 32],
                        start=True, stop=True)
                ntile = hot.tile([128, 512], BF16, tag="ntile")
                nc.vector.tensor_scalar(out=ntile, in0=ppt, scalar1=MAGIC,
                                        scalar2=MAGIC, op0=ALU.add,
                                        op1=ALU.subtract)
                if i % 2 == 0:
                    pat_sb = hot.tile([128, 512], BF16, tag="pat_sb")
                    nc.scalar.copy(out=pat_sb, in_=pat)
                else:
                    pat_sb = None
                state[i] = (ppt, pat, ntile, pat_sb)

            def stage_b(i):
                ppt, pat, ntile, pat_sb = state[i]
                nc.tensor.matmul(out=ppt, lhsT=ct['negI_bf'], rhs=ntile,
                                 start=False, stop=True)
                s = hot.tile([128, 512], BF16, tag="sin_t")
                nc.scalar.activation(out=s, in_=ppt, func=AF.Sin, scale=TWO_PI)
                prod = hot.tile([128, 512], BF16, tag="prod_t")
                nc.vector.tensor_tensor(out=prod, in0=s,
                                        in1=pat_sb if pat_sb is not None else pat,
                                        op=ALU.mult)
                state[i] = prod

            hpt_cur = [None]

            def stage_c(i):
                k, c, cc, gs = chunks[i]
                prod = state.pop(i)
                if cc == 0:
                    hpt_cur[0] = phH.tile([128, 512], F32, tag="phH", name="hpt")
                hpt = hpt_cur[0]
                nc.tensor.matmul(out=hpt,
                                 lhsT=ct['selstrip_bf'][:, 128 - 4 * cc:256 - 4 * cc],
                                 rhs=prod, start=(cc == 0), stop=(cc == gs - 1))
                if cc == gs - 1:
                    hsb = wp.tile([32, 512], F32, tag="hsb")
                    nc.scalar.copy(out=hsb[0:4 * gs], in_=hpt[0:4 * gs])
                    g0 = CHUNK_BASE[k] + c - (gs - 1)
                    nc.sync.dma_start(
                        out=harm_d[4 * int(g0):4 * int(g0) + 4 * gs, :],
                        in_=hsb[0:4 * gs])

            for i in range(Nc + 4):
                if i < Nc:
                    stage_a(i)
                if 2 <= i < Nc + 2:
                    stage_b(i - 2)
                if 4 <= i < Nc + 4:
                    stage_c(i - 4)

    nc.finalize()
    return nc


# ---------------------------------------------------------------- host glue
def _prep_inputs(inputs):
    inp = {k: np.asarray(v, np.float32) for k, v in inputs.items()}
    shared = _build_shared(inp)
    in_maps = []
    for core in range(NCORE):
        m = dict(shared)
        sl = slice(core * BL, (core + 1) * BL)
        m['xT'] = np.concatenate([inp['x'][sl].T, np.ones((1, BL), np.float32)],
                                 axis=0)
        for k in range(6):
            nT = inp[f'noise_{k}'][sl].reshape(FR, SPF[k]).T
            m[f'noiseT{k}'] = np.ascontiguousarray(nT, dtype=np.float32)
        in_maps.append(m)
    return in_maps


def kernel(**inputs):
    if 'nc' not in _nc_cache:
        _nc_cache['nc'] = _build_nc()
    nc = _nc_cache['nc']
    in_maps = _prep_inputs(inputs)
    res = run_bass_kernel_spmd(nc, in_maps, list(range(NCORE)))
    out = np.zeros((B, TOTAL), np.float32)
    for core in range(NCORE):
        r = res.results[core]
        harm = np.asarray(r['harm'], np.float32).reshape(63, BL, 512)
        nz = np.asarray(r['nz'], np.float32).reshape(BL, NNF, int(NOFF[6]))
        for k, bs in enumerate(BAND_SIZES):
            nch = NCH[k]
            hb = harm[CHUNK_BASE[k]:CHUNK_BASE[k] + nch]  # [nch, BL, 512]
            hb = hb.transpose(1, 0, 2).reshape(BL, bs)
            out[core * BL:(core + 1) * BL,
                HARM_OFF[k]:HARM_OFF[k] + bs] = hb
            nzb = nz[:, :, int(NOFF[k]):int(NOFF[k]) + SPF[k]].reshape(BL, bs)
            out[core * BL:(core + 1) * BL, NZ_OFF[k]:NZ_OFF[k] + bs] = nzb
    return out.astype(np.float32)


if __name__ == "__main__":
    import reference
    inp = reference.setup_inputs()
    out = kernel(**{k: np.asarray(v) for k, v in inp.items()})
    print("out", out.shape, out.dtype)


# revision 39
# speedup vs baseline: 1.2671x; 1.0007x over previous
"""Trainium2 Bass kernel for nn_Decoder (DDSP-style decoder) — redesigned.

Data-parallel over batch (32 -> 4 per core x 8 cores). Harmonic synthesis
uses per-frame (bands 0-1) / per-32-sample-subframe quadratic (bands 2-5)
phase bases so every chunk matmul's magnitudes stay small enough for the
fast float32r PE path. Grid coefficients are produced directly in
[frame, (batch,osc)] layout by operand-swapped matmuls and reorganized into
per-chunk lhsT stacks by constant permutation matmuls (no DRAM scratch, no
gather DMAs). The noise branch is real-DFT basis matmuls in float32r on
host-transposed noise.
"""
import numpy as np
import sys

sys.path.insert(0, "/opt/trn_rl_repo")

from concourse import bacc, mybir  # noqa: E402
from concourse.tile import TileContext  # noqa: E402
from concourse.bass_utils import run_bass_kernel_spmd  # noqa: E402

F32 = mybir.dt.float32
F32R = mybir.dt.float32r
BF16 = mybir.dt.bfloat16
ALU = mybir.AluOpType
BAND_SIZES = [512, 1024, 2048, 4096, 8192, 16384]
ADJUST = {512: 0.05, 1024: 0.03, 2048: 0.05, 4096: 0.25, 8192: 1.0, 16384: 20.0}
B, C, N_OSC, NNF = 32, 64, 32, 64
NCORE = 8
BL = B // NCORE
FR = BL * NNF
MAGIC = float(1.5 * 2 ** 23)
TWO_PI = float(2 * np.pi)
TOTAL = 2 * sum(BAND_SIZES)

SPF = [bs // NNF for bs in BAND_SIZES]            # 8..256
NOFF = np.concatenate([[0], np.cumsum(SPF)]).astype(int)   # noise col offsets
NCH = [bs // 512 for bs in BAND_SIZES]            # 1,2,4,8,16,32
CHUNK_BASE = np.concatenate([[0], np.cumsum(NCH)]).astype(int)
HARM_OFF = {}
NZ_OFF = {}
_off = 0
for _k, _bs in enumerate(BAND_SIZES):
    HARM_OFF[_k] = _off
    NZ_OFF[_k] = _off + _bs
    _off += 2 * _bs

_nc_cache = {}

W64_ORDER = ([f'up{i}d{dd}' for i in range(3) for dd in range(3)]
             + [f'find{dd}' for dd in range(3)]
             + [w for k in range(6) for w in
                [f't{k}0', f't{k}1', f't{k}2', f't{k}3', f'bf{k}', f'nup{k}']]
             + ['ident'])
W64_IDX = {n: i for i, n in enumerate(W64_ORDER)}
B64_ORDER = ([f'up{i}' for i in range(3)] + ['fin']
             + [b for k in range(6) for b in
                [f't{k}0', f't{k}1', f't{k}2', f't{k}3', f'bf{k}', f'nup{k}']])
B64_IDX = {n: i for i, n in enumerate(B64_ORDER)}


# ---------------------------------------------------------------- host math
def _band_L2(bs):
    t = np.arange(bs)
    pos = (t + 0.5) * (32.0 / bs) - 0.5
    lo = np.clip(np.floor(pos).astype(int), 0, 31)
    hi = np.clip(lo + 1, 0, 31)
    w = np.clip(pos - lo, 0.0, 1.0)
    L2 = np.zeros((32, bs))
    np.add.at(L2, (lo, t), 1.0 - w)
    np.add.at(L2, (hi, t), w)
    return L2


def _interp_vecs(u):
    r = np.arange(u)
    f = (r + 0.5) / u - 0.5
    gm = np.where(r < u // 2, -f, 0.0)
    g0 = np.where(r < u // 2, 1 + f, 1 - f)
    gp = np.where(r >= u // 2, f, 0.0)
    return gm, g0, gp


def _frame_phase_pi(u):
    Pi = np.zeros((64, 128))
    nq = 512 // u
    nslot = 1 if u == 16 else 2
    for cs in range(nslot):
        for g in range(4):
            for q in range(nq):
                row = cs * 64 + g * nq + q
                f = cs * nq + q
                if g == 0:
                    Pi[max(f - 1, 0), row] = 1.0
                elif g == 1:
                    Pi[f, row] = 1.0
                elif g == 2:
                    Pi[min(f + 1, 31), row] = 1.0
                elif f > 0:
                    Pi[32 + f - 1, row] = 1.0
    return Pi.astype(np.float32)


def _frame_phase_basis(u):
    gm, g0, gp = _interp_vecs(u)
    Gm, G0, Gp = np.cumsum(gm), np.cumsum(g0), np.cumsum(gp)
    nq = 512 // u
    bas = np.zeros((128, 512))
    nrep = 1 if u == 16 else 2
    for rep in range(nrep):
        for q in range(nq):
            cols = slice(q * u, (q + 1) * u)
            bas[rep * 64 + 0 * nq + q, cols] = Gm
            bas[rep * 64 + 1 * nq + q, cols] = G0
            bas[rep * 64 + 2 * nq + q, cols] = Gp
            bas[rep * 64 + 3 * nq + q, cols] = 1.0
    return bas.astype(np.float32)


def _frame_amp_pi(u):
    nq = 512 // u
    if u == 16:
        Pi = np.zeros((32, 96))
        for g in range(3):
            for q in range(nq):
                src = max(q - 1, 0) if g == 0 else (q if g == 1 else min(q + 1, 31))
                Pi[src, g * nq + q] = 1.0
    else:
        Pi = np.zeros((32, 128))
        for cs in range(2):
            for g in range(3):
                for q in range(nq):
                    f = cs * nq + q
                    src = max(f - 1, 0) if g == 0 else (f if g == 1 else min(f + 1, 31))
                    Pi[src, cs * 64 + g * nq + q] = 1.0
    return Pi.astype(np.float32)


def _frame_amp_basis(u, adj):
    gm, g0, gp = _interp_vecs(u)
    inv = 1.0 / adj
    nq = 512 // u
    rows = 96 if u == 16 else 112
    bas = np.zeros((rows, 512))
    nrep = 1 if u == 16 else 2
    for rep in range(nrep):
        for q in range(nq):
            cols = slice(q * u, (q + 1) * u)
            bas[rep * 64 + 0 * nq + q, cols] = gm * inv
            bas[rep * 64 + 1 * nq + q, cols] = g0 * inv
            bas[rep * 64 + 2 * nq + q, cols] = gp * inv
    return bas.astype(np.float32)


def _sub_phase_pis(bs):
    u = bs // 32
    L2 = _band_L2(bs)
    CW = np.zeros_like(L2)
    for F in range(32):
        cols = slice(F * u, (F + 1) * u)
        CW[:, cols] = np.cumsum(L2[:, cols], axis=1)
    pis = []
    for j in range(bs // 1024):
        Pi = np.zeros((64, 128))
        for par in range(2):
            c = 2 * j + par
            for s in range(16):
                t0 = 512 * c + 32 * s
                F = t0 // u
                if t0 % u != 0:
                    Pi[0:32, par * 64 + s] = CW[:, t0 - 1]
                if F >= 1:
                    Pi[32 + F - 1, par * 64 + s] = 1.0
                Pi[0:32, par * 64 + 16 + s] = L2[:, t0]
                Pi[0:32, par * 64 + 32 + s] = L2[:, t0 + 1] - L2[:, t0]
        pis.append(Pi.astype(np.float32))
    return pis


def _sub_phase_basis():
    bas = np.zeros((112, 512))
    i = np.arange(32)
    for rep in range(2):
        for s in range(16):
            cols = slice(32 * s, 32 * (s + 1))
            bas[rep * 64 + s, cols] = 1.0
            bas[rep * 64 + 16 + s, cols] = i + 1
            bas[rep * 64 + 32 + s, cols] = i * (i + 1) / 2.0
    return bas.astype(np.float32)


def _sub_amp_pis(bs, adj):
    L2 = _band_L2(bs)
    inv = 1.0 / adj
    nch = bs // 512
    pis = []
    for j in range((nch + 2) // 3):
        Pi = np.zeros((32, 96))
        for m in range(3):
            c = 3 * j + m
            if c >= nch:
                break
            for s in range(16):
                t0 = 512 * c + 32 * s
                Pi[:, m * 32 + s] = L2[:, t0] * inv
                Pi[:, m * 32 + 16 + s] = (L2[:, t0 + 1] - L2[:, t0]) * inv
        pis.append(Pi.astype(np.float32))
    return pis


def _sub_amp_basis():
    bas = np.zeros((96, 512))
    i = np.arange(32)
    for rep in range(3):
        for s in range(16):
            cols = slice(32 * s, 32 * (s + 1))
            bas[rep * 32 + s, cols] = 1.0
            bas[rep * 32 + 16 + s, cols] = i
    return bas.astype(np.float32)


def _tridiag_M(u):
    M = np.zeros((32, 32))
    for f in range(32):
        M[max(f - 1, 0), f] += u / 8.0
        M[min(f + 1, 31), f] += u / 8.0
        M[f, f] += 3.0 * u / 4.0
    return M.astype(np.float32)


def _band_fir(bs):
    spf = bs // NNF
    nc_ = spf // 2 + 1
    t = np.arange(spf)
    j_re = np.arange(nc_)
    j_im = np.arange(1, nc_ - 1)
    FT = np.concatenate([np.cos(2 * np.pi * np.outer(t, j_re) / spf),
                         -np.sin(2 * np.pi * np.outer(t, j_im) / spf)], axis=1)
    w = np.full(nc_, 2.0)
    w[0] = 1.0
    w[-1] = 1.0
    IR = np.concatenate([
        (w[:, None] * np.cos(2 * np.pi * np.outer(j_re, t) / spf)) / spf,
        (-2.0 * np.sin(2 * np.pi * np.outer(j_im, t) / spf)) / spf,
    ], axis=0) / ADJUST[bs]
    return FT.astype(np.float32), IR.astype(np.float32)


def _build_U(n):
    eye = np.eye(n)
    spec = np.fft.rfft(eye, axis=-1)
    spec = np.pad(spec, ((0, 0), (0, n + 1 - spec.shape[-1])))
    return np.fft.irfft(spec, n=2 * n, axis=-1) * 2


def _mega_entries():
    ents = [('wlin', C + 1, 4 * C), ('ubd4', BL * 4, BL * 8), ('ubd8', BL * 8, BL * 16),
            ('ubd16', BL * 16, BL * 32), ('w64', C, len(W64_ORDER) * C),
            ('bias64', C, len(B64_ORDER)), ('ident128', 128, 128), ('ut', 32, 32)]
    for k in range(6):
        ents.append((f'wfrq{k}', C + 1, N_OSC))
        ents.append((f'wamp{k}', C + 1, N_OSC))
        ents.append((f'M{k}', 32, 32))
    ents += [('pi0', 64, 128), ('pia0', 32, 96), ('pi1', 64, 128), ('pia1', 32, 128)]
    for k in (2, 3, 4, 5):
        nch = NCH[k]
        for j in range(nch // 2):
            ents.append((f'pip{k}_{j}', 64, 128))
        for j in range((nch + 2) // 3):
            ents.append((f'piam{k}_{j}', 32, 96))
    ents += [('bas0', 128, 512), ('bas1', 128, 512), ('basS', 112, 512),
             ('basA0', 96, 512), ('basA1', 112, 512), ('basAS', 96, 512),
             ('selstrip', 128, 256), ('negI', 128, 128)]
    for k in range(6):
        spf = SPF[k]
        nc_ = spf // 2 + 1
        if k < 5:
            ents.append((f'wc{k}', C + 1, spf))
            ents.append((f'ft{k}', spf, spf))
            ents.append((f'ir{k}', spf, spf))
        else:
            ents.append(('wc5a', C + 1, 128))
            ents.append(('wc5b', C + 1, 128))
            ents.append(('ft5_0', 128, 256))
            ents.append(('ft5_1', 128, 256))
            ents.append(('ir5_0', 128, 256))
            ents.append(('ir5_1', 128, 256))
    off = {}
    o = 0
    for name, r, cd in ents:
        off[name] = (r, o, cd)
        o += cd
    return off, o


MEGA_OFF, MEGA_COLS = _mega_entries()


def _build_shared(inp):
    c = {}
    wl = np.zeros((4, C + 1, C), np.float32)
    for t in range(4):
        wl[t, :C] = inp['up_lin_w'][:, t::4]
        wl[t, C] = inp['up_lin_b'][t::4]
    c['wlin'] = wl.transpose(1, 0, 2).reshape(C + 1, 4 * C)
    for n in (4, 8, 16):
        U = _build_U(n)
        ub = np.zeros((BL * n, BL * 2 * n), np.float32)
        for b in range(BL):
            ub[b * n:(b + 1) * n, b * 2 * n:(b + 1) * 2 * n] = U
        c[f'ubd{n}'] = ub

    w64 = np.zeros((C, len(W64_ORDER) * C), np.float32)

    def put64(name, m):
        i = W64_IDX[name]
        w64[:, i * C:(i + 1) * C] = m

    for i in range(3):
        for dd in range(3):
            put64(f'up{i}d{dd}', inp['up_conv_w'][i, :, :, dd].T)
    for dd in range(3):
        put64(f'find{dd}', inp['up_final_w'][:, :, dd].T)
    for k in range(6):
        for j in range(4):
            put64(f't{k}{j}', inp['t_w'][k, j].T + np.eye(C, dtype=np.float32))
        put64(f'bf{k}', inp['band_final_w'][k].T)
        put64(f'nup{k}', inp['noise_up_w'][k].T)
    put64('ident', np.eye(C))
    c['w64'] = w64

    b64 = np.zeros((C, len(B64_ORDER)), np.float32)
    for i in range(3):
        b64[:, B64_IDX[f'up{i}']] = inp['up_conv_b'][i]
    b64[:, B64_IDX['fin']] = inp['up_final_b']
    for k in range(6):
        for j in range(4):
            b64[:, B64_IDX[f't{k}{j}']] = inp['t_b'][k, j]
        b64[:, B64_IDX[f'bf{k}']] = inp['band_final_b'][k]
        b64[:, B64_IDX[f'nup{k}']] = inp['noise_up_b'][k]
    c['bias64'] = b64
    c['ident128'] = np.eye(128, dtype=np.float32)

    sel = np.zeros((128, 256), np.float32)
    for b in range(BL):
        sel[b * N_OSC:(b + 1) * N_OSC, 128 + b] = 1.0
    c['selstrip'] = sel
    c['negI'] = (-np.eye(128)).astype(np.float32)
    c['ut'] = np.triu(np.ones((32, 32))).astype(np.float32)

    for k, bs in enumerate(BAND_SIZES):
        u = bs // 32
        wf = np.zeros((C + 1, N_OSC), np.float32)
        wf[:C] = inp['osc_freq_w'][k].T
        wf[C] = inp['osc_freq_b'][k]
        c[f'wfrq{k}'] = wf
        wa = np.zeros((C + 1, N_OSC), np.float32)
        wa[:C] = inp['osc_amp_w'][k].T
        wa[C] = inp['osc_amp_b'][k]
        c[f'wamp{k}'] = wa
        c[f'M{k}'] = _tridiag_M(u)

    c['pi0'] = _frame_phase_pi(16)
    c['pia0'] = _frame_amp_pi(16)
    c['pi1'] = _frame_phase_pi(32)
    c['pia1'] = _frame_amp_pi(32)
    for k in (2, 3, 4, 5):
        bs = BAND_SIZES[k]
        for j, Pi in enumerate(_sub_phase_pis(bs)):
            c[f'pip{k}_{j}'] = Pi
        for j, Pi in enumerate(_sub_amp_pis(bs, ADJUST[bs])):
            c[f'piam{k}_{j}'] = Pi
    c['bas0'] = _frame_phase_basis(16)
    c['bas1'] = _frame_phase_basis(32)
    c['basS'] = _sub_phase_basis()
    c['basA0'] = _frame_amp_basis(16, ADJUST[512])
    c['basA1'] = _frame_amp_basis(32, ADJUST[1024])
    c['basAS'] = _sub_amp_basis()

    for k, bs in enumerate(BAND_SIZES):
        spf = SPF[k]
        nc_ = spf // 2 + 1
        wcf = np.zeros((C + 1, spf), np.float32)
        wc = np.zeros((C + 1, nc_), np.float32)
        wc[:C] = inp[f'noise_coeff_w_{k}'].T
        wc[C] = inp[f'noise_coeff_b_{k}']
        if k == 0:
            wc[:, 1:] = 0.0
        wcf[:, 0:nc_] = wc
        wcf[:, nc_:spf] = wc[:, 1:nc_ - 1]
        FT, IR = _band_fir(bs)
        if k < 5:
            c[f'wc{k}'] = wcf
            c[f'ft{k}'] = FT
            c[f'ir{k}'] = IR
        else:
            c['wc5a'] = wcf[:, 0:128]
            c['wc5b'] = wcf[:, 128:256]
            c['ft5_0'] = FT[0:128]
            c['ft5_1'] = FT[128:256]
            c['ir5_0'] = IR[0:128]
            c['ir5_1'] = IR[128:256]

    mega = np.zeros((128, MEGA_COLS), np.float32)
    for name, (r, o, cd) in MEGA_OFF.items():
        mega[0:r, o:o + cd] = c[name]
    return {'mega': mega}


# ---------------------------------------------------------------- bass build
def _build_nc():
    nc = bacc.Bacc('TRN2', num_devices=NCORE)
    AF = mybir.ActivationFunctionType

    d = {}
    d['xT'] = nc.dram_tensor("xT", [C + 1, BL], F32, kind="ExternalInput")
    d['mega'] = nc.dram_tensor("mega", [128, MEGA_COLS], F32, kind="ExternalInput")
    for k in range(6):
        d[f'noiseT{k}'] = nc.dram_tensor(f"noiseT{k}", [SPF[k], FR], F32,
                                         kind="ExternalInput")
    harm_d = nc.dram_tensor("harm", [4 * 63, 512], F32, kind="ExternalOutput")
    nz_d = nc.dram_tensor("nz", [FR, int(NOFF[6])], F32, kind="ExternalOutput")


    with TileContext(nc) as tc:
        with tc.tile_pool(name="const", bufs=1) as cp, \
             tc.tile_pool(name="work", bufs=2) as wp, \
             tc.tile_pool(name="hot", bufs=3) as hot, \
             tc.tile_pool(name="phF", bufs=3, space="PSUM") as phF, \
             tc.tile_pool(name="phA", bufs=3, space="PSUM") as phA, \
             tc.tile_pool(name="phH", bufs=2, space="PSUM") as phH:

            def psF(p0, f0):
                t = phF.tile([128, 512], F32, tag="phF", name="psF")
                return t[0:p0, 0:f0]

            def psA(p0, f0):
                t = phA.tile([128, 512], F32, tag="phA", name="psA")
                return t[0:p0, 0:f0]

            def psH(p0, f0):
                t = phH.tile([128, 512], F32, tag="phH", name="psH")
                return t[0:p0, 0:f0]

            mega = cp.tile([128, MEGA_COLS], F32, tag="mega")
            _nsplit = 6
            _cut = [MEGA_COLS * i // _nsplit for i in range(_nsplit + 1)]
            for _i in range(_nsplit):
                nc.gpsimd.dma_start(out=mega[:, _cut[_i]:_cut[_i + 1]],
                                    in_=d['mega'][:, _cut[_i]:_cut[_i + 1]])

            ct = {}
            for name, (r, o, cd) in MEGA_OFF.items():
                ct[name] = mega[0:r, o:o + cd]
            for name in ('selstrip', 'negI', 'basA0', 'basA1', 'basAS'):
                r, o, cd = MEGA_OFF[name]
                t = cp.tile([r, cd], BF16, tag=f"bf_{name}")
                nc.gpsimd.dma_start(out=t, in_=d['mega'][0:r, o:o + cd])
                ct[f'{name}_bf'] = t
            for name in ('bas0', 'bas1', 'basS'):
                r, o, cd = MEGA_OFF[name]
                t = cp.tile([r, cd], F32R, tag=f"r_{name}")
                nc.gpsimd.dma_start(out=t, in_=d['mega'][0:r, o:o + cd])
                ct[f'{name}_r'] = t

            def w64s(name):
                i = W64_IDX[name]
                return ct['w64'][:, i * C:(i + 1) * C]

            def b64s(name):
                return ct['bias64'][:, B64_IDX[name]:B64_IDX[name] + 1]

            ident64 = w64s('ident')

            xT = cp.tile([C + 1, BL], F32, tag="xT")
            nc.sync.dma_start(out=xT, in_=d['xT'][:, :])

            # ---------------- frontend (as baseline)
            h = wp.tile([C, 16], F32, tag="h0")
            for t in range(4):
                pt = psF(C, BL)
                nc.tensor.matmul(out=pt, lhsT=ct['wlin'][:, t * C:(t + 1) * C],
                                 rhs=xT, start=True, stop=True)
                nc.vector.tensor_copy(out=h.rearrange("c (b t) -> c b t", t=4)[:, :, t],
                                      in_=pt)
            for i, n in enumerate((4, 8, 16)):
                pt1 = psF(BL * n, C)
                nc.tensor.transpose(out=pt1, in_=h, identity=ident64)
                t1 = wp.tile([BL * n, C], F32, tag=f"fe_t1_{i}")
                nc.vector.tensor_copy(out=t1, in_=pt1)
                pt2 = psF(BL * 2 * n, C)
                nc.tensor.matmul(out=pt2, lhsT=ct[f'ubd{n}'], rhs=t1, start=True,
                                 stop=True)
                t2 = wp.tile([BL * 2 * n, C], F32, tag=f"fe_t2_{i}")
                nc.vector.tensor_copy(out=t2, in_=pt2)
                pt3 = psF(C, BL * 2 * n)
                nc.tensor.transpose(out=pt3, in_=t2,
                                    identity=ct['ident128'][0:BL * 2 * n, 0:BL * 2 * n])
                hu = wp.tile([C, BL * 2 * n], F32, tag=f"fe_hu_{i}")
                nc.vector.tensor_copy(out=hu, in_=pt3)
                m = 2 * n
                hu3 = hu.rearrange("c (b t) -> c b t", b=BL)
                pc = psF(C, BL * m).rearrange("c (b t) -> c b t", b=BL)
                nc.tensor.matmul(out=pc[:, :, :], lhsT=w64s(f'up{i}d1'), rhs=hu3[:, :, :],
                                 start=True, stop=False)
                nc.tensor.matmul(out=pc[:, :, 1:m], lhsT=w64s(f'up{i}d0'),
                                 rhs=hu3[:, :, 0:m - 1], start=False, stop=False)
                nc.tensor.matmul(out=pc[:, :, 0:m - 1], lhsT=w64s(f'up{i}d2'),
                                 rhs=hu3[:, :, 1:m], start=False, stop=True)
                h = wp.tile([C, BL * m], F32, tag=f"fe_h_{i}")
                nc.scalar.activation(out=h.rearrange("c (b t) -> c b t", b=BL), in_=pc,
                                     func=AF.Prelu, bias=b64s(f'up{i}'), scale=1.0,
                                     alpha=0.2)
            h3 = h.rearrange("c (b t) -> c b t", b=BL)
            pf = psF(C, BL * 32).rearrange("c (b t) -> c b t", b=BL)
            nc.tensor.matmul(out=pf[:, :, :], lhsT=w64s('find1'), rhs=h3[:, :, :],
                             start=True, stop=False)
            nc.tensor.matmul(out=pf[:, :, 1:32], lhsT=w64s('find0'), rhs=h3[:, :, 0:31],
                             start=False, stop=False)
            nc.tensor.matmul(out=pf[:, :, 0:31], lhsT=w64s('find2'), rhs=h3[:, :, 1:32],
                             start=False, stop=True)
            hfin = cp.tile([C, 128], F32, tag="hfin")
            nc.vector.tensor_scalar(out=hfin.rearrange("c (b t) -> c b t", b=BL),
                                    in0=pf, scalar1=b64s('fin'), scalar2=None,
                                    op0=ALU.add)

            # ---------------- per-band setup
            nTs_all = {}
            for k in range(6):
                if k < 5:
                    nT = wp.tile([SPF[k], FR], F32, tag=f"nT{k}", name="nT", bufs=1)
                    nc.sync.dma_start(out=nT, in_=d[f'noiseT{k}'][:, :])
                    nTs_all[k] = (nT,)
                else:
                    nT0 = wp.tile([128, FR], F32, tag="nT50", bufs=1)
                    nT1 = wp.tile([128, FR], F32, tag="nT51", bufs=1)
                    nc.sync.dma_start(out=nT0, in_=d['noiseT5'][0:128, :])
                    nc.sync.dma_start(out=nT1, in_=d['noiseT5'][128:256, :])
                    nTs_all[5] = (nT0, nT1)
            def noise_branch(k):
                bs = BAND_SIZES[k]
                spf = SPF[k]
                    zf3 = zfas[k][0:C, :].rearrange("c (b t) -> c b t", b=BL)
                    zrep = zf3.unsqueeze(-1).broadcast_to([C, BL, 32, 2])
                    pn = psA(C, FR)
                    nc.tensor.matmul(out=pn, lhsT=w64s(f'nup{k}'), rhs=zrep,
                                         start=True, stop=True)
                    naug = wp.tile([C + 1, FR], F32, tag="naug")
                    nc.scalar.activation(out=naug[0:C, :], in_=pn, func=AF.Prelu,
                                             bias=b64s(f'nup{k}'), scale=1.0, alpha=0.2)
                    nc.vector.memset(naug[C:C + 1, :], 1.0)

                    if k < 5:
                        nT = nTs_all[k][0]
                        pcA = psH(spf, FR)
                        nc.tensor.matmul(out=pcA, lhsT=ct[f'wc{k}'], rhs=naug,
                                             start=True, stop=True)
                        chat = wp.tile([spf, FR], F32, tag="chat")
                        nc.scalar.copy(out=chat, in_=pcA)
                        psp = psH(spf, FR)
                        nc.tensor.matmul(out=psp, lhsT=ct[f'ft{k}'], rhs=nT,
                                             start=True, stop=True)
                        sA = wp.tile([spf, FR], F32, tag="sA")
                        nc.vector.tensor_tensor(out=sA, in0=chat, in1=psp, op=ALU.mult)
                        sAs = [sA]
                    else:
                        nT0, nT1 = nTs_all[5]
                        sAs = []
                        for half, wch in ((0, 'wc5a'), (1, 'wc5b')):
                            pcA = psH(128, FR)
                            nc.tensor.matmul(out=pcA, lhsT=ct[wch], rhs=naug,
                                                 start=True, stop=True)
                            chat = wp.tile([128, FR], F32, tag=f"chat5{half}")
                            nc.scalar.copy(out=chat, in_=pcA)
                            psp = psH(128, FR)
                            nc.tensor.matmul(out=psp,
                                                 lhsT=ct['ft5_0'][:, 128 * half:128 * (half + 1)],
                                                 rhs=nT0, start=True, stop=False)
                            nc.tensor.matmul(out=psp,
                                                 lhsT=ct['ft5_1'][:, 128 * half:128 * (half + 1)],
                                                 rhs=nT1, start=False, stop=True)
                            sA = wp.tile([128, FR], F32, tag=f"sA5{half}")
                            nc.vector.tensor_tensor(out=sA, in0=chat, in1=psp, op=ALU.mult)
                            sAs.append(sA)

                    for fg in range(2):
                        pnz = psA(128, spf)
                        if k < 5:
                            nc.tensor.matmul(out=pnz,
                                                 lhsT=sAs[0][:, 128 * fg:128 * (fg + 1)],
                                                 rhs=ct[f'ir{k}'], start=True, stop=True)
                        else:
                            nc.tensor.matmul(out=pnz,
                                                 lhsT=sAs[0][:, 128 * fg:128 * (fg + 1)],
                                                 rhs=ct['ir5_0'], start=True, stop=False)
                            nc.tensor.matmul(out=pnz,
                                                 lhsT=sAs[1][:, 128 * fg:128 * (fg + 1)],
                                                 rhs=ct['ir5_1'], start=False, stop=True)
                        nzs = wp.tile([128, spf], F32, tag="nzs")
                        nc.scalar.copy(out=nzs, in_=pnz)
                        nc.sync.dma_start(
                            out=nz_d[128 * fg:128 * (fg + 1),
                                         int(NOFF[k]):int(NOFF[k]) + spf],
                            in_=nzs)


            stacks = {}
            astacks = {}
            zfas = {}
            srcs = {}
            ampgs = {}
            LFS = [0.05 if bs == 512 else 0.01 for bs in BAND_SIZES]

            # pass 1: residual stacks, layer-interleaved across bands
            zs = {k: hfin for k in range(6)}
            for j in range(4):
                for k in range(6):
                    pz = psF(C, 128)
                    nc.tensor.matmul(out=pz, lhsT=w64s(f't{k}{j}'), rhs=zs[k],
                                     start=True, stop=True)
                    z = wp.tile([C, 128], F32, tag=f"z{k}_{j % 2}", name="z", bufs=1)
                    nc.scalar.activation(out=z, in_=pz, func=AF.Prelu,
                                         bias=b64s(f't{k}{j}'), scale=1.0, alpha=0.2)
                    zs[k] = z
            for k in range(6):
                pz = psF(C, 128)
                nc.tensor.matmul(out=pz, lhsT=w64s(f'bf{k}'), rhs=zs[k],
                                 start=True, stop=True)
                zfa = wp.tile([C + 1, 128], F32, tag=f"zfa{k}", name="zfa", bufs=1)
                nc.vector.tensor_scalar(out=zfa[0:C, :], in0=pz,
                                        scalar1=b64s(f'bf{k}'), scalar2=None,
                                        op0=ALU.add)
                nc.vector.memset(zfa[C:C + 1, :], 1.0)
                zfas[k] = zfa

            # pass 2: freq/amp grids
            for k in range(6):
                zfa = zfas[k]
                pgF = psH(N_OSC, 128)
                for b in range(BL):
                    nc.tensor.matmul(out=pgF[:, 32 * b:32 * (b + 1)],
                                     lhsT=zfa[:, 32 * b:32 * (b + 1)],
                                     rhs=ct[f'wfrq{k}'], start=True, stop=True)
                sig = wp.tile([N_OSC, 128], F32, tag=f"sig{k}", name="sig", bufs=1)
                nc.scalar.activation(out=sig, in_=pgF, func=AF.Sigmoid, scale=1.0)
                src = wp.tile([64, 128], F32, tag=f"src{k}", name="src", bufs=1)
                nc.vector.tensor_scalar(out=src[0:32, :], in0=sig,
                                        scalar1=float((1.0 - LFS[k]) / 2.0),
                                        scalar2=float(LFS[k] / 2.0),
                                        op0=ALU.mult, op1=ALU.add)
                srcs[k] = src
                pgA = psH(N_OSC, 128)
                for b in range(BL):
                    nc.tensor.matmul(out=pgA[:, 32 * b:32 * (b + 1)],
                                     lhsT=zfa[:, 32 * b:32 * (b + 1)],
                                     rhs=ct[f'wamp{k}'], start=True, stop=True)
                ampg = wp.tile([N_OSC, 128], F32, tag=f"ampg{k}", name="ampg", bufs=1)
                nc.scalar.activation(out=ampg, in_=pgA, func=AF.Abs, scale=1.0)
                ampgs[k] = ampg

            # pass 3: frame carries
            Srs = {}
            for k in range(6):
                pS = psH(N_OSC, 128)
                nc.tensor.matmul(out=pS, lhsT=ct[f'M{k}'], rhs=srcs[k][0:32, :],
                                 start=True, stop=True)
                rndS = wp.tile([N_OSC, 128], F32, tag=f"rndS{k}", name="rndS", bufs=1)
                nc.vector.tensor_scalar(out=rndS, in0=pS, scalar1=MAGIC,
                                        scalar2=MAGIC, op0=ALU.add,
                                        op1=ALU.subtract)
                Sr = wp.tile([N_OSC, 128], F32, tag=f"Sr{k}", name="Sr", bufs=1)
                nc.vector.tensor_tensor(out=Sr, in0=pS, in1=rndS, op=ALU.subtract)
                Srs[k] = Sr
            for k in range(6):
                pP = psH(N_OSC, 128)
                nc.tensor.matmul(out=pP, lhsT=ct['ut'], rhs=Srs[k], start=True,
                                 stop=True)
                rndP = wp.tile([N_OSC, 128], F32, tag=f"rndP{k}", name="rndP", bufs=1)
                nc.vector.tensor_scalar(out=rndP, in0=pP, scalar1=MAGIC,
                                        scalar2=MAGIC, op0=ALU.add,
                                        op1=ALU.subtract)
                nc.vector.tensor_tensor(out=srcs[k][32:64, :], in0=pP, in1=rndP,
                                        op=ALU.subtract)

            # pass 4: per-band stacks + noise
            for k, bs in enumerate(BAND_SIZES):
                nch = NCH[k]
                src = srcs[k]
                ampg = ampgs[k]
                if k == 0:
                    stk = cp.tile([128, 128], F32, tag="stk0")
                    pb = psF(128, 128)
                    nc.tensor.matmul(out=pb, lhsT=ct['pi0'], rhs=src, start=True,
                                     stop=True)
                    nc.scalar.copy(out=stk, in_=pb)
                elif k == 1:
                    stk = cp.tile([128, 128], F32, tag="stk1")
                    pb = psF(128, 128)
                    nc.tensor.matmul(out=pb, lhsT=ct['pi1'], rhs=src, start=True,
                                     stop=True)
                    nc.scalar.copy(out=stk, in_=pb)
                else:
                    stk = cp.tile([128, (nch // 2) * 128], F32, tag=f"stk{k}")
                    nblkp = nch // 2
                    for j0 in range(0, nblkp, 4):
                        jn = min(4, nblkp - j0)
                        pb = psF(128, 128 * jn)
                        for jj in range(jn):
                            nc.tensor.matmul(out=pb[:, 128 * jj:128 * (jj + 1)],
                                             lhsT=ct[f'pip{k}_{j0 + jj}'], rhs=src,
                                             start=True, stop=True)
                        rnd = wp.tile([128, 512], F32, tag="rndB")
                        nc.vector.tensor_scalar(out=rnd[:, 0:128 * jn], in0=pb,
                                                scalar1=MAGIC, scalar2=MAGIC,
                                                op0=ALU.add, op1=ALU.subtract)
                        nc.vector.tensor_tensor(
                            out=stk[:, 128 * j0:128 * (j0 + jn)],
                            in0=pb, in1=rnd[:, 0:128 * jn], op=ALU.subtract)
                if k == 0:
                    stkr = cp.tile([128, 128], F32R, tag="stkr0", name="stkr")
                elif k == 1:
                    stkr = cp.tile([128, 128], F32R, tag="stkr1", name="stkr")
                else:
                    stkr = cp.tile([128, (nch // 2) * 128], F32R, tag=f"stkr{k}",
                                   name="stkr")
                nc.gpsimd.dma_start(out=stkr, in_=stk)
                stacks[k] = stkr

                if k == 0:
                    ast = cp.tile([96, 128], BF16, tag="ast0")
                    pb = psF(96, 128)
                    nc.tensor.matmul(out=pb, lhsT=ct['pia0'], rhs=ampg, start=True,
                                     stop=True)
                    nc.scalar.copy(out=ast, in_=pb)
                elif k == 1:
                    ast = cp.tile([128, 128], BF16, tag="ast1")
                    pb = psF(128, 128)
                    nc.tensor.matmul(out=pb, lhsT=ct['pia1'], rhs=ampg, start=True,
                                     stop=True)
                    nc.scalar.copy(out=ast, in_=pb)
                else:
                    nblk = (nch + 2) // 3
                    ast = cp.tile([96, nblk * 128], BF16, tag=f"ast{k}")
                    for j0 in range(0, nblk, 4):
                        jn = min(4, nblk - j0)
                        pb = psF(96, 128 * jn)
                        for jj in range(jn):
                            nc.tensor.matmul(out=pb[:, 128 * jj:128 * (jj + 1)],
                                             lhsT=ct[f'piam{k}_{j0 + jj}'], rhs=ampg,
                                             start=True, stop=True)
                        nc.scalar.copy(out=ast[:, 128 * j0:128 * (j0 + jn)], in_=pb)
                astacks[k] = ast
                noise_branch(k)

            # ---------------- chunk loop (per-chunk, deep software pipeline)
            chunks = []
            for k in (1, 2, 3, 4, 5, 0):
                nch = NCH[k]
                for gstart in range(0, nch, 8):
                    gs = min(8, nch - gstart)
                    for cc in range(gs):
                        chunks.append((k, gstart + cc, cc, gs))
            Nc = len(chunks)
            state = {}

            def stage_a(i):
                k, c, cc, gs = chunks[i]
                ppt = phF.tile([128, 512], F32, tag="phF", name="ppt")
                if k == 0:
                    nc.tensor.matmul(out=ppt, lhsT=stacks[0], rhs=ct['bas0_r'],
                                     start=True, stop=False)
                elif k == 1:
                    nc.tensor.matmul(out=ppt, lhsT=stacks[1][64 * c:64 * c + 64],
                                     rhs=ct['bas1_r'][64 * c:64 * c + 64],
                                     start=True, stop=False)
                else:
                    j, par = c // 2, c % 2
                    nc.tensor.matmul(
                        out=ppt,
                        lhsT=stacks[k][64 * par:64 * par + 48,
                                       128 * j:128 * (j + 1)],
                        rhs=ct['basS_r'][64 * par:64 * par + 48],
                        start=True, stop=False)
                pat = phA.tile([128, 512], F32, tag="phA", name="pat")
                if k == 0:
                    nc.tensor.matmul(out=pat, lhsT=astacks[0],
                                     rhs=ct['basA0_bf'], start=True, stop=True)
                elif k == 1:
                    nc.tensor.matmul(out=pat, lhsT=astacks[1][64 * c:64 * c + 48],
                                     rhs=ct['basA1_bf'][64 * c:64 * c + 48],
                                     start=True, stop=True)
                else:
                    ja, ma = c // 3, c % 3
                    nc.tensor.matmul(
                        out=pat,
                        lhsT=astacks[k][32 * ma:32 * ma + 32,
                                        128 * ja:128 * (ja + 1)],
                        rhs=ct['basAS_bf'][32 * ma:32 * ma + 32],
                        start=True, stop=True)
                ntile = hot.tile([128, 512], BF16, tag="ntile")
                nc.vector.tensor_scalar(out=ntile, in0=ppt, scalar1=MAGIC,
                                        scalar2=MAGIC, op0=ALU.add,
                                        op1=ALU.subtract)
                if i % 2 == 0:
                    pat_sb = hot.tile([128, 512], BF16, tag="pat_sb")
                    nc.scalar.copy(out=pat_sb, in_=pat)
                else:
                    pat_sb = None
                state[i] = (ppt, pat, ntile, pat_sb)

            def stage_b(i):
                ppt, pat, ntile, pat_sb = state[i]
                nc.tensor.matmul(out=ppt, lhsT=ct['negI_bf'], rhs=ntile,
                                 start=False, stop=True)
                s = hot.tile([128, 512], BF16, tag="sin_t")
                nc.scalar.activation(out=s, in_=ppt, func=AF.Sin, scale=TWO_PI)
                prod = hot.tile([128, 512], BF16, tag="prod_t")
                nc.vector.tensor_tensor(out=prod, in0=s,
                                        in1=pat_sb if pat_sb is not None else pat,
                                        op=ALU.mult)
                state[i] = prod

            hpt_cur = [None]

            def stage_c(i):
                k, c, cc, gs = chunks[i]
                prod = state.pop(i)
                if cc == 0:
                    hpt_cur[0] = phH.tile([128, 512], F32, tag="phH", name="hpt")
                hpt = hpt_cur[0]
                nc.tensor.matmul(out=hpt,
                                 lhsT=ct['selstrip_bf'][:, 128 - 4 * cc:256 - 4 * cc],
                                 rhs=prod, start=(cc == 0), stop=(cc == gs - 1))
                if cc == gs - 1:
                    hsb = wp.tile([32, 512], F32, tag="hsb")
                    nc.scalar.copy(out=hsb[0:4 * gs], in_=hpt[0:4 * gs])
                    g0 = CHUNK_BASE[k] + c - (gs - 1)
                    nc.sync.dma_start(
                        out=harm_d[4 * int(g0):4 * int(g0) + 4 * gs, :],
                        in_=hsb[0:4 * gs])

            for i in range(Nc + 4):
                if i < Nc:
                    stage_a(i)
                if 2 <= i < Nc + 2:
                    stage_b(i - 2)
                if 4 <= i < Nc + 4:
                    stage_c(i - 4)

    nc.finalize()
    return nc


# ---------------------------------------------------------------- host glue
def _prep_inputs(inputs):
    inp = {k: np.asarray(v, np.float32) for k, v in inputs.items()}
    shared = _build_shared(inp)
    in_maps = []
    for core in range(NCORE):
        m = dict(shared)
        sl = slice(core * BL, (core + 1) * BL)
        m['xT'] = np.concatenate([inp['x'][sl].T, np.ones((1, BL), np.float32)],
                                 axis=0)
        for k in range(6):
            nT = inp[f'noise_{k}'][sl].reshape(FR, SPF[k]).T
            m[f'noiseT{k}'] = np.ascontiguousarray(nT, dtype=np.float32)
        in_maps.append(m)
    return in_maps


def kernel(**inputs):
    if 'nc' not in _nc_cache:
        _nc_cache['nc'] = _build_nc()
    nc = _nc_cache['nc']
    in_maps = _prep_inputs(inputs)
    res = run_bass_kernel_spmd(nc, in_maps, list(range(NCORE)))
    out = np.zeros((B, TOTAL), np.float32)
    for core in range(NCORE):
        r = res.results[core]
        harm = np.asarray(r['harm'], np.float32).reshape(63, BL, 512)
        nz = np.asarray(r['nz'], np.float32).reshape(BL, NNF, int(NOFF[6]))
        for k, bs in enumerate(BAND_SIZES):
            nch = NCH[k]
            hb = harm[CHUNK_BASE[k]:CHUNK_BASE[k] + nch]  # [nch, BL, 512]
            hb = hb.transpose(1, 0, 2).reshape(BL, bs)
            out[core * BL:(core + 1) * BL,
                HARM_OFF[k]:HARM_OFF[k] + bs] = hb
            nzb = nz[:, :, int(NOFF[k]):int(NOFF[k]) + SPF[k]].reshape(BL, bs)
            out[core * BL:(core + 1) * BL, NZ_OFF[k]:NZ_OFF[k] + bs] = nzb
    return out.astype(np.float32)


if __name__ == "__main__":
    import reference
    inp = reference.setup_inputs()
    out = kernel(**{k: np.asarray(v) for k, v in inp.items()})
    print("out", out.shape, out.dtype)
